# revision 22
# baseline (speedup 1.0000x reference)
"""Trainium2 Bass kernel for a transformer decoder block (self-attn + cross-attn + MLP).

Sharding: data-parallel over (batch, strided query blocks) = 8 shards, no
collectives. Core (b, h) owns query blocks {h, h+2, h+4, h+6} of 128 tokens;
the host permutes tokens so own queries come first. The strided split makes
the causal block structure identical on every core: query block i attends to
own key chunks j<=i and other-half key chunks j<=i, with only the two
diagonal blocks needing masks (a constant triangular mask and a per-core
all-or-nothing flag), applied by tiny PE matmuls into the score PSUM.

Layout: transposed activations [feature partition, token free]. Weights are
pre-tiled on the host to [128, ...] fp8 (e4m3, pow2-scaled) so every weight
DMA is a contiguous 2D copy. Big GEMMs run fp8 DoubleRow (2 x 128 contraction
per pass); QK stays bf16. Softmax: exp(s*scale + ln8) gives 8*P directly in
fp8; denominators come from a ones column in the extended V and are folded
into a per-head reciprocal+broadcast epilogue (no cross-engine round trips).
LayerNorm stats (ones-matmuls + squares) chase the producing projection; the
normalize is pipelined in token blocks so matmuls start while later blocks
normalize.
"""

import sys

sys.path.insert(0, "/opt/trn_rl_repo")

import math
import numpy as np
import ml_dtypes

import concourse.bass as bass
import concourse.bacc as bacc
import concourse.mybir as mybir
from concourse import tile
from concourse.bass_utils import run_bass_kernel_spmd

dt = mybir.dt
AF = mybir.ActivationFunctionType
PM = mybir.MatmulPerfMode

# Problem dims (hardcoded per contest contract)
B, T, D, H, HD = 4, 1024, 1024, 16, 64
S, DE, DM = 576, 768, 4096
TQ = T // 2          # queries per core
DC = D // 128        # feature chunks (8)
EC = DE // 128       # enc feature chunks (6)
MC = DM // 128       # mlp hidden chunks (32)
SCALE = HD ** -0.5
EPS = 1e-5
A_X = 16.0           # xhat / enc fp8 scale
A_P = 8.0            # softmax numerator scale (via exp bias ln A_P)
LN8 = math.log(A_P)
NEG = -1.0e7         # additive mask value

F8NP = ml_dtypes.float8_e4m3
BFNP = ml_dtypes.bfloat16
F8 = dt.float8e4

# which GEMMs use fp8 DoubleRow (others bf16): knobs for accuracy fallback
F8_STAGES = {"qkv", "v", "proj", "qc", "kc", "vc", "av", "avc", "out", "mlp1", "mlp2"}

_cached = {}


def _pow2_scale(w):
    am = float(np.abs(w).max())
    return 2.0 ** int(np.floor(np.log2(240.0 / am)))


def _tile_rows(w, nrow, nkc, m):
    """w [nkc*128, nrow*m] -> [128, nrow*nkc*m] with [p, r, kc, m] order."""
    K, N = w.shape
    assert K == nkc * 128 and N == nrow * m
    wt = w.reshape(nkc, 128, nrow, m).transpose(1, 2, 0, 3)
    return np.ascontiguousarray(wt.reshape(128, nrow * nkc * m))


def _prepare_inputs(x, enc, tgt_key_padding_mask, enc_padding_mask,
                    ln1_w, ln1_b, qkv_w, qkv_b, proj_w, proj_b,
                    ln2_w, ln2_b, q_w, q_b, k_w, k_b, v_w, v_b, out_w, out_b,
                    ln3_w, ln3_b, mlp1_w, mlp1_b, mlp2_w, mlp2_b):
    f32 = np.float32
    asf = lambda a: np.asarray(a, dtype=f32)
    x, enc = asf(x), asf(enc)
    ln1_w, ln1_b, ln2_w, ln2_b, ln3_w, ln3_b = map(asf, (ln1_w, ln1_b, ln2_w, ln2_b, ln3_w, ln3_b))
    qkv_w, qkv_b, proj_w, proj_b = map(asf, (qkv_w, qkv_b, proj_w, proj_b))
    q_w, q_b, k_w, k_b, v_w, v_b, out_w, out_b = map(
        asf, (q_w, q_b, k_w, k_b, v_w, v_b, out_w, out_b))
    mlp1_w, mlp1_b, mlp2_w, mlp2_b = map(asf, (mlp1_w, mlp1_b, mlp2_w, mlp2_b))

    # host-side folds (as baseline): LN affine into weights, k-bias dropped
    # (softmax-invariant), v-biases folded into the following projection bias.
    wqkv_f = qkv_w * ln1_w[:, None]
    bqkv = qkv_b + qkv_w.T @ ln1_b
    b_q = bqkv[0:D]
    b_v = bqkv[2 * D:3 * D]
    bprojf = proj_b + proj_w.T @ b_v
    wqf = q_w * ln2_w[:, None]
    bqcf = q_b + q_w.T @ ln2_b
    boutf = out_b + out_w.T @ v_b
    wm1f = mlp1_w * ln3_w[:, None]
    bm1f = mlp1_b + mlp1_w.T @ ln3_b

    # per-tensor pow2 scales; baked into the compiled program's drain scales
    wq_ = wqkv_f[:, 0:D]; wk_ = wqkv_f[:, D:2 * D]; wv_ = wqkv_f[:, 2 * D:3 * D]
    sc = {
        "q": _pow2_scale(wq_), "k": _pow2_scale(wk_), "v": _pow2_scale(wv_),
        "p": _pow2_scale(proj_w), "qc": _pow2_scale(wqf), "kc": _pow2_scale(k_w),
        "vc": _pow2_scale(v_w), "o": _pow2_scale(out_w),
        "m1": _pow2_scale(wm1f), "m2": _pow2_scale(mlp2_w),
    }
    c8 = lambda w, s: np.ascontiguousarray((w * s).astype(F8NP))
    shared = {
        "wq8": c8(_tile_rows(wq_, DC, DC, 128), sc["q"]),
        "wk8": c8(_tile_rows(wk_, DC, DC, 128), sc["k"]),
        "wv8": c8(_tile_rows(wv_, 2, DC, 512), sc["v"]),
        "wp8": c8(_tile_rows(proj_w, DC, DC, 128), sc["p"]),
        "wqc8": c8(_tile_rows(wqf, DC, DC, 128), sc["qc"]),
        "wkc8": c8(_tile_rows(k_w, DC, EC, 128), sc["kc"]),
        "wvc8": c8(_tile_rows(v_w, 2, EC, 512), sc["vc"]),
        "wo8": c8(_tile_rows(out_w, DC, DC, 128), sc["o"]),
        "wm18": c8(_tile_rows(wm1f, MC, DC, 128), sc["m1"]),
        "wm28": c8(_tile_rows(mlp2_w, DC, MC, 128), sc["m2"]),
    }
    col = lambda v: np.ascontiguousarray(v.reshape(-1, 1).astype(f32))
    shared.update({
        "bq": col(b_q), "bproj": col(bprojf), "bqc": col(bqcf),
        "bout": col(boutf), "bm1": col(bm1f), "bm2": col(mlp2_b),
    })
    # encT pre-tiled fp8*A_X: [128, EC*S]
    encT = enc.transpose(0, 2, 1)  # [B, DE, S]

    # mask tiles [128, 3*128] bf16: [triT | flag | identity]
    # triT[i, j] = M[j, i] where M[key, q] = 0 if key<=q else NEG (same block)
    tri = np.where(np.arange(128)[:, None] <= np.arange(128)[None, :], 0.0, NEG)
    triT = tri.T.astype(BFNP)
    ident = np.eye(128, dtype=BFNP)

    in_maps, metas = [], []
    for c in range(8):
        b, h = c // 2, c % 2
        own_blocks = np.arange(h, 8, 2)
        other_blocks = np.arange(1 - h, 8, 2)
        own = (own_blocks[:, None] * 128 + np.arange(128)[None, :]).reshape(-1)
        other = (other_blocks[:, None] * 128 + np.arange(128)[None, :]).reshape(-1)
        perm = np.concatenate([own, other])
        xT_np = np.ascontiguousarray(x[b][perm].T)  # [D, T] own-first
        enc8 = np.ascontiguousarray(
            (encT[b].reshape(EC, 128, S).transpose(1, 0, 2).reshape(128, EC * S)
             * A_X).astype(F8NP))
        flag = np.full((128, 128), NEG if h == 0 else 0.0, dtype=BFNP)
        mask3 = np.ascontiguousarray(np.concatenate([triT, flag, ident], axis=1))
        im = dict(shared)
        im["xT"] = xT_np
        im["encT8"] = enc8
        im["mask3"] = mask3
        in_maps.append(im)
        metas.append((b, own_blocks))
    return in_maps, metas, sc


def _build_body(nc, tc, P, sc):
    from contextlib import ExitStack
    f32, f32r, bf16 = dt.float32, dt.float32r, dt.bfloat16
    ctx = ExitStack()
    with ctx:
        const = ctx.enter_context(tc.tile_pool(name="const", bufs=1))
        rows = ctx.enter_context(tc.tile_pool(name="rows", bufs=2))
        rows1 = ctx.enter_context(tc.tile_pool(name="rows1", bufs=2))
        bcp = ctx.enter_context(tc.tile_pool(name="bcp", bufs=2))
        sb_sm = ctx.enter_context(tc.tile_pool(name="sb_sm", bufs=3))
        ps = ctx.enter_context(tc.tile_pool(name="ps", bufs=2, space="PSUM"))
        resp = ctx.enter_context(tc.tile_pool(name="resp", bufs=1))

        ones32 = const.tile([128, 1], f32, tag="ones32")
        nc.vector.memset(ones32[:, :], 1.0)
        ones = const.tile([128, 1], f32r, tag="ones")
        nc.scalar.activation(ones[:, :], ones32[:, :], AF.Copy)
        ones_bf = const.tile([128, 1], bf16, tag="ones_bf")
        nc.vector.memset(ones_bf[:, :], 1.0)
        eps2 = const.tile([1, 1], f32, tag="eps2")
        nc.vector.memset(eps2[:, :], EPS / (A_X * A_X))
        ln8_t = const.tile([128, 1], f32, tag="ln8")
        nc.vector.memset(ln8_t[:, :], LN8)
        mask_t = const.tile([128, 384], bf16, tag="mask3")
        nc.sync.dma_start(out=mask_t[:, :], in_=P["mask3"][:, :])
        triT, flagT, ident = mask_t[:, 0:128], mask_t[:, 128:256], mask_t[:, 256:384]

        def load_bias(drh, nr, tag):
            t = const.tile([128, nr], f32, tag=tag, name="b_" + tag)
            nc.sync.dma_start(out=t.rearrange("p (r one) -> p r one", one=1),
                              in_=drh.rearrange("(r p) one -> p r one", p=128))
            return t

        bq_t = load_bias(P["bq"], DC, "bq")
        bp_t = load_bias(P["bproj"], DC, "bproj")
        bqc_t = load_bias(P["bqc"], DC, "bqc")
        bo_t = load_bias(P["bout"], DC, "bout")
        bm1_t = load_bias(P["bm1"], MC, "bm1")
        bm2_t = load_bias(P["bm2"], DC, "bm2")

        # residual stream (bf16) + cross tensors spanning many phases
        x2T = resp.tile([128, DC * TQ], bf16, tag="x2T")
        x3T = resp.tile([128, DC * TQ], bf16, tag="x3T")
        qcT = resp.tile([128, DC * TQ], bf16, tag="qcT")
        kcT = resp.tile([128, DC * S], bf16, tag="kcT")
        vcext = resp.tile([128, 5 * H * 65], F8, tag="vcext")
        caT = resp.tile([128, DC * TQ], F8, tag="caT")

        # ---- LayerNorm helpers (transposed layout, stats via ones-matmul) ----
        def ln_chain(sts, stq, c0, W, rb_t, nb_t):
            R = rows1.tile([1, 2048], f32, tag="lnrow", bufs=1)
            t1n, t2 = R[0:1, 0:W], R[0:1, 512:512 + W]
            t5, msq = R[0:1, 1024:1024 + W], R[0:1, 1536:1536 + W]
            nc.vector.tensor_scalar_mul(t1n, sts[0:1, c0:c0 + W], -1.0 / D)
            nc.vector.tensor_scalar_mul(t2, stq[0:1, c0:c0 + W], 1.0 / D)
            nc.vector.tensor_mul(msq, t1n, t1n)
            nc.vector.tensor_sub(t2, t2, msq)
            nc.scalar.activation(t5, t2, AF.Abs_reciprocal_sqrt,
                                 bias=eps2[0:1, 0:1], scale=1.0 / (A_X * A_X))
            nc.vector.tensor_mul(t1n, t1n, t5)        # -mean * A_X * rstd
            nc.gpsimd.partition_broadcast(rb_t[:, 0:W], t5)
            nc.gpsimd.partition_broadcast(nb_t[:, 0:W], t1n)

        gp_mul = (nc.gpsimd.tensor_mul if hasattr(nc.gpsimd, "tensor_mul")
                  else nc.vector.tensor_mul)

        def ln_norm_chunk(dst, xsrc, rb_t, nb_t, W):
            xs = rows.tile([128, 512], f32, tag="xs")
            gp_mul(xs[:, 0:W], xsrc, rb_t[:, 0:W])
            nc.vector.tensor_add(dst, xs[:, 0:W], nb_t[:, 0:W])

        # =====================  phase 1-8: LN1, self-attn, proj, LN2, qc  ====
        with tc.tile_pool(name="xtp", bufs=1) as xtp, \
             tc.tile_pool(name="sfp", bufs=1) as sfp:
            xT_t = xtp.tile([128, DC * T], f32r, tag="xT")
            for kc in range(DC):
                nc.sync.dma_start(out=xT_t[:, kc * T:(kc + 1) * T],
                                  in_=P["xT"][kc * 128:(kc + 1) * 128, :])
            xhat1 = sfp.tile([128, DC * T], F8, tag="xhat1")
            kT = sfp.tile([128, DC * T], bf16, tag="kT")
            qT = sfp.tile([128, DC * TQ], bf16, tag="qT")
            vext = sfp.tile([128, 8 * H * 65], F8, tag="vext")
            saT = sfp.tile([128, DC * TQ], F8, tag="saT")

            nc.vector.memset(
                vext.rearrange("p (c e) -> p c e", e=65)[:, :, 64:65], 1.0)
            nc.vector.memset(vcext[64:128, 4 * H * 65:5 * H * 65], 0.0)
            nc.vector.memset(
                vcext.rearrange("p (c e) -> p c e", e=65)[:, 0:4 * H, 64:65], 1.0)
            nc.vector.memset(
                vcext.rearrange("p (c e) -> p c e", e=65)[0:64, 4 * H:5 * H, 64:65], 1.0)

            xh3 = xhat1.rearrange("p (kc t) -> p kc t", t=T)

            with tc.tile_pool(name="ckA", bufs=1) as ckA:
                encT_t = ckA.tile([128, EC * S], F8, tag="encT")
                nc.sync.dma_start(out=encT_t[:, :], in_=P["encT8"][:, :])
                wkcall = ckA.tile([128, DC * EC * 128], F8, tag="wkc")
                nc.sync.dma_start(out=wkcall[:, :], in_=P["wkc8"][:, :])
                wvcall = ckA.tile([128, 2 * EC * 512], F8, tag="wvc")
                nc.sync.dma_start(out=wvcall[:, :], in_=P["wvc8"][:, :])
                encv = encT_t.rearrange("p (ec s) -> p ec s", s=S)

                def emit_kc_unit(r, et):
                    pt = ps.tile([128, 512], f32, tag="mm")
                    wv_ = wkcall.rearrange("p (r ec m) -> p r ec m", r=DC, m=128)
                    for i in range(EC // 2):
                        nc.tensor.matmul(pt[:, 0:288], wv_[:, r, 2 * i:2 * i + 2, :],
                                         encv[:, 2 * i:2 * i + 2, et * 288:(et + 1) * 288],
                                         start=(i == 0), stop=(i == 2),
                                         perf_mode=PM.DoubleRow)
                    nc.scalar.activation(kcT[:, r * S + et * 288: r * S + (et + 1) * 288],
                                         pt[:, 0:288], AF.Identity, bias=0.0,
                                         scale=1.0 / (A_X * sc["kc"]))

                kc_units = [(r, et) for r in range(DC) for et in range(2)]

                # ---- LN1 stats (chasing x DMA) with cross-kc interleaved ----
                with tc.tile_pool(name="wqks", bufs=4) as wqks, \
                     tc.tile_pool(name="pst", bufs=1, space="PSUM") as pst:
                    st = {nm: pst.tile([128, 512], f32, tag="st_" + nm,
                                       name="st_" + nm)
                          for nm in ("s0", "s1", "q0", "q1")}
                    for kc in range(DC):
                        for _ in range(2):
                            if kc_units:
                                emit_kc_unit(*kc_units.pop(0))
                        for tt in range(2):
                            sq = sb_sm.tile([128, 512], f32r, tag="sq2")
                            nc.scalar.activation(
                                sq[:, :],
                                xT_t[:, kc * T + tt * 512: kc * T + tt * 512 + 512],
                                AF.Square)
                            nc.tensor.matmul(st["s%d" % tt][0:1, :], ones[:, :],
                                             xT_t[:, kc * T + tt * 512: kc * T + tt * 512 + 512],
                                             start=(kc == 0), stop=(kc == DC - 1),
                                             skip_group_check=True)
                            nc.tensor.matmul(st["q%d" % tt][0:1, :], ones[:, :],
                                             sq[:, :],
                                             start=(kc == 0), stop=(kc == DC - 1),
                                             skip_group_check=True)
                    while kc_units:
                        emit_kc_unit(*kc_units.pop(0))

                    # ---- LN1 tt0 chain + norm, q/k rows chase ----
                    rb0 = bcp.tile([128, 512], f32, tag="rb")
                    nb0 = bcp.tile([128, 512], f32, tag="nb")
                    ln_chain(st["s0"], st["q0"], 0, 512, rb0, nb0)
                    for kc in range(DC):
                        ln_norm_chunk(xhat1[:, kc * T: kc * T + 512],
                                      xT_t[:, kc * T: kc * T + 512], rb0, nb0, 512)

                    def qk_row(wdram, r, dst, bias, sscale, tcols):
                        wt = wqks.tile([128, DC * 128], F8, tag="wr")
                        nc.sync.dma_start(out=wt[:, :],
                                          in_=wdram[:, r * DC * 128:(r + 1) * DC * 128])
                        wv_ = wt.rearrange("p (kc m) -> p kc m", m=128)
                        pt = ps.tile([128, 512], f32, tag="mm")
                        for i in range(4):
                            nc.tensor.matmul(pt[:, :], wv_[:, 2 * i:2 * i + 2, :],
                                             xh3[:, 2 * i:2 * i + 2, tcols:tcols + 512],
                                             start=(i == 0), stop=(i == 3),
                                             perf_mode=PM.DoubleRow)
                        if bias is None:
                            nc.scalar.activation(dst, pt[:, :], AF.Identity,
                                                 bias=0.0, scale=sscale)
                        else:
                            nc.scalar.activation(dst, pt[:, :], AF.Identity,
                                                 bias=bias, scale=sscale)

                    for r in range(DC):
                        qk_row(P["wq8"], r, qT[:, r * TQ:(r + 1) * TQ],
                               bq_t[:, r:r + 1], 1.0 / (A_X * sc["q"]), 0)
                        qk_row(P["wk8"], r, kT[:, r * T: r * T + 512],
                               None, 1.0 / (A_X * sc["k"]), 0)

                    # ---- LN1 tt1 chain + norm, k-tt1 rows chase ----
                    rb1 = bcp.tile([128, 512], f32, tag="rb")
                    nb1 = bcp.tile([128, 512], f32, tag="nb")
                    ln_chain(st["s1"], st["q1"], 0, 512, rb1, nb1)
                    for kc in range(DC):
                        ln_norm_chunk(xhat1[:, kc * T + 512: kc * T + 1024],
                                      xT_t[:, kc * T + 512: kc * T + 1024], rb1, nb1, 512)
                    for r in range(DC):
                        qk_row(P["wk8"], r, kT[:, r * T + 512: r * T + 1024],
                               None, 1.0 / (A_X * sc["k"]), 512)

                # ---- V units + self-attention heads ----
                vxv = vext.rearrange("p (tk j e) -> p tk j e", tk=8, j=H)
                wvcv = wvcall.rearrange("p (vf ec m) -> p vf ec m", vf=2, m=512)
                vcxv = vcext.rearrange("p (tk j e) -> p tk j e", tk=5, j=H)

                def emit_vc_unit(vf, tokc):
                    npart = 128 if tokc < 4 else 64
                    pv = ps.tile([128, 512], f32, tag="mm")
                    for i in range(EC // 2):
                        nc.tensor.matmul(pv[:npart, :],
                                         encv[:, 2 * i:2 * i + 2, tokc * 128:tokc * 128 + npart],
                                         wvcv[:, vf, 2 * i:2 * i + 2, :],
                                         start=(i == 0), stop=(i == 2),
                                         perf_mode=PM.DoubleRow)
                    nc.vector.tensor_scalar_mul(
                        vcxv[:npart, tokc, 8 * vf:8 * vf + 8, 0:64],
                        pv[:npart].rearrange("p (j d) -> p j d", j=8), 1.0 / sc["vc"])

                with tc.tile_pool(name="wvp", bufs=1) as wvp:
                    wvall = wvp.tile([128, 2 * DC * 512], F8, tag="wv")
                    nc.sync.dma_start(out=wvall[:, :], in_=P["wv8"][:, :])
                    wvv = wvall.rearrange("p (vf kc m) -> p vf kc m", vf=2, m=512)

                    def emit_v_unit(vf, tokc):
                        pv = ps.tile([128, 512], f32, tag="mm")
                        for i in range(4):
                            nc.tensor.matmul(pv[:, :],
                                             xh3[:, 2 * i:2 * i + 2,
                                                 tokc * 128:(tokc + 1) * 128],
                                             wvv[:, vf, 2 * i:2 * i + 2, :],
                                             start=(i == 0), stop=(i == 3),
                                             perf_mode=PM.DoubleRow)
                        nc.vector.tensor_scalar_mul(
                            vxv[:, tokc, 8 * vf:8 * vf + 8, 0:64],
                            pv.rearrange("p (j d) -> p j d", j=8), 1.0 / sc["v"])

                    for tokc in range(8):
                        emit_v_unit(0, tokc)

                    NOFF = [0, 1024, 1792, 2304]
                    vex5 = vext.rearrange("p (g c je) -> p g c je", g=2, c=4)
                    vc_units = [(vf, tokc) for vf in range(2) for tokc in range(5)]

                    with tc.tile_pool(name="pp", bufs=2) as pp, \
                         tc.tile_pool(name="pssc", bufs=2, space="PSUM") as pssc, \
                         tc.tile_pool(name="ps2", bufs=2, space="PSUM") as ps2:
                        def self_head(h):
                            hp, hc = (h % 2) * 64, h // 2
                            Pt = pp.tile([128, 2560], F8, tag="P")
                            av = ps2.tile([65, 512], f32, tag="av")
                            pend = None
                            for j in range(4):
                                N = 512 - 128 * j
                                sps = pssc.tile([128, 1024], f32, tag="sc")
                                nc.tensor.matmul(
                                    sps[:, 0:N],
                                    kT[hp:hp + 64, hc * T + j * 128: hc * T + j * 128 + 128],
                                    qT[hp:hp + 64, hc * TQ + j * 128: hc * TQ + TQ],
                                    start=True, stop=False, skip_group_check=True)
                                nc.tensor.matmul(
                                    sps[:, 512:512 + N],
                                    kT[hp:hp + 64, hc * T + 512 + j * 128: hc * T + 512 + j * 128 + 128],
                                    qT[hp:hp + 64, hc * TQ + j * 128: hc * TQ + TQ],
                                    start=True, stop=False, skip_group_check=True)
                                nc.tensor.matmul(sps[:, 0:128], triT, ident,
                                                 start=False, stop=True,
                                                 skip_group_check=True)
                                nc.tensor.matmul(sps[:, 512:640], flagT, ident,
                                                 start=False, stop=True,
                                                 skip_group_check=True)
                                if pend is not None:
                                    jp, Np = pend
                                    nc.tensor.matmul(
                                        av[:, 128 * jp:512],
                                        vex5[:, :, jp, h * 65:(h + 1) * 65],
                                        Pt[:, NOFF[jp]:NOFF[jp] + 2 * Np]
                                        .rearrange("p (two n) -> p two n", two=2),
                                        start=(jp == 0), stop=False,
                                        perf_mode=PM.DoubleRow, skip_group_check=True)
                                nc.scalar.activation(
                                    Pt[:, NOFF[j]:NOFF[j] + 2 * N]
                                    .rearrange("p (two n) -> p two n", two=2),
                                    sps.rearrange("p (two n) -> p two n", two=2)[:, :, 0:N],
                                    AF.Exp, bias=ln8_t[:, 0:1], scale=SCALE)
                                pend = (j, N)
                            jp, Np = pend
                            nc.tensor.matmul(
                                av[:, 128 * jp:512],
                                vex5[:, :, jp, h * 65:(h + 1) * 65],
                                Pt[:, NOFF[jp]:NOFF[jp] + 2 * Np]
                                .rearrange("p (two n) -> p two n", two=2),
                                start=False, stop=True,
                                perf_mode=PM.DoubleRow, skip_group_check=True)
                            # epilogue: saT = av_rows * (1/den)  (= 16*sa in fp8)
                            rrow = rows1.tile([1, 512], f32, tag="rrow")
                            nc.vector.reciprocal(rrow[:, :], av[64:65, :])
                            rb64 = rows.tile([64, 512], f32, tag="rb64")
                            nc.gpsimd.partition_broadcast(rb64[:, :], rrow[:, :])
                            sa_tmp = rows.tile([64, 512], f32, tag="satmp")
                            nc.vector.tensor_mul(sa_tmp[:, :], av[0:64, :], rb64[:, :])
                            nc.scalar.activation(saT[hp:hp + 64, hc * TQ:(hc + 1) * TQ],
                                                 sa_tmp[:, :], AF.Identity,
                                                 bias=0.0, scale=1.0)

                        for h in range(H):
                            self_head(h)
                            if h < 8:
                                emit_v_unit(1, h)
                            elif vc_units:
                                emit_vc_unit(*vc_units.pop(0))
                        while vc_units:
                            emit_vc_unit(*vc_units.pop(0))

            # ---- proj + residual -> x2T, LN2 stats chase, LN2 + qc ----
            with tc.tile_pool(name="pstL", bufs=1, space="PSUM") as pstL:
                st2s = pstL.tile([128, 512], f32, tag="st2s")
                st2q = pstL.tile([128, 512], f32, tag="st2q")
                with tc.tile_pool(name="wpp", bufs=1) as wpp:
                    wpall = wpp.tile([128, DC * DC * 128], F8, tag="wpj")
                    nc.sync.dma_start(out=wpall[:, :], in_=P["wp8"][:, :])
                    sa3 = saT.rearrange("p (c t) -> p c t", t=TQ)
                    wpv = wpall.rearrange("p (r kc m) -> p r kc m", r=DC, m=128)
                    for r in range(DC):
                        pt = ps.tile([128, 512], f32, tag="mm")
                        for i in range(4):
                            nc.tensor.matmul(pt[:, :], wpv[:, r, 2 * i:2 * i + 2, :],
                                             sa3[:, 2 * i:2 * i + 2, :],
                                             start=(i == 0), stop=(i == 3),
                                             perf_mode=PM.DoubleRow)
                        t1 = sb_sm.tile([128, 512], f32, tag="drain")
                        nc.scalar.activation(t1[:, :], pt[:, :], AF.Identity,
                                             bias=bp_t[:, r:r + 1],
                                             scale=1.0 / (A_X * sc["p"]))
                        nc.vector.tensor_add(x2T[:, r * TQ:(r + 1) * TQ], t1[:, :],
                                             xT_t[:, r * T: r * T + TQ].bitcast(f32))
                        sq = sb_sm.tile([128, 512], bf16, tag="sqb")
                        nc.scalar.activation(sq[:, :], x2T[:, r * TQ:(r + 1) * TQ],
                                             AF.Square)
                        nc.tensor.matmul(st2s[0:1, :], ones_bf[:, :],
                                         x2T[:, r * TQ:(r + 1) * TQ],
                                         start=(r == 0), stop=(r == DC - 1),
                                         skip_group_check=True)
                        nc.tensor.matmul(st2q[0:1, :], ones_bf[:, :], sq[:, :],
                                         start=(r == 0), stop=(r == DC - 1),
                                         skip_group_check=True)

                # ---- LN2 (2 blocks) + qc rows chase ----
                with tc.tile_pool(name="qcp", bufs=1) as qcp:
                    x2hat = qcp.tile([128, DC * TQ], F8, tag="x2hat")
                    wqcall = qcp.tile([128, DC * DC * 128], F8, tag="wqc")
                    nc.sync.dma_start(out=wqcall[:, :], in_=P["wqc8"][:, :])
                    x2h3 = x2hat.rearrange("p (kc t) -> p kc t", t=TQ)
                    wqcv = wqcall.rearrange("p (r kc m) -> p r kc m", r=DC, m=128)
                    for blk in range(2):
                        c0 = blk * 256
                        rbb = bcp.tile([128, 512], f32, tag="rb")
                        nbb = bcp.tile([128, 512], f32, tag="nb")
                        ln_chain(st2s, st2q, c0, 256, rbb, nbb)
                        for kc in range(DC):
                            ln_norm_chunk(x2hat[:, kc * TQ + c0: kc * TQ + c0 + 256],
                                          x2T[:, kc * TQ + c0: kc * TQ + c0 + 256],
                                          rbb, nbb, 256)
                        for r in range(DC):
                            pt = ps.tile([128, 512], f32, tag="mm")
                            for i in range(4):
                                nc.tensor.matmul(pt[:, 0:256],
                                                 wqcv[:, r, 2 * i:2 * i + 2, :],
                                                 x2h3[:, 2 * i:2 * i + 2, c0:c0 + 256],
                                                 start=(i == 0), stop=(i == 3),
                                                 perf_mode=PM.DoubleRow)
                            nc.scalar.activation(qcT[:, r * TQ + c0: r * TQ + c0 + 256],
                                                 pt[:, 0:256], AF.Identity,
                                                 bias=bqc_t[:, r:r + 1],
                                                 scale=1.0 / (A_X * sc["qc"]))
        # xtp/sfp freed here

        # =====================  phase 9-12: cross-attn, out, MLP  ============
        with tc.tile_pool(name="mlpp", bufs=1) as mlpp:
            x3hat = mlpp.tile([128, DC * TQ], F8, tag="x3hat")
            hT = mlpp.tile([128, MC * TQ], F8, tag="hT")
            with tc.tile_pool(name="m1wp", bufs=1) as m1wp:
                wm1all = m1wp.tile([128, MC * DC * 128], F8, tag="wm1")
                nc.sync.dma_start(out=wm1all[:, :], in_=P["wm18"][:, :])
                vcx5 = vcext.rearrange("p (c je) -> p c je", c=5)

                with tc.tile_pool(name="ppc", bufs=2) as ppc, \
                     tc.tile_pool(name="pssc2", bufs=2, space="PSUM") as pssc2, \
                     tc.tile_pool(name="ps2b", bufs=2, space="PSUM") as ps2b:
                    def cross_head(h):
                        hp, hc = (h % 2) * 64, h // 2
                        Pc = ppc.tile([128, 2560], F8, tag="Pc")
                        av = ps2b.tile([65, 512], f32, tag="av")
                        for g in range(2):
                            sps = pssc2.tile([128, 1024], f32, tag="sc")
                            for jj in range(2):
                                c = 2 * g + jj
                                nc.tensor.matmul(
                                    sps[:, jj * 512:(jj + 1) * 512],
                                    kcT[hp:hp + 64, hc * S + c * 128: hc * S + c * 128 + 128],
                                    qcT[hp:hp + 64, hc * TQ:(hc + 1) * TQ],
                                    start=True, stop=True, skip_group_check=True)
                            if g == 1:
                                nc.tensor.matmul(av[:, :],
                                                 vcx5[:, 0:2, h * 65:(h + 1) * 65],
                                                 Pc[:, 0:1024]
                                                 .rearrange("p (two n) -> p two n", two=2),
                                                 start=True, stop=False,
                                                 perf_mode=PM.DoubleRow,
                                                 skip_group_check=True)
                            nc.scalar.activation(Pc[:, g * 1024:(g + 1) * 1024],
                                                 sps[:, :], AF.Exp,
                                                 bias=ln8_t[:, 0:1], scale=SCALE)
                        sps4 = pssc2.tile([128, 1024], f32, tag="sc")
                        nc.tensor.matmul(sps4[0:64, 0:512],
                                         kcT[hp:hp + 64, hc * S + 512: hc * S + 576],
                                         qcT[hp:hp + 64, hc * TQ:(hc + 1) * TQ],
                                         start=True, stop=True, skip_group_check=True)
                        nc.tensor.matmul(av[:, :], vcx5[:, 2:4, h * 65:(h + 1) * 65],
                                         Pc[:, 1024:2048]
                                         .rearrange("p (two n) -> p two n", two=2),
                                         start=False, stop=False,
                                         perf_mode=PM.DoubleRow, skip_group_check=True)
                        nc.scalar.activation(Pc[0:64, 2048:2560], sps4[0:64, 0:512],
                                             AF.Exp, bias=ln8_t[0:64, 0:1], scale=SCALE)
                        nc.tensor.matmul(av[:, :], vcx5[0:64, 4, h * 65:(h + 1) * 65],
                                         Pc[0:64, 2048:2560],
                                         start=False, stop=True, skip_group_check=True)
                        rrow = rows1.tile([1, 512], f32, tag="rrow")
                        nc.vector.reciprocal(rrow[:, :], av[64:65, :])
                        rb64 = rows.tile([64, 512], f32, tag="rb64")
                        nc.gpsimd.partition_broadcast(rb64[:, :], rrow[:, :])
                        ca_tmp = rows.tile([64, 512], f32, tag="satmp")
                        nc.vector.tensor_mul(ca_tmp[:, :], av[0:64, :], rb64[:, :])
                        nc.scalar.activation(caT[hp:hp + 64, hc * TQ:(hc + 1) * TQ],
                                             ca_tmp[:, :], AF.Identity,
                                             bias=0.0, scale=1.0)

                    for h in range(H):
                        cross_head(h)

                # ---- out proj + residual -> x3T, LN3 stats chase ----
                with tc.tile_pool(name="pstM", bufs=1, space="PSUM") as pstM:
                    st3s = pstM.tile([128, 512], f32, tag="st3s")
                    st3q = pstM.tile([128, 512], f32, tag="st3q")
                    ca3 = caT.rearrange("p (c t) -> p c t", t=TQ)
                    with tc.tile_pool(name="wos", bufs=3) as wos:
                        for r in range(DC):
                            wt = wos.tile([128, DC * 128], F8, tag="wor")
                            nc.sync.dma_start(
                                out=wt[:, :],
                                in_=P["wo8"][:, r * DC * 128:(r + 1) * DC * 128])
                            wv_ = wt.rearrange("p (kc m) -> p kc m", m=128)
                            pt = ps.tile([128, 512], f32, tag="mm")
                            for i in range(4):
                                nc.tensor.matmul(pt[:, :], wv_[:, 2 * i:2 * i + 2, :],
                                                 ca3[:, 2 * i:2 * i + 2, :],
                                                 start=(i == 0), stop=(i == 3),
                                                 perf_mode=PM.DoubleRow)
                            t1 = sb_sm.tile([128, 512], f32, tag="drain")
                            nc.scalar.activation(t1[:, :], pt[:, :], AF.Identity,
                                                 bias=bo_t[:, r:r + 1],
                                                 scale=1.0 / (A_X * sc["o"]))
                            nc.vector.tensor_add(x3T[:, r * TQ:(r + 1) * TQ], t1[:, :],
                                                 x2T[:, r * TQ:(r + 1) * TQ])
                            sq = sb_sm.tile([128, 512], bf16, tag="sqb")
                            nc.scalar.activation(sq[:, :], x3T[:, r * TQ:(r + 1) * TQ],
                                                 AF.Square)
                            nc.tensor.matmul(st3s[0:1, :], ones_bf[:, :],
                                             x3T[:, r * TQ:(r + 1) * TQ],
                                             start=(r == 0), stop=(r == DC - 1),
                                             skip_group_check=True)
                            nc.tensor.matmul(st3q[0:1, :], ones_bf[:, :], sq[:, :],
                                             start=(r == 0), stop=(r == DC - 1),
                                             skip_group_check=True)

                    # ---- LN3 (2 blocks) + mlp1 rows chase ----
                    x3h3 = x3hat.rearrange("p (kc t) -> p kc t", t=TQ)
                    wm1v = wm1all.rearrange("p (r kc m) -> p r kc m", r=MC, m=128)
                    for blk in range(2):
                        c0 = blk * 256
                        rbb = bcp.tile([128, 512], f32, tag="rb")
                        nbb = bcp.tile([128, 512], f32, tag="nb")
                        ln_chain(st3s, st3q, c0, 256, rbb, nbb)
                        for kc in range(DC):
                            ln_norm_chunk(x3hat[:, kc * TQ + c0: kc * TQ + c0 + 256],
                                          x3T[:, kc * TQ + c0: kc * TQ + c0 + 256],
                                          rbb, nbb, 256)
                        for r in range(MC):
                            pt = ps.tile([128, 512], f32, tag="mm")
                            for i in range(4):
                                nc.tensor.matmul(pt[:, 0:256],
                                                 wm1v[:, r, 2 * i:2 * i + 2, :],
                                                 x3h3[:, 2 * i:2 * i + 2, c0:c0 + 256],
                                                 start=(i == 0), stop=(i == 3),
                                                 perf_mode=PM.DoubleRow)
                            nc.scalar.activation(hT[:, r * TQ + c0: r * TQ + c0 + 256],
                                                 pt[:, 0:256], AF.Gelu,
                                                 bias=bm1_t[:, r:r + 1],
                                                 scale=1.0 / (A_X * sc["m1"]))

            # ---- mlp2 + residual -> yT (streamed weights) ----
            hT3 = hT.rearrange("p (kc t) -> p kc t", t=TQ)
            with tc.tile_pool(name="wm2s", bufs=3) as wm2s:
                for r in range(DC):
                    wt = wm2s.tile([128, MC * 128], F8, tag="wm2r")
                    nc.sync.dma_start(
                        out=wt[:, :],
                        in_=P["wm28"][:, r * MC * 128:(r + 1) * MC * 128])
                    wv_ = wt.rearrange("p (kc m) -> p kc m", m=128)
                    pt = ps.tile([128, 512], f32, tag="mm")
                    for i in range(MC // 2):
                        nc.tensor.matmul(pt[:, :], wv_[:, 2 * i:2 * i + 2, :],
                                         hT3[:, 2 * i:2 * i + 2, :],
                                         start=(i == 0), stop=(i == MC // 2 - 1),
                                         perf_mode=PM.DoubleRow)
                    t1 = sb_sm.tile([128, 512], f32, tag="drain")
                    nc.scalar.activation(t1[:, :], pt[:, :], AF.Identity,
                                         bias=bm2_t[:, r:r + 1], scale=1.0 / sc["m2"])
                    yt = sb_sm.tile([128, 512], f32, tag="drain")
                    nc.vector.tensor_add(yt[:, :], t1[:, :], x3T[:, r * TQ:(r + 1) * TQ])
                    nc.sync.dma_start(out=P["yT"][r * 128:(r + 1) * 128, :], in_=yt[:, :])


def _build_program(sc):
    nc = bacc.Bacc()
    P = {}
    P["xT"] = nc.declare_dram_parameter("xT", [D, T], dt.float32r, isOutput=False)
    P["encT8"] = nc.declare_dram_parameter("encT8", [128, EC * S], F8, isOutput=False)
    P["mask3"] = nc.declare_dram_parameter("mask3", [128, 3 * 128], dt.bfloat16, isOutput=False)
    for nm, shp in [("wq8", DC * DC * 128), ("wk8", DC * DC * 128),
                    ("wv8", 2 * DC * 512), ("wp8", DC * DC * 128),
                    ("wqc8", DC * DC * 128), ("wkc8", DC * EC * 128),
                    ("wvc8", 2 * EC * 512), ("wo8", DC * DC * 128),
                    ("wm18", MC * DC * 128), ("wm28", DC * MC * 128)]:
        P[nm] = nc.declare_dram_parameter(nm, [128, shp], F8, isOutput=False)
    for nm, n in [("bq", D), ("bproj", D), ("bqc", D), ("bout", D),
                  ("bm1", DM), ("bm2", D)]:
        P[nm] = nc.declare_dram_parameter(nm, [n, 1], dt.float32, isOutput=False)
    P["yT"] = nc.declare_dram_parameter("yT", [D, TQ], dt.float32, isOutput=True)

    with tile.TileContext(nc) as tc:
        _build_body(nc, tc, P, sc)
    nc.compile()
    return nc


def _get_program(sc):
    if "nc" not in _cached:
        _cached["nc"] = _build_program(sc)
    return _cached["nc"]


last_result = None


def kernel(**inputs):
    global last_result
    import os
    trace = bool(os.environ.get("KERNEL_TRACE"))
    in_maps, metas, sc = _prepare_inputs(**inputs)
    nc = _get_program(sc)
    res = run_bass_kernel_spmd(nc, in_maps, list(range(8)), trace=trace)
    last_result = res
    out = np.empty((B, T, D), dtype=np.float32)
    for c, (b, own_blocks) in enumerate(metas):
        yTc = res.results[c]["yT"]            # [D, TQ]
        yt = yTc.T.reshape(4, 128, D)
        for i, blk in enumerate(own_blocks):
            out[b, blk * 128:(blk + 1) * 128, :] = yt[i]
    return out


# revision 26
# speedup vs baseline: 1.0791x; 1.0791x over previous
"""Trainium2 Bass kernel for a transformer decoder block (self-attn + cross-attn + MLP).

Sharding: data-parallel over (batch, strided query blocks) = 8 shards, no
collectives. Core (b, h) owns query blocks {h, h+2, h+4, h+6} of 128 tokens;
the host permutes tokens so own queries come first. The strided split makes
the causal block structure identical on every core: query block i attends to
own key chunks j<=i and other-half key chunks j<=i, with only the two
diagonal blocks needing masks (a constant triangular mask and a per-core
all-or-nothing flag), applied by tiny PE matmuls into the score PSUM.

Layout: transposed activations [feature partition, token free]. Weights are
pre-tiled on the host to [128, ...] fp8 (e4m3, pow2-scaled) so every weight
DMA is a contiguous 2D copy. Big GEMMs run fp8 DoubleRow (2 x 128 contraction
per pass); QK stays bf16. Softmax: exp(s*scale + ln8) gives 8*P directly in
fp8; denominators come from a ones column in the extended V and are folded
into a per-head reciprocal+broadcast epilogue (no cross-engine round trips).
LayerNorm stats (ones-matmuls + squares) chase the producing projection; the
normalize is pipelined in token blocks so matmuls start while later blocks
normalize.
"""

import sys

sys.path.insert(0, "/opt/trn_rl_repo")

import math
import numpy as np
import ml_dtypes

import concourse.bass as bass
import concourse.bacc as bacc
import concourse.mybir as mybir
from concourse import tile
from concourse.bass_utils import run_bass_kernel_spmd

dt = mybir.dt
AF = mybir.ActivationFunctionType
PM = mybir.MatmulPerfMode

# Problem dims (hardcoded per contest contract)
B, T, D, H, HD = 4, 1024, 1024, 16, 64
S, DE, DM = 576, 768, 4096
TQ = T // 2          # queries per core
DC = D // 128        # feature chunks (8)
EC = DE // 128       # enc feature chunks (6)
MC = DM // 128       # mlp hidden chunks (32)
SCALE = HD ** -0.5
EPS = 1e-5
A_X = 16.0           # xhat / enc fp8 scale
A_P = 8.0            # softmax numerator scale (via exp bias ln A_P)
LN8 = math.log(A_P)
NEG = -1.0e7         # additive mask value

F8NP = ml_dtypes.float8_e4m3
BFNP = ml_dtypes.bfloat16
F8 = dt.float8e4

# which GEMMs use fp8 DoubleRow (others bf16): knobs for accuracy fallback
F8_STAGES = {"qkv", "v", "proj", "qc", "kc", "vc", "av", "avc", "out", "mlp1", "mlp2"}

_cached = {}


def _pow2_scale(w):
    am = float(np.abs(w).max())
    return 2.0 ** int(np.floor(np.log2(240.0 / am)))


def _tile_rows(w, nrow, nkc, m):
    """w [nkc*128, nrow*m] -> [128, nrow*nkc*m] with [p, r, kc, m] order."""
    K, N = w.shape
    assert K == nkc * 128 and N == nrow * m
    wt = w.reshape(nkc, 128, nrow, m).transpose(1, 2, 0, 3)
    return np.ascontiguousarray(wt.reshape(128, nrow * nkc * m))


def _prepare_inputs(x, enc, tgt_key_padding_mask, enc_padding_mask,
                    ln1_w, ln1_b, qkv_w, qkv_b, proj_w, proj_b,
                    ln2_w, ln2_b, q_w, q_b, k_w, k_b, v_w, v_b, out_w, out_b,
                    ln3_w, ln3_b, mlp1_w, mlp1_b, mlp2_w, mlp2_b):
    f32 = np.float32
    asf = lambda a: np.asarray(a, dtype=f32)
    x, enc = asf(x), asf(enc)
    ln1_w, ln1_b, ln2_w, ln2_b, ln3_w, ln3_b = map(asf, (ln1_w, ln1_b, ln2_w, ln2_b, ln3_w, ln3_b))
    qkv_w, qkv_b, proj_w, proj_b = map(asf, (qkv_w, qkv_b, proj_w, proj_b))
    q_w, q_b, k_w, k_b, v_w, v_b, out_w, out_b = map(
        asf, (q_w, q_b, k_w, k_b, v_w, v_b, out_w, out_b))
    mlp1_w, mlp1_b, mlp2_w, mlp2_b = map(asf, (mlp1_w, mlp1_b, mlp2_w, mlp2_b))

    # host-side folds (as baseline): LN affine into weights, k-bias dropped
    # (softmax-invariant), v-biases folded into the following projection bias.
    wqkv_f = qkv_w * ln1_w[:, None]
    bqkv = qkv_b + qkv_w.T @ ln1_b
    b_q = bqkv[0:D]
    b_v = bqkv[2 * D:3 * D]
    bprojf = proj_b + proj_w.T @ b_v
    wqf = q_w * ln2_w[:, None]
    bqcf = q_b + q_w.T @ ln2_b
    boutf = out_b + out_w.T @ v_b
    wm1f = mlp1_w * ln3_w[:, None]
    bm1f = mlp1_b + mlp1_w.T @ ln3_b

    # per-tensor pow2 scales; baked into the compiled program's drain scales
    wq_ = wqkv_f[:, 0:D]; wk_ = wqkv_f[:, D:2 * D]; wv_ = wqkv_f[:, 2 * D:3 * D]
    sc = {
        "q": _pow2_scale(wq_), "k": _pow2_scale(wk_), "v": _pow2_scale(wv_),
        "p": _pow2_scale(proj_w), "qc": _pow2_scale(wqf), "kc": _pow2_scale(k_w),
        "vc": _pow2_scale(v_w), "o": _pow2_scale(out_w),
        "m1": _pow2_scale(wm1f), "m2": _pow2_scale(mlp2_w),
    }
    c8 = lambda w, s: np.ascontiguousarray((w * s).astype(F8NP))
    shared = {
        "wq8": c8(_tile_rows(wq_, DC, DC, 128), sc["q"]),
        "wk8": c8(_tile_rows(wk_, DC, DC, 128), sc["k"]),
        "wv8": c8(_tile_rows(wv_, 2, DC, 512), sc["v"]),
        "wp8": c8(_tile_rows(proj_w, DC, DC, 128), sc["p"]),
        "wqc8": c8(_tile_rows(wqf, DC, DC, 128), sc["qc"]),
        "wkc8": c8(_tile_rows(k_w, DC, EC, 128), sc["kc"]),
        "wvc8": c8(_tile_rows(v_w, 2, EC, 512), sc["vc"]),
        "wo8": c8(_tile_rows(out_w, DC, DC, 128), sc["o"]),
        "wm18": c8(_tile_rows(wm1f, MC, DC, 128), sc["m1"]),
        "wm28": c8(_tile_rows(mlp2_w, DC, MC, 128), sc["m2"]),
    }
    col = lambda v: np.ascontiguousarray(v.reshape(-1, 1).astype(f32))
    shared.update({
        "bq": col(b_q), "bproj": col(bprojf), "bqc": col(bqcf),
        "bout": col(boutf), "bm1": col(bm1f), "bm2": col(mlp2_b),
    })
    # encT pre-tiled fp8*A_X: [128, EC*S]
    encT = enc.transpose(0, 2, 1)  # [B, DE, S]

    # mask tiles [128, 3*128] bf16: [triT | flag | identity]
    # triT[i, j] = M[j, i] where M[key, q] = 0 if key<=q else NEG (same block)
    tri = np.where(np.arange(128)[:, None] <= np.arange(128)[None, :], 0.0, NEG)
    triT = tri.T.astype(BFNP)
    ident = np.eye(128, dtype=BFNP)

    in_maps, metas = [], []
    for c in range(8):
        b, h = c // 2, c % 2
        own_blocks = np.arange(h, 8, 2)
        other_blocks = np.arange(1 - h, 8, 2)
        own = (own_blocks[:, None] * 128 + np.arange(128)[None, :]).reshape(-1)
        other = (other_blocks[:, None] * 128 + np.arange(128)[None, :]).reshape(-1)
        perm = np.concatenate([own, other])
        xT_np = np.ascontiguousarray(x[b][perm].T)  # [D, T] own-first
        enc8 = np.ascontiguousarray(
            (encT[b].reshape(EC, 128, S).transpose(1, 0, 2).reshape(128, EC * S)
             * A_X).astype(F8NP))
        flag = np.full((128, 128), NEG if h == 0 else 0.0, dtype=BFNP)
        mask3 = np.ascontiguousarray(np.concatenate([triT, flag, ident], axis=1))
        im = dict(shared)
        im["xT"] = xT_np
        im["encT8"] = enc8
        im["mask3"] = mask3
        in_maps.append(im)
        metas.append((b, own_blocks))
    return in_maps, metas, sc


def _build_body(nc, tc, P, sc):
    from contextlib import ExitStack
    f32, f32r, bf16 = dt.float32, dt.float32r, dt.bfloat16
    ctx = ExitStack()
    with ctx:
        const = ctx.enter_context(tc.tile_pool(name="const", bufs=1))
        rows = ctx.enter_context(tc.tile_pool(name="rows", bufs=2))
        rows1 = ctx.enter_context(tc.tile_pool(name="rows1", bufs=2))
        bcp = ctx.enter_context(tc.tile_pool(name="bcp", bufs=2))
        sb_sm = ctx.enter_context(tc.tile_pool(name="sb_sm", bufs=3))
        ps = ctx.enter_context(tc.tile_pool(name="ps", bufs=2, space="PSUM"))
        resp = ctx.enter_context(tc.tile_pool(name="resp", bufs=1))

        ones32 = const.tile([128, 1], f32, tag="ones32")
        nc.vector.memset(ones32[:, :], 1.0)
        ones = const.tile([128, 1], f32r, tag="ones")
        nc.scalar.activation(ones[:, :], ones32[:, :], AF.Copy)
        ones_bf = const.tile([128, 1], bf16, tag="ones_bf")
        nc.vector.memset(ones_bf[:, :], 1.0)
        eps2 = const.tile([1, 1], f32, tag="eps2")
        nc.vector.memset(eps2[:, :], EPS / (A_X * A_X))
        ln8_t = const.tile([128, 1], f32, tag="ln8")
        nc.vector.memset(ln8_t[:, :], LN8)
        mask_t = const.tile([128, 384], bf16, tag="mask3")
        nc.sync.dma_start(out=mask_t[:, :], in_=P["mask3"][:, :])
        triT, flagT, ident = mask_t[:, 0:128], mask_t[:, 128:256], mask_t[:, 256:384]

        def load_bias(drh, nr, tag):
            t = const.tile([128, nr], f32, tag=tag, name="b_" + tag)
            nc.sync.dma_start(out=t.rearrange("p (r one) -> p r one", one=1),
                              in_=drh.rearrange("(r p) one -> p r one", p=128))
            return t

        bq_t = load_bias(P["bq"], DC, "bq")
        bp_t = load_bias(P["bproj"], DC, "bproj")
        bqc_t = load_bias(P["bqc"], DC, "bqc")
        bo_t = load_bias(P["bout"], DC, "bout")
        bm1_t = load_bias(P["bm1"], MC, "bm1")
        bm2_t = load_bias(P["bm2"], DC, "bm2")

        # residual stream (bf16) + cross tensors spanning many phases
        x2T = resp.tile([128, DC * TQ], bf16, tag="x2T")
        x3T = resp.tile([128, DC * TQ], bf16, tag="x3T")
        qcT = resp.tile([128, DC * TQ], bf16, tag="qcT")
        kcT = resp.tile([128, DC * S], bf16, tag="kcT")
        vcext = resp.tile([128, 5 * H * 65], F8, tag="vcext")
        caT = resp.tile([128, DC * TQ], F8, tag="caT")

        # ---- LayerNorm helpers (transposed layout, stats via ones-matmul) ----
        def ln_chain(sts, stq, c0, W, rb_t, nb_t):
            R = rows1.tile([1, 2048], f32, tag="lnrow", bufs=1)
            t1n, t2 = R[0:1, 0:W], R[0:1, 512:512 + W]
            t5, msq = R[0:1, 1024:1024 + W], R[0:1, 1536:1536 + W]
            nc.vector.tensor_scalar_mul(t1n, sts[0:1, c0:c0 + W], -1.0 / D)
            nc.vector.tensor_scalar_mul(t2, stq[0:1, c0:c0 + W], 1.0 / D)
            nc.vector.tensor_mul(msq, t1n, t1n)
            nc.vector.tensor_sub(t2, t2, msq)
            nc.scalar.activation(t5, t2, AF.Abs_reciprocal_sqrt,
                                 bias=eps2[0:1, 0:1], scale=1.0 / (A_X * A_X))
            nc.vector.tensor_mul(t1n, t1n, t5)        # -mean * A_X * rstd
            nc.gpsimd.partition_broadcast(rb_t[:, 0:W], t5)
            nc.gpsimd.partition_broadcast(nb_t[:, 0:W], t1n)

        gp_mul = (nc.gpsimd.tensor_mul if hasattr(nc.gpsimd, "tensor_mul")
                  else nc.vector.tensor_mul)

        def ln_norm_chunk(dst, xsrc, rb_t, nb_t, W):
            xs = rows.tile([128, 512], f32, tag="xs")
            gp_mul(xs[:, 0:W], xsrc, rb_t[:, 0:W])
            nc.vector.tensor_add(dst, xs[:, 0:W], nb_t[:, 0:W])

        # =====================  phase 1-8: LN1, self-attn, proj, LN2, qc  ====
        with tc.tile_pool(name="xtp", bufs=1) as xtp, \
             tc.tile_pool(name="sfp", bufs=1) as sfp:
            xT_t = xtp.tile([128, DC * T], f32r, tag="xT")
            for kc in range(DC):
                nc.sync.dma_start(out=xT_t[:, kc * T:(kc + 1) * T],
                                  in_=P["xT"][kc * 128:(kc + 1) * 128, :])
            xhat1 = sfp.tile([128, DC * T], F8, tag="xhat1")
            kT = sfp.tile([128, DC * T], bf16, tag="kT")
            qT = sfp.tile([128, DC * TQ], bf16, tag="qT")
            vext = sfp.tile([128, 8 * H * 65], F8, tag="vext")
            saT = sfp.tile([128, DC * TQ], F8, tag="saT")

            nc.vector.memset(
                vext.rearrange("p (c e) -> p c e", e=65)[:, :, 64:65], 1.0)
            nc.vector.memset(vcext[64:128, 4 * H * 65:5 * H * 65], 0.0)
            nc.vector.memset(
                vcext.rearrange("p (c e) -> p c e", e=65)[:, 0:4 * H, 64:65], 1.0)
            nc.vector.memset(
                vcext.rearrange("p (c e) -> p c e", e=65)[0:64, 4 * H:5 * H, 64:65], 1.0)

            xh3 = xhat1.rearrange("p (kc t) -> p kc t", t=T)

            with tc.tile_pool(name="ckA", bufs=1) as ckA:
                encT_t = ckA.tile([128, EC * S], F8, tag="encT")
                nc.sync.dma_start(out=encT_t[:, :], in_=P["encT8"][:, :])
                wkcall = ckA.tile([128, DC * EC * 128], F8, tag="wkc")
                nc.sync.dma_start(out=wkcall[:, :], in_=P["wkc8"][:, :])
                wvcall = ckA.tile([128, 2 * EC * 512], F8, tag="wvc")
                nc.sync.dma_start(out=wvcall[:, :], in_=P["wvc8"][:, :])
                encv = encT_t.rearrange("p (ec s) -> p ec s", s=S)

                def emit_kc_unit(r, et):
                    pt = ps.tile([128, 512], f32, tag="mm")
                    wv_ = wkcall.rearrange("p (r ec m) -> p r ec m", r=DC, m=128)
                    for i in range(EC // 2):
                        nc.tensor.matmul(pt[:, 0:288], wv_[:, r, 2 * i:2 * i + 2, :],
                                         encv[:, 2 * i:2 * i + 2, et * 288:(et + 1) * 288],
                                         start=(i == 0), stop=(i == 2),
                                         perf_mode=PM.DoubleRow)
                    nc.scalar.activation(kcT[:, r * S + et * 288: r * S + (et + 1) * 288],
                                         pt[:, 0:288], AF.Identity, bias=0.0,
                                         scale=1.0 / (A_X * sc["kc"]))

                kc_units = [(r, et) for r in range(DC) for et in range(2)]
                vxv = vext.rearrange("p (tk j e) -> p tk j e", tk=8, j=H)
                wvcv = wvcall.rearrange("p (vf ec m) -> p vf ec m", vf=2, m=512)
                vcxv = vcext.rearrange("p (tk j e) -> p tk j e", tk=5, j=H)

                def emit_vc_unit(vf, tokc):
                    npart = 128 if tokc < 4 else 64
                    pv = ps.tile([128, 512], f32, tag="mm")
                    for i in range(EC // 2):
                        nc.tensor.matmul(pv[:npart, :],
                                         encv[:, 2 * i:2 * i + 2, tokc * 128:tokc * 128 + npart],
                                         wvcv[:, vf, 2 * i:2 * i + 2, :],
                                         start=(i == 0), stop=(i == 2),
                                         perf_mode=PM.DoubleRow)
                    nc.vector.tensor_scalar_mul(
                        vcxv[:npart, tokc, 8 * vf:8 * vf + 8, 0:64],
                        pv[:npart].rearrange("p (j d) -> p j d", j=8), 1.0 / sc["vc"])


                # ---- LN1 stats (chasing x DMA) with cross-kc interleaved ----
                with tc.tile_pool(name="wqks", bufs=4) as wqks, \
                     tc.tile_pool(name="pst", bufs=1, space="PSUM") as pst:
                    st = {nm: pst.tile([128, 512], f32, tag="st_" + nm,
                                       name="st_" + nm)
                          for nm in ("s0", "s1", "q0", "q1")}
                    for kc in range(DC):
                        for tt in range(2):
                            sq = sb_sm.tile([128, 512], f32r, tag="sq2")
                            nc.scalar.activation(
                                sq[:, :],
                                xT_t[:, kc * T + tt * 512: kc * T + tt * 512 + 512],
                                AF.Square)
                            nc.tensor.matmul(st["s%d" % tt][0:1, :], ones[:, :],
                                             xT_t[:, kc * T + tt * 512: kc * T + tt * 512 + 512],
                                             start=(kc == 0), stop=(kc == DC - 1),
                                             skip_group_check=True)
                            nc.tensor.matmul(st["q%d" % tt][0:1, :], ones[:, :],
                                             sq[:, :],
                                             start=(kc == 0), stop=(kc == DC - 1),
                                             skip_group_check=True)
                    # ---- LN1 tt0 chain + norm, q/k rows chase ----
                    rb0 = bcp.tile([128, 512], f32, tag="rb")
                    nb0 = bcp.tile([128, 512], f32, tag="nb")
                    ln_chain(st["s0"], st["q0"], 0, 512, rb0, nb0)
                    for kc in range(DC):
                        ln_norm_chunk(xhat1[:, kc * T: kc * T + 512],
                                      xT_t[:, kc * T: kc * T + 512], rb0, nb0, 512)
                    while kc_units:
                        emit_kc_unit(*kc_units.pop(0))
                    for vf in range(2):
                        for tokc in range(5):
                            emit_vc_unit(vf, tokc)

                    def qk_row(wdram, r, dst, bias, sscale, tcols):
                        wt = wqks.tile([128, DC * 128], F8, tag="wr")
                        nc.sync.dma_start(out=wt[:, :],
                                          in_=wdram[:, r * DC * 128:(r + 1) * DC * 128])
                        wv_ = wt.rearrange("p (kc m) -> p kc m", m=128)
                        pt = ps.tile([128, 512], f32, tag="mm")
                        for i in range(4):
                            nc.tensor.matmul(pt[:, :], wv_[:, 2 * i:2 * i + 2, :],
                                             xh3[:, 2 * i:2 * i + 2, tcols:tcols + 512],
                                             start=(i == 0), stop=(i == 3),
                                             perf_mode=PM.DoubleRow)
                        if bias is None:
                            nc.scalar.activation(dst, pt[:, :], AF.Identity,
                                                 bias=0.0, scale=sscale)
                        else:
                            nc.scalar.activation(dst, pt[:, :], AF.Identity,
                                                 bias=bias, scale=sscale)

                    for r in range(DC):
                        qk_row(P["wq8"], r, qT[:, r * TQ:(r + 1) * TQ],
                               bq_t[:, r:r + 1], 1.0 / (A_X * sc["q"]), 0)
                        qk_row(P["wk8"], r, kT[:, r * T: r * T + 512],
                               None, 1.0 / (A_X * sc["k"]), 0)

                    # ---- LN1 tt1 chain + norm, k-tt1 rows chase ----
                    rb1 = bcp.tile([128, 512], f32, tag="rb")
                    nb1 = bcp.tile([128, 512], f32, tag="nb")
                    ln_chain(st["s1"], st["q1"], 0, 512, rb1, nb1)
                    for kc in range(DC):
                        ln_norm_chunk(xhat1[:, kc * T + 512: kc * T + 1024],
                                      xT_t[:, kc * T + 512: kc * T + 1024], rb1, nb1, 512)
                    for r in range(DC):
                        qk_row(P["wk8"], r, kT[:, r * T + 512: r * T + 1024],
                               None, 1.0 / (A_X * sc["k"]), 512)

                # ---- V units + self-attention heads ----

                with tc.tile_pool(name="wvp", bufs=1) as wvp:
                    wvall = wvp.tile([128, 2 * DC * 512], F8, tag="wv")
                    nc.sync.dma_start(out=wvall[:, :], in_=P["wv8"][:, :])
                    wvv = wvall.rearrange("p (vf kc m) -> p vf kc m", vf=2, m=512)

                    def emit_v_unit(vf, tokc):
                        pv = ps.tile([128, 512], f32, tag="mm")
                        for i in range(4):
                            nc.tensor.matmul(pv[:, :],
                                             xh3[:, 2 * i:2 * i + 2,
                                                 tokc * 128:(tokc + 1) * 128],
                                             wvv[:, vf, 2 * i:2 * i + 2, :],
                                             start=(i == 0), stop=(i == 3),
                                             perf_mode=PM.DoubleRow)
                        nc.vector.tensor_scalar_mul(
                            vxv[:, tokc, 8 * vf:8 * vf + 8, 0:64],
                            pv.rearrange("p (j d) -> p j d", j=8), 1.0 / sc["v"])

                    for vf in range(2):
                        for tokc in range(8):
                            emit_v_unit(vf, tokc)

                    NOFF = [0, 1024, 1792, 2304]
                    vex5 = vext.rearrange("p (g c je) -> p g c je", g=2, c=4)

                    with tc.tile_pool(name="pp", bufs=2) as pp, \
                         tc.tile_pool(name="pssc", bufs=2, space="PSUM") as pssc, \
                         tc.tile_pool(name="ps2", bufs=2, space="PSUM") as ps2:
                        def self_head(h):
                            hp, hc = (h % 2) * 64, h // 2
                            Pt = pp.tile([128, 2560], F8, tag="P")
                            av = ps2.tile([65, 512], f32, tag="av")
                            pend = None
                            for j in range(4):
                                N = 512 - 128 * j
                                sps = pssc.tile([128, 1024], f32, tag="sc")
                                nc.tensor.matmul(
                                    sps[:, 0:N],
                                    kT[hp:hp + 64, hc * T + j * 128: hc * T + j * 128 + 128],
                                    qT[hp:hp + 64, hc * TQ + j * 128: hc * TQ + TQ],
                                    start=True, stop=False, skip_group_check=True)
                                nc.tensor.matmul(
                                    sps[:, 512:512 + N],
                                    kT[hp:hp + 64, hc * T + 512 + j * 128: hc * T + 512 + j * 128 + 128],
                                    qT[hp:hp + 64, hc * TQ + j * 128: hc * TQ + TQ],
                                    start=True, stop=False, skip_group_check=True)
                                nc.tensor.matmul(sps[:, 0:128], triT, ident,
                                                 start=False, stop=True,
                                                 skip_group_check=True)
                                nc.tensor.matmul(sps[:, 512:640], flagT, ident,
                                                 start=False, stop=True,
                                                 skip_group_check=True)
                                if pend is not None:
                                    jp, Np = pend
                                    nc.tensor.matmul(
                                        av[:, 128 * jp:512],
                                        vex5[:, :, jp, h * 65:(h + 1) * 65],
                                        Pt[:, NOFF[jp]:NOFF[jp] + 2 * Np]
                                        .rearrange("p (two n) -> p two n", two=2),
                                        start=(jp == 0), stop=False,
                                        perf_mode=PM.DoubleRow, skip_group_check=True)
                                nc.scalar.activation(
                                    Pt[:, NOFF[j]:NOFF[j] + 2 * N]
                                    .rearrange("p (two n) -> p two n", two=2),
                                    sps.rearrange("p (two n) -> p two n", two=2)[:, :, 0:N],
                                    AF.Exp, bias=ln8_t[:, 0:1], scale=SCALE)
                                pend = (j, N)
                            jp, Np = pend
                            nc.tensor.matmul(
                                av[:, 128 * jp:512],
                                vex5[:, :, jp, h * 65:(h + 1) * 65],
                                Pt[:, NOFF[jp]:NOFF[jp] + 2 * Np]
                                .rearrange("p (two n) -> p two n", two=2),
                                start=False, stop=True,
                                perf_mode=PM.DoubleRow, skip_group_check=True)
                            # epilogue: saT = av_rows * (1/den)  (= 16*sa in fp8)
                            den_sb = rows1.tile([1, 512], f32, tag="densb")
                            nc.scalar.activation(den_sb[:, :], av[64:65, :],
                                                 AF.Identity, bias=0.0, scale=1.0)
                            rrow = rows1.tile([1, 512], f32, tag="rrow")
                            nc.vector.reciprocal_approx_fast(rrow[:, :], den_sb[:, :])
                            rb64 = rows.tile([64, 512], f32, tag="rb64")
                            nc.gpsimd.partition_broadcast(rb64[:, :], rrow[:, :])
                            sa_tmp = rows.tile([64, 512], f32, tag="satmp")
                            nc.vector.tensor_mul(sa_tmp[:, :], av[0:64, :], rb64[:, :])
                            nc.scalar.activation(saT[hp:hp + 64, hc * TQ:(hc + 1) * TQ],
                                                 sa_tmp[:, :], AF.Identity,
                                                 bias=0.0, scale=1.0)

                        for h in range(H):
                            self_head(h)

            # ---- proj + residual -> x2T, LN2 stats chase, LN2 + qc ----
            with tc.tile_pool(name="pstL", bufs=1, space="PSUM") as pstL:
                st2s = pstL.tile([128, 512], f32, tag="st2s")
                st2q = pstL.tile([128, 512], f32, tag="st2q")
                with tc.tile_pool(name="wpp", bufs=1) as wpp:
                    wpall = wpp.tile([128, DC * DC * 128], F8, tag="wpj")
                    nc.sync.dma_start(out=wpall[:, :], in_=P["wp8"][:, :])
                    sa3 = saT.rearrange("p (c t) -> p c t", t=TQ)
                    wpv = wpall.rearrange("p (r kc m) -> p r kc m", r=DC, m=128)
                    for r in range(DC):
                        pt = ps.tile([128, 512], f32, tag="mm")
                        for i in range(4):
                            nc.tensor.matmul(pt[:, :], wpv[:, r, 2 * i:2 * i + 2, :],
                                             sa3[:, 2 * i:2 * i + 2, :],
                                             start=(i == 0), stop=(i == 3),
                                             perf_mode=PM.DoubleRow)
                        t1 = sb_sm.tile([128, 512], f32, tag="drain")
                        nc.scalar.activation(t1[:, :], pt[:, :], AF.Identity,
                                             bias=bp_t[:, r:r + 1],
                                             scale=1.0 / (A_X * sc["p"]))
                        nc.vector.tensor_add(x2T[:, r * TQ:(r + 1) * TQ], t1[:, :],
                                             xT_t[:, r * T: r * T + TQ].bitcast(f32))
                        sq = sb_sm.tile([128, 512], bf16, tag="sqb")
                        nc.scalar.activation(sq[:, :], x2T[:, r * TQ:(r + 1) * TQ],
                                             AF.Square)
                        nc.tensor.matmul(st2s[0:1, :], ones_bf[:, :],
                                         x2T[:, r * TQ:(r + 1) * TQ],
                                         start=(r == 0), stop=(r == DC - 1),
                                         skip_group_check=True)
                        nc.tensor.matmul(st2q[0:1, :], ones_bf[:, :], sq[:, :],
                                         start=(r == 0), stop=(r == DC - 1),
                                         skip_group_check=True)

                # ---- LN2 (2 blocks) + qc rows chase ----
                with tc.tile_pool(name="qcp", bufs=1) as qcp:
                    x2hat = qcp.tile([128, DC * TQ], F8, tag="x2hat")
                    wqcall = qcp.tile([128, DC * DC * 128], F8, tag="wqc")
                    nc.sync.dma_start(out=wqcall[:, :], in_=P["wqc8"][:, :])
                    x2h3 = x2hat.rearrange("p (kc t) -> p kc t", t=TQ)
                    wqcv = wqcall.rearrange("p (r kc m) -> p r kc m", r=DC, m=128)
                    for blk in range(2):
                        c0 = blk * 256
                        rbb = bcp.tile([128, 512], f32, tag="rb")
                        nbb = bcp.tile([128, 512], f32, tag="nb")
                        ln_chain(st2s, st2q, c0, 256, rbb, nbb)
                        for kc in range(DC):
                            ln_norm_chunk(x2hat[:, kc * TQ + c0: kc * TQ + c0 + 256],
                                          x2T[:, kc * TQ + c0: kc * TQ + c0 + 256],
                                          rbb, nbb, 256)
                        for r in range(DC):
                            pt = ps.tile([128, 512], f32, tag="mm")
                            for i in range(4):
                                nc.tensor.matmul(pt[:, 0:256],
                                                 wqcv[:, r, 2 * i:2 * i + 2, :],
                                                 x2h3[:, 2 * i:2 * i + 2, c0:c0 + 256],
                                                 start=(i == 0), stop=(i == 3),
                                                 perf_mode=PM.DoubleRow)
                            nc.scalar.activation(qcT[:, r * TQ + c0: r * TQ + c0 + 256],
                                                 pt[:, 0:256], AF.Identity,
                                                 bias=bqc_t[:, r:r + 1],
                                                 scale=1.0 / (A_X * sc["qc"]))
        # xtp/sfp freed here

        # =====================  phase 9-12: cross-attn, out, MLP  ============
        with tc.tile_pool(name="mlpp", bufs=1) as mlpp:
            x3hat = mlpp.tile([128, DC * TQ], F8, tag="x3hat")
            hT = mlpp.tile([128, MC * TQ], F8, tag="hT")
            with tc.tile_pool(name="m1wp", bufs=1) as m1wp:
                wm1all = m1wp.tile([128, MC * DC * 128], F8, tag="wm1")
                nc.sync.dma_start(out=wm1all[:, :], in_=P["wm18"][:, :])
                vcx5 = vcext.rearrange("p (c je) -> p c je", c=5)

                with tc.tile_pool(name="ppc", bufs=2) as ppc, \
                     tc.tile_pool(name="pssc2", bufs=2, space="PSUM") as pssc2, \
                     tc.tile_pool(name="ps2b", bufs=2, space="PSUM") as ps2b:
                    def cross_head(h):
                        hp, hc = (h % 2) * 64, h // 2
                        Pc = ppc.tile([128, 2560], F8, tag="Pc")
                        av = ps2b.tile([65, 512], f32, tag="av")
                        for g in range(2):
                            sps = pssc2.tile([128, 1024], f32, tag="sc")
                            for jj in range(2):
                                c = 2 * g + jj
                                nc.tensor.matmul(
                                    sps[:, jj * 512:(jj + 1) * 512],
                                    kcT[hp:hp + 64, hc * S + c * 128: hc * S + c * 128 + 128],
                                    qcT[hp:hp + 64, hc * TQ:(hc + 1) * TQ],
                                    start=True, stop=True, skip_group_check=True)
                            if g == 1:
                                nc.tensor.matmul(av[:, :],
                                                 vcx5[:, 0:2, h * 65:(h + 1) * 65],
                                                 Pc[:, 0:1024]
                                                 .rearrange("p (two n) -> p two n", two=2),
                                                 start=True, stop=False,
                                                 perf_mode=PM.DoubleRow,
                                                 skip_group_check=True)
                            nc.scalar.activation(Pc[:, g * 1024:(g + 1) * 1024],
                                                 sps[:, :], AF.Exp,
                                                 bias=ln8_t[:, 0:1], scale=SCALE)
                        sps4 = pssc2.tile([128, 1024], f32, tag="sc")
                        nc.tensor.matmul(sps4[0:64, 0:512],
                                         kcT[hp:hp + 64, hc * S + 512: hc * S + 576],
                                         qcT[hp:hp + 64, hc * TQ:(hc + 1) * TQ],
                                         start=True, stop=True, skip_group_check=True)
                        nc.tensor.matmul(av[:, :], vcx5[:, 2:4, h * 65:(h + 1) * 65],
                                         Pc[:, 1024:2048]
                                         .rearrange("p (two n) -> p two n", two=2),
                                         start=False, stop=False,
                                         perf_mode=PM.DoubleRow, skip_group_check=True)
                        nc.scalar.activation(Pc[0:64, 2048:2560], sps4[0:64, 0:512],
                                             AF.Exp, bias=ln8_t[0:64, 0:1], scale=SCALE)
                        nc.tensor.matmul(av[:, :], vcx5[0:64, 4, h * 65:(h + 1) * 65],
                                         Pc[0:64, 2048:2560],
                                         start=False, stop=True, skip_group_check=True)
                        den_sb = rows1.tile([1, 512], f32, tag="densb")
                        nc.scalar.activation(den_sb[:, :], av[64:65, :],
                                             AF.Identity, bias=0.0, scale=1.0)
                        rrow = rows1.tile([1, 512], f32, tag="rrow")
                        nc.vector.reciprocal_approx_fast(rrow[:, :], den_sb[:, :])
                        rb64 = rows.tile([64, 512], f32, tag="rb64")
                        nc.gpsimd.partition_broadcast(rb64[:, :], rrow[:, :])
                        ca_tmp = rows.tile([64, 512], f32, tag="satmp")
                        nc.vector.tensor_mul(ca_tmp[:, :], av[0:64, :], rb64[:, :])
                        nc.scalar.activation(caT[hp:hp + 64, hc * TQ:(hc + 1) * TQ],
                                             ca_tmp[:, :], AF.Identity,
                                             bias=0.0, scale=1.0)

                    for h in range(H):
                        cross_head(h)

                # ---- out proj + residual -> x3T, LN3 stats chase ----
                with tc.tile_pool(name="pstM", bufs=1, space="PSUM") as pstM:
                    st3s = pstM.tile([128, 512], f32, tag="st3s")
                    st3q = pstM.tile([128, 512], f32, tag="st3q")
                    ca3 = caT.rearrange("p (c t) -> p c t", t=TQ)
                    with tc.tile_pool(name="wos", bufs=3) as wos:
                        for r in range(DC):
                            wt = wos.tile([128, DC * 128], F8, tag="wor")
                            nc.sync.dma_start(
                                out=wt[:, :],
                                in_=P["wo8"][:, r * DC * 128:(r + 1) * DC * 128])
                            wv_ = wt.rearrange("p (kc m) -> p kc m", m=128)
                            pt = ps.tile([128, 512], f32, tag="mm")
                            for i in range(4):
                                nc.tensor.matmul(pt[:, :], wv_[:, 2 * i:2 * i + 2, :],
                                                 ca3[:, 2 * i:2 * i + 2, :],
                                                 start=(i == 0), stop=(i == 3),
                                                 perf_mode=PM.DoubleRow)
                            t1 = sb_sm.tile([128, 512], f32, tag="drain")
                            nc.scalar.activation(t1[:, :], pt[:, :], AF.Identity,
                                                 bias=bo_t[:, r:r + 1],
                                                 scale=1.0 / (A_X * sc["o"]))
                            nc.vector.tensor_add(x3T[:, r * TQ:(r + 1) * TQ], t1[:, :],
                                                 x2T[:, r * TQ:(r + 1) * TQ])
                            sq = sb_sm.tile([128, 512], bf16, tag="sqb")
                            nc.scalar.activation(sq[:, :], x3T[:, r * TQ:(r + 1) * TQ],
                                                 AF.Square)
                            nc.tensor.matmul(st3s[0:1, :], ones_bf[:, :],
                                             x3T[:, r * TQ:(r + 1) * TQ],
                                             start=(r == 0), stop=(r == DC - 1),
                                             skip_group_check=True)
                            nc.tensor.matmul(st3q[0:1, :], ones_bf[:, :], sq[:, :],
                                             start=(r == 0), stop=(r == DC - 1),
                                             skip_group_check=True)

                    # ---- LN3 (2 blocks) + mlp1 rows chase ----
                    x3h3 = x3hat.rearrange("p (kc t) -> p kc t", t=TQ)
                    wm1v = wm1all.rearrange("p (r kc m) -> p r kc m", r=MC, m=128)
                    for blk in range(2):
                        c0 = blk * 256
                        rbb = bcp.tile([128, 512], f32, tag="rb")
                        nbb = bcp.tile([128, 512], f32, tag="nb")
                        ln_chain(st3s, st3q, c0, 256, rbb, nbb)
                        for kc in range(DC):
                            ln_norm_chunk(x3hat[:, kc * TQ + c0: kc * TQ + c0 + 256],
                                          x3T[:, kc * TQ + c0: kc * TQ + c0 + 256],
                                          rbb, nbb, 256)
                        for r in range(MC):
                            pt = ps.tile([128, 512], f32, tag="mm")
                            for i in range(4):
                                nc.tensor.matmul(pt[:, 0:256],
                                                 wm1v[:, r, 2 * i:2 * i + 2, :],
                                                 x3h3[:, 2 * i:2 * i + 2, c0:c0 + 256],
                                                 start=(i == 0), stop=(i == 3),
                                                 perf_mode=PM.DoubleRow)
                            nc.scalar.activation(hT[:, r * TQ + c0: r * TQ + c0 + 256],
                                                 pt[:, 0:256], AF.Gelu,
                                                 bias=bm1_t[:, r:r + 1],
                                                 scale=1.0 / (A_X * sc["m1"]))

            # ---- mlp2 + residual -> yT (streamed weights) ----
            hT3 = hT.rearrange("p (kc t) -> p kc t", t=TQ)
            with tc.tile_pool(name="wm2s", bufs=3) as wm2s:
                for r in range(DC):
                    wt = wm2s.tile([128, MC * 128], F8, tag="wm2r")
                    nc.sync.dma_start(
                        out=wt[:, :],
                        in_=P["wm28"][:, r * MC * 128:(r + 1) * MC * 128])
                    wv_ = wt.rearrange("p (kc m) -> p kc m", m=128)
                    pt = ps.tile([128, 512], f32, tag="mm")
                    for i in range(MC // 2):
                        nc.tensor.matmul(pt[:, :], wv_[:, 2 * i:2 * i + 2, :],
                                         hT3[:, 2 * i:2 * i + 2, :],
                                         start=(i == 0), stop=(i == MC // 2 - 1),
                                         perf_mode=PM.DoubleRow)
                    t1 = sb_sm.tile([128, 512], f32, tag="drain")
                    nc.scalar.activation(t1[:, :], pt[:, :], AF.Identity,
                                         bias=bm2_t[:, r:r + 1], scale=1.0 / sc["m2"])
                    yt = sb_sm.tile([128, 512], f32, tag="drain")
                    nc.vector.tensor_add(yt[:, :], t1[:, :], x3T[:, r * TQ:(r + 1) * TQ])
                    nc.sync.dma_start(out=P["yT"][r * 128:(r + 1) * 128, :], in_=yt[:, :])


def _build_program(sc):
    nc = bacc.Bacc()
    P = {}
    P["xT"] = nc.declare_dram_parameter("xT", [D, T], dt.float32r, isOutput=False)
    P["encT8"] = nc.declare_dram_parameter("encT8", [128, EC * S], F8, isOutput=False)
    P["mask3"] = nc.declare_dram_parameter("mask3", [128, 3 * 128], dt.bfloat16, isOutput=False)
    for nm, shp in [("wq8", DC * DC * 128), ("wk8", DC * DC * 128),
                    ("wv8", 2 * DC * 512), ("wp8", DC * DC * 128),
                    ("wqc8", DC * DC * 128), ("wkc8", DC * EC * 128),
                    ("wvc8", 2 * EC * 512), ("wo8", DC * DC * 128),
                    ("wm18", MC * DC * 128), ("wm28", DC * MC * 128)]:
        P[nm] = nc.declare_dram_parameter(nm, [128, shp], F8, isOutput=False)
    for nm, n in [("bq", D), ("bproj", D), ("bqc", D), ("bout", D),
                  ("bm1", DM), ("bm2", D)]:
        P[nm] = nc.declare_dram_parameter(nm, [n, 1], dt.float32, isOutput=False)
    P["yT"] = nc.declare_dram_parameter("yT", [D, TQ], dt.float32, isOutput=True)

    with tile.TileContext(nc) as tc:
        _build_body(nc, tc, P, sc)
    nc.compile()
    return nc


def _get_program(sc):
    if "nc" not in _cached:
        _cached["nc"] = _build_program(sc)
    return _cached["nc"]


last_result = None


def kernel(**inputs):
    global last_result
    import os
    trace = bool(os.environ.get("KERNEL_TRACE"))
    in_maps, metas, sc = _prepare_inputs(**inputs)
    nc = _get_program(sc)
    res = run_bass_kernel_spmd(nc, in_maps, list(range(8)), trace=trace)
    last_result = res
    out = np.empty((B, T, D), dtype=np.float32)
    for c, (b, own_blocks) in enumerate(metas):
        yTc = res.results[c]["yT"]            # [D, TQ]
        yt = yTc.T.reshape(4, 128, D)
        for i, blk in enumerate(own_blocks):
            out[b, blk * 128:(blk + 1) * 128, :] = yt[i]
    return out


# revision 27
# speedup vs baseline: 1.4551x; 1.3485x over previous
"""Trainium2 Bass kernel for a transformer decoder block (self-attn + cross-attn + MLP).

Sharding: data-parallel over (batch, strided query blocks) = 8 shards, no
collectives. Core (b, h) owns query blocks {h, h+2, h+4, h+6} of 128 tokens;
the host permutes tokens so own queries come first. The strided split makes
the causal block structure identical on every core: query block i attends to
own key chunks j<=i and other-half key chunks j<=i, with only the two
diagonal blocks needing masks (a constant triangular mask and a per-core
all-or-nothing flag), applied by tiny PE matmuls into the score PSUM.

Layout: transposed activations [feature partition, token free]. Weights are
pre-tiled on the host to [128, ...] fp8 (e4m3, pow2-scaled) so every weight
DMA is a contiguous 2D copy. Big GEMMs run fp8 DoubleRow (2 x 128 contraction
per pass); QK stays bf16. Softmax: exp(s*scale + ln8) gives 8*P directly in
fp8; denominators come from a ones column in the extended V and are folded
into a per-head reciprocal+broadcast epilogue (no cross-engine round trips).
LayerNorm stats (ones-matmuls + squares) chase the producing projection; the
normalize is pipelined in token blocks so matmuls start while later blocks
normalize.
"""

import sys

sys.path.insert(0, "/opt/trn_rl_repo")

import math
import numpy as np
import ml_dtypes

import concourse.bass as bass
import concourse.bacc as bacc
import concourse.mybir as mybir
from concourse import tile
from concourse.bass_utils import run_bass_kernel_spmd

dt = mybir.dt
AF = mybir.ActivationFunctionType
PM = mybir.MatmulPerfMode

# Problem dims (hardcoded per contest contract)
B, T, D, H, HD = 4, 1024, 1024, 16, 64
S, DE, DM = 576, 768, 4096
TQ = T // 2          # queries per core
DC = D // 128        # feature chunks (8)
EC = DE // 128       # enc feature chunks (6)
MC = DM // 128       # mlp hidden chunks (32)
SCALE = HD ** -0.5
EPS = 1e-5
A_X = 16.0           # xhat / enc fp8 scale
A_P = 8.0            # softmax numerator scale (via exp bias ln A_P)
LN8 = math.log(A_P)
NEG = -1.0e7         # additive mask value

F8NP = ml_dtypes.float8_e4m3
BFNP = ml_dtypes.bfloat16
F8 = dt.float8e4

# which GEMMs use fp8 DoubleRow (others bf16): knobs for accuracy fallback
F8_STAGES = {"qkv", "v", "proj", "qc", "kc", "vc", "av", "avc", "out", "mlp1", "mlp2"}

_cached = {}


def _pow2_scale(w):
    am = float(np.abs(w).max())
    return 2.0 ** int(np.floor(np.log2(240.0 / am)))


def _tile_rows(w, nrow, nkc, m):
    """w [nkc*128, nrow*m] -> [128, nrow*nkc*m] with [p, r, kc, m] order."""
    K, N = w.shape
    assert K == nkc * 128 and N == nrow * m
    wt = w.reshape(nkc, 128, nrow, m).transpose(1, 2, 0, 3)
    return np.ascontiguousarray(wt.reshape(128, nrow * nkc * m))


def _prepare_inputs(x, enc, tgt_key_padding_mask, enc_padding_mask,
                    ln1_w, ln1_b, qkv_w, qkv_b, proj_w, proj_b,
                    ln2_w, ln2_b, q_w, q_b, k_w, k_b, v_w, v_b, out_w, out_b,
                    ln3_w, ln3_b, mlp1_w, mlp1_b, mlp2_w, mlp2_b):
    f32 = np.float32
    asf = lambda a: np.asarray(a, dtype=f32)
    x, enc = asf(x), asf(enc)
    ln1_w, ln1_b, ln2_w, ln2_b, ln3_w, ln3_b = map(asf, (ln1_w, ln1_b, ln2_w, ln2_b, ln3_w, ln3_b))
    qkv_w, qkv_b, proj_w, proj_b = map(asf, (qkv_w, qkv_b, proj_w, proj_b))
    q_w, q_b, k_w, k_b, v_w, v_b, out_w, out_b = map(
        asf, (q_w, q_b, k_w, k_b, v_w, v_b, out_w, out_b))
    mlp1_w, mlp1_b, mlp2_w, mlp2_b = map(asf, (mlp1_w, mlp1_b, mlp2_w, mlp2_b))

    # host-side folds (as baseline): LN affine into weights, k-bias dropped
    # (softmax-invariant), v-biases folded into the following projection bias.
    wqkv_f = qkv_w * ln1_w[:, None]
    bqkv = qkv_b + qkv_w.T @ ln1_b
    b_q = bqkv[0:D]
    b_v = bqkv[2 * D:3 * D]
    bprojf = proj_b + proj_w.T @ b_v
    wqf = q_w * ln2_w[:, None]
    bqcf = q_b + q_w.T @ ln2_b
    boutf = out_b + out_w.T @ v_b
    wm1f = mlp1_w * ln3_w[:, None]
    bm1f = mlp1_b + mlp1_w.T @ ln3_b

    # per-tensor pow2 scales; baked into the compiled program's drain scales
    wq_ = wqkv_f[:, 0:D]; wk_ = wqkv_f[:, D:2 * D]; wv_ = wqkv_f[:, 2 * D:3 * D]
    sc = {
        "q": _pow2_scale(wq_), "k": _pow2_scale(wk_), "v": _pow2_scale(wv_),
        "p": _pow2_scale(proj_w), "qc": _pow2_scale(wqf), "kc": _pow2_scale(k_w),
        "vc": _pow2_scale(v_w), "o": _pow2_scale(out_w),
        "m1": _pow2_scale(wm1f), "m2": _pow2_scale(mlp2_w),
    }
    c8 = lambda w, s: np.ascontiguousarray((w * s).astype(F8NP))
    shared = {
        "wq8": c8(_tile_rows(wq_, DC, DC, 128), sc["q"]),
        "wk8": c8(_tile_rows(wk_, DC, DC, 128), sc["k"]),
        "wv8": c8(_tile_rows(wv_, 2, DC, 512), sc["v"]),
        "wp8": c8(_tile_rows(proj_w, DC, DC, 128), sc["p"]),
        "wqc8": c8(_tile_rows(wqf, DC, DC, 128), sc["qc"]),
        "wkc8": c8(_tile_rows(k_w, DC, EC, 128), sc["kc"]),
        "wvc8": c8(_tile_rows(v_w, 2, EC, 512), sc["vc"]),
        "wo8": c8(_tile_rows(out_w, DC, DC, 128), sc["o"]),
        "wm18": c8(_tile_rows(wm1f, MC, DC, 128), sc["m1"]),
        "wm28": c8(_tile_rows(mlp2_w, DC, MC, 128), sc["m2"]),
    }
    col = lambda v: np.ascontiguousarray(v.reshape(-1, 1).astype(f32))
    shared.update({
        "bq": col(b_q), "bproj": col(bprojf), "bqc": col(bqcf),
        "bout": col(boutf), "bm1": col(bm1f), "bm2": col(mlp2_b),
    })
    # encT pre-tiled fp8*A_X: [128, EC*S]
    encT = enc.transpose(0, 2, 1)  # [B, DE, S]

    # mask tiles [128, 3*128] bf16: [triT | flag | identity]
    # triT[i, j] = M[j, i] where M[key, q] = 0 if key<=q else NEG (same block)
    tri = np.where(np.arange(128)[:, None] <= np.arange(128)[None, :], 0.0, NEG)
    triT = tri.T.astype(BFNP)
    ident = np.eye(128, dtype=BFNP)

    in_maps, metas = [], []
    for c in range(8):
        b, h = c // 2, c % 2
        own_blocks = np.arange(h, 8, 2)
        other_blocks = np.arange(1 - h, 8, 2)
        own = (own_blocks[:, None] * 128 + np.arange(128)[None, :]).reshape(-1)
        other = (other_blocks[:, None] * 128 + np.arange(128)[None, :]).reshape(-1)
        perm = np.concatenate([own, other])
        xT_np = np.ascontiguousarray(x[b][perm].T)  # [D, T] own-first
        enc8 = np.ascontiguousarray(
            (encT[b].reshape(EC, 128, S).transpose(1, 0, 2).reshape(128, EC * S)
             * A_X).astype(F8NP))
        flag = np.full((128, 128), NEG if h == 0 else 0.0, dtype=BFNP)
        mask3 = np.ascontiguousarray(np.concatenate([triT, flag, ident], axis=1))
        im = dict(shared)
        im["xT"] = xT_np
        im["encT8"] = enc8
        im["mask3"] = mask3
        in_maps.append(im)
        metas.append((b, own_blocks))
    return in_maps, metas, sc


def _build_body(nc, tc, P, sc):
    from contextlib import ExitStack
    f32, f32r, bf16 = dt.float32, dt.float32r, dt.bfloat16
    ctx = ExitStack()
    with ctx:
        const = ctx.enter_context(tc.tile_pool(name="const", bufs=1))
        rows = ctx.enter_context(tc.tile_pool(name="rows", bufs=2))
        rows1 = ctx.enter_context(tc.tile_pool(name="rows1", bufs=2))
        bcp = ctx.enter_context(tc.tile_pool(name="bcp", bufs=2))
        sb_sm = ctx.enter_context(tc.tile_pool(name="sb_sm", bufs=3))
        ps = ctx.enter_context(tc.tile_pool(name="ps", bufs=2, space="PSUM"))
        resp = ctx.enter_context(tc.tile_pool(name="resp", bufs=1))

        ones32 = const.tile([128, 1], f32, tag="ones32")
        nc.vector.memset(ones32[:, :], 1.0)
        ones = const.tile([128, 1], f32r, tag="ones")
        nc.scalar.activation(ones[:, :], ones32[:, :], AF.Copy)
        ones_bf = const.tile([128, 1], bf16, tag="ones_bf")
        nc.vector.memset(ones_bf[:, :], 1.0)
        eps2 = const.tile([1, 1], f32, tag="eps2")
        nc.vector.memset(eps2[:, :], EPS / (A_X * A_X))
        ln8_t = const.tile([128, 1], f32, tag="ln8")
        nc.vector.memset(ln8_t[:, :], LN8)
        mask_t = const.tile([128, 384], bf16, tag="mask3")
        triT, flagT, ident = mask_t[:, 0:128], mask_t[:, 128:256], mask_t[:, 256:384]

        def load_bias(drh, nr, tag):
            t = const.tile([128, nr], f32, tag=tag, name="b_" + tag)
            nc.sync.dma_start(out=t.rearrange("p (r one) -> p r one", one=1),
                              in_=drh.rearrange("(r p) one -> p r one", p=128))
            return t

        def load_consts():
            nc.sync.dma_start(out=mask_t[:, :], in_=P["mask3"][:, :])
            return (load_bias(P["bq"], DC, "bq"), load_bias(P["bproj"], DC, "bproj"),
                    load_bias(P["bqc"], DC, "bqc"), load_bias(P["bout"], DC, "bout"),
                    load_bias(P["bm1"], MC, "bm1"), load_bias(P["bm2"], DC, "bm2"))

        # residual stream (bf16) + cross tensors spanning many phases
        x2T = resp.tile([128, DC * TQ], bf16, tag="x2T")
        x3T = resp.tile([128, DC * TQ], bf16, tag="x3T")
        qcT = resp.tile([128, DC * TQ], bf16, tag="qcT")
        kcT = resp.tile([128, DC * S], bf16, tag="kcT")
        vcext = resp.tile([128, 5 * H * 65], F8, tag="vcext")
        caT = resp.tile([128, DC * TQ], F8, tag="caT")

        # ---- LayerNorm helpers (transposed layout, stats via ones-matmul) ----
        def ln_chain(sts, stq, c0, W, rb_t, nb_t):
            R = rows1.tile([1, 2048], f32, tag="lnrow", bufs=1)
            t1n, t2 = R[0:1, 0:W], R[0:1, 512:512 + W]
            t5, msq = R[0:1, 1024:1024 + W], R[0:1, 1536:1536 + W]
            nc.vector.tensor_scalar_mul(t1n, sts[0:1, c0:c0 + W], -1.0 / D)
            nc.vector.tensor_scalar_mul(t2, stq[0:1, c0:c0 + W], 1.0 / D)
            nc.vector.tensor_mul(msq, t1n, t1n)
            nc.vector.tensor_sub(t2, t2, msq)
            nc.scalar.activation(t5, t2, AF.Abs_reciprocal_sqrt,
                                 bias=eps2[0:1, 0:1], scale=1.0 / (A_X * A_X))
            nc.vector.tensor_mul(t1n, t1n, t5)        # -mean * A_X * rstd
            nc.gpsimd.partition_broadcast(rb_t[:, 0:W], t5)
            nc.gpsimd.partition_broadcast(nb_t[:, 0:W], t1n)

        gp_mul = nc.vector.tensor_mul  # keep gpsimd broadcast-only (ucode lib swaps cost ~10us)

        def ln_norm_chunk(dst, xsrc, rb_t, nb_t, W):
            xs = rows.tile([128, 512], f32, tag="xs")
            gp_mul(xs[:, 0:W], xsrc, rb_t[:, 0:W])
            nc.vector.tensor_add(dst, xs[:, 0:W], nb_t[:, 0:W])

        # =====================  phase 1-8: LN1, self-attn, proj, LN2, qc  ====
        with tc.tile_pool(name="xtp", bufs=1) as xtp, \
             tc.tile_pool(name="sfp", bufs=1) as sfp:
            xT_t = xtp.tile([128, DC * T], f32r, tag="xT")
            for kc in range(DC):
                nc.sync.dma_start(out=xT_t[:, kc * T:(kc + 1) * T],
                                  in_=P["xT"][kc * 128:(kc + 1) * 128, :])
            bq_t, bp_t, bqc_t, bo_t, bm1_t, bm2_t = load_consts()
            xhat1 = sfp.tile([128, DC * T], F8, tag="xhat1")
            kT = sfp.tile([128, DC * T], bf16, tag="kT")
            qT = sfp.tile([128, DC * TQ], bf16, tag="qT")
            vext = sfp.tile([128, 8 * H * 65], F8, tag="vext")
            saT = sfp.tile([128, DC * TQ], F8, tag="saT")

            nc.vector.memset(
                vext.rearrange("p (c e) -> p c e", e=65)[:, :, 64:65], 1.0)
            nc.vector.memset(vcext[64:128, 4 * H * 65:5 * H * 65], 0.0)
            nc.vector.memset(
                vcext.rearrange("p (c e) -> p c e", e=65)[:, 0:4 * H, 64:65], 1.0)
            nc.vector.memset(
                vcext.rearrange("p (c e) -> p c e", e=65)[0:64, 4 * H:5 * H, 64:65], 1.0)

            xh3 = xhat1.rearrange("p (kc t) -> p kc t", t=T)

            with tc.tile_pool(name="ckA", bufs=1) as ckA:
                encT_t = ckA.tile([128, EC * S], F8, tag="encT")
                nc.sync.dma_start(out=encT_t[:, :], in_=P["encT8"][:, :])
                wkcall = ckA.tile([128, DC * EC * 128], F8, tag="wkc")
                nc.sync.dma_start(out=wkcall[:, :], in_=P["wkc8"][:, :])
                wvcall = ckA.tile([128, 2 * EC * 512], F8, tag="wvc")
                nc.sync.dma_start(out=wvcall[:, :], in_=P["wvc8"][:, :])
                encv = encT_t.rearrange("p (ec s) -> p ec s", s=S)

                def emit_kc_unit(r, et):
                    pt = ps.tile([128, 512], f32, tag="mm")
                    wv_ = wkcall.rearrange("p (r ec m) -> p r ec m", r=DC, m=128)
                    for i in range(EC // 2):
                        nc.tensor.matmul(pt[:, 0:288], wv_[:, r, 2 * i:2 * i + 2, :],
                                         encv[:, 2 * i:2 * i + 2, et * 288:(et + 1) * 288],
                                         start=(i == 0), stop=(i == 2),
                                         perf_mode=PM.DoubleRow)
                    nc.scalar.activation(kcT[:, r * S + et * 288: r * S + (et + 1) * 288],
                                         pt[:, 0:288], AF.Identity, bias=0.0,
                                         scale=1.0 / (A_X * sc["kc"]))

                kc_units = [(r, et) for r in range(DC) for et in range(2)]
                vxv = vext.rearrange("p (tk j e) -> p tk j e", tk=8, j=H)
                wvcv = wvcall.rearrange("p (vf ec m) -> p vf ec m", vf=2, m=512)
                vcxv = vcext.rearrange("p (tk j e) -> p tk j e", tk=5, j=H)

                def emit_vc_unit(vf, tokc):
                    npart = 128 if tokc < 4 else 64
                    pv = ps.tile([128, 512], f32, tag="mm")
                    for i in range(EC // 2):
                        nc.tensor.matmul(pv[:npart, :],
                                         encv[:, 2 * i:2 * i + 2, tokc * 128:tokc * 128 + npart],
                                         wvcv[:, vf, 2 * i:2 * i + 2, :],
                                         start=(i == 0), stop=(i == 2),
                                         perf_mode=PM.DoubleRow)
                    nc.vector.tensor_scalar_mul(
                        vcxv[:npart, tokc, 8 * vf:8 * vf + 8, 0:64],
                        pv[:npart].rearrange("p (j d) -> p j d", j=8), 1.0 / sc["vc"])


                # ---- LN1 stats (chasing x DMA) with cross-kc interleaved ----
                with tc.tile_pool(name="wqks", bufs=4) as wqks, \
                     tc.tile_pool(name="pst", bufs=1, space="PSUM") as pst:
                    st = {nm: pst.tile([128, 512], f32, tag="st_" + nm,
                                       name="st_" + nm)
                          for nm in ("s0", "s1", "q0", "q1")}
                    for kc in range(DC):
                        for tt in range(2):
                            sq = sb_sm.tile([128, 512], f32r, tag="sq2")
                            nc.scalar.activation(
                                sq[:, :],
                                xT_t[:, kc * T + tt * 512: kc * T + tt * 512 + 512],
                                AF.Square)
                            nc.tensor.matmul(st["s%d" % tt][0:1, :], ones[:, :],
                                             xT_t[:, kc * T + tt * 512: kc * T + tt * 512 + 512],
                                             start=(kc == 0), stop=(kc == DC - 1),
                                             skip_group_check=True)
                            nc.tensor.matmul(st["q%d" % tt][0:1, :], ones[:, :],
                                             sq[:, :],
                                             start=(kc == 0), stop=(kc == DC - 1),
                                             skip_group_check=True)
                    # ---- LN1 tt0 chain + norm, q/k rows chase ----
                    rb0 = bcp.tile([128, 512], f32, tag="rb")
                    nb0 = bcp.tile([128, 512], f32, tag="nb")
                    ln_chain(st["s0"], st["q0"], 0, 512, rb0, nb0)
                    for kc in range(DC):
                        ln_norm_chunk(xhat1[:, kc * T: kc * T + 512],
                                      xT_t[:, kc * T: kc * T + 512], rb0, nb0, 512)
                    while kc_units:
                        emit_kc_unit(*kc_units.pop(0))
                    for vf in range(2):
                        for tokc in range(5):
                            emit_vc_unit(vf, tokc)

                    def qk_row(wdram, r, dst, bias, sscale, tcols):
                        wt = wqks.tile([128, DC * 128], F8, tag="wr")
                        nc.sync.dma_start(out=wt[:, :],
                                          in_=wdram[:, r * DC * 128:(r + 1) * DC * 128])
                        wv_ = wt.rearrange("p (kc m) -> p kc m", m=128)
                        pt = ps.tile([128, 512], f32, tag="mm")
                        for i in range(4):
                            nc.tensor.matmul(pt[:, :], wv_[:, 2 * i:2 * i + 2, :],
                                             xh3[:, 2 * i:2 * i + 2, tcols:tcols + 512],
                                             start=(i == 0), stop=(i == 3),
                                             perf_mode=PM.DoubleRow)
                        if bias is None:
                            nc.scalar.activation(dst, pt[:, :], AF.Identity,
                                                 bias=0.0, scale=sscale)
                        else:
                            nc.scalar.activation(dst, pt[:, :], AF.Identity,
                                                 bias=bias, scale=sscale)

                    for r in range(DC):
                        qk_row(P["wq8"], r, qT[:, r * TQ:(r + 1) * TQ],
                               bq_t[:, r:r + 1], 1.0 / (A_X * sc["q"]), 0)
                        qk_row(P["wk8"], r, kT[:, r * T: r * T + 512],
                               None, 1.0 / (A_X * sc["k"]), 0)

                    # ---- LN1 tt1 chain + norm, k-tt1 rows chase ----
                    rb1 = bcp.tile([128, 512], f32, tag="rb")
                    nb1 = bcp.tile([128, 512], f32, tag="nb")
                    ln_chain(st["s1"], st["q1"], 0, 512, rb1, nb1)
                    for kc in range(DC):
                        ln_norm_chunk(xhat1[:, kc * T + 512: kc * T + 1024],
                                      xT_t[:, kc * T + 512: kc * T + 1024], rb1, nb1, 512)
                    for r in range(DC):
                        qk_row(P["wk8"], r, kT[:, r * T + 512: r * T + 1024],
                               None, 1.0 / (A_X * sc["k"]), 512)

                # ---- V units + self-attention heads ----

                with tc.tile_pool(name="wvp", bufs=1) as wvp:
                    wvall = wvp.tile([128, 2 * DC * 512], F8, tag="wv")
                    nc.sync.dma_start(out=wvall[:, :], in_=P["wv8"][:, :])
                    wvv = wvall.rearrange("p (vf kc m) -> p vf kc m", vf=2, m=512)

                    def emit_v_unit(vf, tokc):
                        pv = ps.tile([128, 512], f32, tag="mm")
                        for i in range(4):
                            nc.tensor.matmul(pv[:, :],
                                             xh3[:, 2 * i:2 * i + 2,
                                                 tokc * 128:(tokc + 1) * 128],
                                             wvv[:, vf, 2 * i:2 * i + 2, :],
                                             start=(i == 0), stop=(i == 3),
                                             perf_mode=PM.DoubleRow)
                        nc.vector.tensor_scalar_mul(
                            vxv[:, tokc, 8 * vf:8 * vf + 8, 0:64],
                            pv.rearrange("p (j d) -> p j d", j=8), 1.0 / sc["v"])

                    for vf in range(2):
                        for tokc in range(8):
                            emit_v_unit(vf, tokc)

                    NOFF = [0, 1024, 1792, 2304]
                    vex5 = vext.rearrange("p (g c je) -> p g c je", g=2, c=4)

                    with tc.tile_pool(name="pp", bufs=2) as pp, \
                         tc.tile_pool(name="pssc", bufs=2, space="PSUM") as pssc, \
                         tc.tile_pool(name="ps2", bufs=2, space="PSUM") as ps2:
                        def self_head(h):
                            hp, hc = (h % 2) * 64, h // 2
                            Pt = pp.tile([128, 2560], F8, tag="P")
                            av = ps2.tile([65, 512], f32, tag="av")
                            pend = None
                            for j in range(4):
                                N = 512 - 128 * j
                                sps = pssc.tile([128, 1024], f32, tag="sc")
                                nc.tensor.matmul(
                                    sps[:, 0:N],
                                    kT[hp:hp + 64, hc * T + j * 128: hc * T + j * 128 + 128],
                                    qT[hp:hp + 64, hc * TQ + j * 128: hc * TQ + TQ],
                                    start=True, stop=False, skip_group_check=True)
                                nc.tensor.matmul(
                                    sps[:, 512:512 + N],
                                    kT[hp:hp + 64, hc * T + 512 + j * 128: hc * T + 512 + j * 128 + 128],
                                    qT[hp:hp + 64, hc * TQ + j * 128: hc * TQ + TQ],
                                    start=True, stop=False, skip_group_check=True)
                                nc.tensor.matmul(sps[:, 0:128], triT, ident,
                                                 start=False, stop=True,
                                                 skip_group_check=True)
                                nc.tensor.matmul(sps[:, 512:640], flagT, ident,
                                                 start=False, stop=True,
                                                 skip_group_check=True)
                                if pend is not None:
                                    jp, Np = pend
                                    nc.tensor.matmul(
                                        av[:, 128 * jp:512],
                                        vex5[:, :, jp, h * 65:(h + 1) * 65],
                                        Pt[:, NOFF[jp]:NOFF[jp] + 2 * Np]
                                        .rearrange("p (two n) -> p two n", two=2),
                                        start=(jp == 0), stop=False,
                                        perf_mode=PM.DoubleRow, skip_group_check=True)
                                nc.scalar.activation(
                                    Pt[:, NOFF[j]:NOFF[j] + 2 * N]
                                    .rearrange("p (two n) -> p two n", two=2),
                                    sps.rearrange("p (two n) -> p two n", two=2)[:, :, 0:N],
                                    AF.Exp, bias=ln8_t[:, 0:1], scale=SCALE)
                                pend = (j, N)
                            jp, Np = pend
                            nc.tensor.matmul(
                                av[:, 128 * jp:512],
                                vex5[:, :, jp, h * 65:(h + 1) * 65],
                                Pt[:, NOFF[jp]:NOFF[jp] + 2 * Np]
                                .rearrange("p (two n) -> p two n", two=2),
                                start=False, stop=True,
                                perf_mode=PM.DoubleRow, skip_group_check=True)
                            # epilogue: saT = av_rows * (1/den)  (= 16*sa in fp8)
                            den_sb = rows1.tile([1, 512], f32, tag="densb")
                            nc.scalar.activation(den_sb[:, :], av[64:65, :],
                                                 AF.Identity, bias=0.0, scale=1.0)
                            rrow = rows1.tile([1, 512], f32, tag="rrow")
                            nc.vector.reciprocal_approx_fast(rrow[:, :], den_sb[:, :])
                            rb64 = rows.tile([64, 512], f32, tag="rb64")
                            nc.gpsimd.partition_broadcast(rb64[:, :], rrow[:, :])
                            sa_tmp = rows.tile([64, 512], f32, tag="satmp")
                            nc.vector.tensor_mul(sa_tmp[:, :], av[0:64, :], rb64[:, :])
                            nc.scalar.activation(saT[hp:hp + 64, hc * TQ:(hc + 1) * TQ],
                                                 sa_tmp[:, :], AF.Identity,
                                                 bias=0.0, scale=1.0)

                        for h in range(H):
                            self_head(h)

            # ---- proj + residual -> x2T, LN2 stats chase, LN2 + qc ----
            with tc.tile_pool(name="pstL", bufs=1, space="PSUM") as pstL:
                st2s = pstL.tile([128, 512], f32, tag="st2s")
                st2q = pstL.tile([128, 512], f32, tag="st2q")
                with tc.tile_pool(name="wpp", bufs=1) as wpp:
                    wpall = wpp.tile([128, DC * DC * 128], F8, tag="wpj")
                    nc.sync.dma_start(out=wpall[:, :], in_=P["wp8"][:, :])
                    sa3 = saT.rearrange("p (c t) -> p c t", t=TQ)
                    wpv = wpall.rearrange("p (r kc m) -> p r kc m", r=DC, m=128)
                    for r in range(DC):
                        pt = ps.tile([128, 512], f32, tag="mm")
                        for i in range(4):
                            nc.tensor.matmul(pt[:, :], wpv[:, r, 2 * i:2 * i + 2, :],
                                             sa3[:, 2 * i:2 * i + 2, :],
                                             start=(i == 0), stop=(i == 3),
                                             perf_mode=PM.DoubleRow)
                        t1 = sb_sm.tile([128, 512], f32, tag="drain")
                        nc.scalar.activation(t1[:, :], pt[:, :], AF.Identity,
                                             bias=bp_t[:, r:r + 1],
                                             scale=1.0 / (A_X * sc["p"]))
                        nc.vector.tensor_add(x2T[:, r * TQ:(r + 1) * TQ], t1[:, :],
                                             xT_t[:, r * T: r * T + TQ].bitcast(f32))
                        sq = sb_sm.tile([128, 512], bf16, tag="sqb")
                        nc.scalar.activation(sq[:, :], x2T[:, r * TQ:(r + 1) * TQ],
                                             AF.Square)
                        nc.tensor.matmul(st2s[0:1, :], ones_bf[:, :],
                                         x2T[:, r * TQ:(r + 1) * TQ],
                                         start=(r == 0), stop=(r == DC - 1),
                                         skip_group_check=True)
                        nc.tensor.matmul(st2q[0:1, :], ones_bf[:, :], sq[:, :],
                                         start=(r == 0), stop=(r == DC - 1),
                                         skip_group_check=True)

                # ---- LN2 (2 blocks) + qc rows chase ----
                with tc.tile_pool(name="qcp", bufs=1) as qcp:
                    x2hat = qcp.tile([128, DC * TQ], F8, tag="x2hat")
                    wqcall = qcp.tile([128, DC * DC * 128], F8, tag="wqc")
                    nc.sync.dma_start(out=wqcall[:, :], in_=P["wqc8"][:, :])
                    x2h3 = x2hat.rearrange("p (kc t) -> p kc t", t=TQ)
                    wqcv = wqcall.rearrange("p (r kc m) -> p r kc m", r=DC, m=128)
                    for blk in range(2):
                        c0 = blk * 256
                        rbb = bcp.tile([128, 512], f32, tag="rb")
                        nbb = bcp.tile([128, 512], f32, tag="nb")
                        ln_chain(st2s, st2q, c0, 256, rbb, nbb)
                        for kc in range(DC):
                            ln_norm_chunk(x2hat[:, kc * TQ + c0: kc * TQ + c0 + 256],
                                          x2T[:, kc * TQ + c0: kc * TQ + c0 + 256],
                                          rbb, nbb, 256)
                        for r in range(DC):
                            pt = ps.tile([128, 512], f32, tag="mm")
                            for i in range(4):
                                nc.tensor.matmul(pt[:, 0:256],
                                                 wqcv[:, r, 2 * i:2 * i + 2, :],
                                                 x2h3[:, 2 * i:2 * i + 2, c0:c0 + 256],
                                                 start=(i == 0), stop=(i == 3),
                                                 perf_mode=PM.DoubleRow)
                            nc.scalar.activation(qcT[:, r * TQ + c0: r * TQ + c0 + 256],
                                                 pt[:, 0:256], AF.Identity,
                                                 bias=bqc_t[:, r:r + 1],
                                                 scale=1.0 / (A_X * sc["qc"]))
        # xtp/sfp freed here

        # =====================  phase 9-12: cross-attn, out, MLP  ============
        with tc.tile_pool(name="mlpp", bufs=1) as mlpp:
            x3hat = mlpp.tile([128, DC * TQ], F8, tag="x3hat")
            hT = mlpp.tile([128, MC * TQ], F8, tag="hT")
            with tc.tile_pool(name="m1wp", bufs=1) as m1wp:
                wm1all = m1wp.tile([128, MC * DC * 128], F8, tag="wm1")
                nc.sync.dma_start(out=wm1all[:, :], in_=P["wm18"][:, :])
                vcx5 = vcext.rearrange("p (c je) -> p c je", c=5)

                with tc.tile_pool(name="ppc", bufs=2) as ppc, \
                     tc.tile_pool(name="pssc2", bufs=2, space="PSUM") as pssc2, \
                     tc.tile_pool(name="ps2b", bufs=2, space="PSUM") as ps2b:
                    def cross_head(h):
                        hp, hc = (h % 2) * 64, h // 2
                        Pc = ppc.tile([128, 2560], F8, tag="Pc")
                        av = ps2b.tile([65, 512], f32, tag="av")
                        for g in range(2):
                            sps = pssc2.tile([128, 1024], f32, tag="sc")
                            for jj in range(2):
                                c = 2 * g + jj
                                nc.tensor.matmul(
                                    sps[:, jj * 512:(jj + 1) * 512],
                                    kcT[hp:hp + 64, hc * S + c * 128: hc * S + c * 128 + 128],
                                    qcT[hp:hp + 64, hc * TQ:(hc + 1) * TQ],
                                    start=True, stop=True, skip_group_check=True)
                            if g == 1:
                                nc.tensor.matmul(av[:, :],
                                                 vcx5[:, 0:2, h * 65:(h + 1) * 65],
                                                 Pc[:, 0:1024]
                                                 .rearrange("p (two n) -> p two n", two=2),
                                                 start=True, stop=False,
                                                 perf_mode=PM.DoubleRow,
                                                 skip_group_check=True)
                            nc.scalar.activation(Pc[:, g * 1024:(g + 1) * 1024],
                                                 sps[:, :], AF.Exp,
                                                 bias=ln8_t[:, 0:1], scale=SCALE)
                        sps4 = pssc2.tile([128, 1024], f32, tag="sc")
                        nc.tensor.matmul(sps4[0:64, 0:512],
                                         kcT[hp:hp + 64, hc * S + 512: hc * S + 576],
                                         qcT[hp:hp + 64, hc * TQ:(hc + 1) * TQ],
                                         start=True, stop=True, skip_group_check=True)
                        nc.tensor.matmul(av[:, :], vcx5[:, 2:4, h * 65:(h + 1) * 65],
                                         Pc[:, 1024:2048]
                                         .rearrange("p (two n) -> p two n", two=2),
                                         start=False, stop=False,
                                         perf_mode=PM.DoubleRow, skip_group_check=True)
                        nc.scalar.activation(Pc[0:64, 2048:2560], sps4[0:64, 0:512],
                                             AF.Exp, bias=ln8_t[0:64, 0:1], scale=SCALE)
                        nc.tensor.matmul(av[:, :], vcx5[0:64, 4, h * 65:(h + 1) * 65],
                                         Pc[0:64, 2048:2560],
                                         start=False, stop=True, skip_group_check=True)
                        den_sb = rows1.tile([1, 512], f32, tag="densb")
                        nc.scalar.activation(den_sb[:, :], av[64:65, :],
                                             AF.Identity, bias=0.0, scale=1.0)
                        rrow = rows1.tile([1, 512], f32, tag="rrow")
                        nc.vector.reciprocal_approx_fast(rrow[:, :], den_sb[:, :])
                        rb64 = rows.tile([64, 512], f32, tag="rb64")
                        nc.gpsimd.partition_broadcast(rb64[:, :], rrow[:, :])
                        ca_tmp = rows.tile([64, 512], f32, tag="satmp")
                        nc.vector.tensor_mul(ca_tmp[:, :], av[0:64, :], rb64[:, :])
                        nc.scalar.activation(caT[hp:hp + 64, hc * TQ:(hc + 1) * TQ],
                                             ca_tmp[:, :], AF.Identity,
                                             bias=0.0, scale=1.0)

                    for h in range(H):
                        cross_head(h)

                # ---- out proj + residual -> x3T, LN3 stats chase ----
                with tc.tile_pool(name="pstM", bufs=1, space="PSUM") as pstM:
                    st3s = pstM.tile([128, 512], f32, tag="st3s")
                    st3q = pstM.tile([128, 512], f32, tag="st3q")
                    ca3 = caT.rearrange("p (c t) -> p c t", t=TQ)
                    with tc.tile_pool(name="wos", bufs=3) as wos:
                        for r in range(DC):
                            wt = wos.tile([128, DC * 128], F8, tag="wor")
                            nc.sync.dma_start(
                                out=wt[:, :],
                                in_=P["wo8"][:, r * DC * 128:(r + 1) * DC * 128])
                            wv_ = wt.rearrange("p (kc m) -> p kc m", m=128)
                            pt = ps.tile([128, 512], f32, tag="mm")
                            for i in range(4):
                                nc.tensor.matmul(pt[:, :], wv_[:, 2 * i:2 * i + 2, :],
                                                 ca3[:, 2 * i:2 * i + 2, :],
                                                 start=(i == 0), stop=(i == 3),
                                                 perf_mode=PM.DoubleRow)
                            t1 = sb_sm.tile([128, 512], f32, tag="drain")
                            nc.scalar.activation(t1[:, :], pt[:, :], AF.Identity,
                                                 bias=bo_t[:, r:r + 1],
                                                 scale=1.0 / (A_X * sc["o"]))
                            nc.vector.tensor_add(x3T[:, r * TQ:(r + 1) * TQ], t1[:, :],
                                                 x2T[:, r * TQ:(r + 1) * TQ])
                            sq = sb_sm.tile([128, 512], bf16, tag="sqb")
                            nc.scalar.activation(sq[:, :], x3T[:, r * TQ:(r + 1) * TQ],
                                                 AF.Square)
                            nc.tensor.matmul(st3s[0:1, :], ones_bf[:, :],
                                             x3T[:, r * TQ:(r + 1) * TQ],
                                             start=(r == 0), stop=(r == DC - 1),
                                             skip_group_check=True)
                            nc.tensor.matmul(st3q[0:1, :], ones_bf[:, :], sq[:, :],
                                             start=(r == 0), stop=(r == DC - 1),
                                             skip_group_check=True)

                    # ---- LN3 (2 blocks) + mlp1 rows chase ----
                    x3h3 = x3hat.rearrange("p (kc t) -> p kc t", t=TQ)
                    wm1v = wm1all.rearrange("p (r kc m) -> p r kc m", r=MC, m=128)
                    for blk in range(2):
                        c0 = blk * 256
                        rbb = bcp.tile([128, 512], f32, tag="rb")
                        nbb = bcp.tile([128, 512], f32, tag="nb")
                        ln_chain(st3s, st3q, c0, 256, rbb, nbb)
                        for kc in range(DC):
                            ln_norm_chunk(x3hat[:, kc * TQ + c0: kc * TQ + c0 + 256],
                                          x3T[:, kc * TQ + c0: kc * TQ + c0 + 256],
                                          rbb, nbb, 256)
                        for r in range(MC):
                            pt = ps.tile([128, 512], f32, tag="mm")
                            for i in range(4):
                                nc.tensor.matmul(pt[:, 0:256],
                                                 wm1v[:, r, 2 * i:2 * i + 2, :],
                                                 x3h3[:, 2 * i:2 * i + 2, c0:c0 + 256],
                                                 start=(i == 0), stop=(i == 3),
                                                 perf_mode=PM.DoubleRow)
                            nc.scalar.activation(hT[:, r * TQ + c0: r * TQ + c0 + 256],
                                                 pt[:, 0:256], AF.Gelu,
                                                 bias=bm1_t[:, r:r + 1],
                                                 scale=1.0 / (A_X * sc["m1"]))

            # ---- mlp2 + residual -> yT (streamed weights) ----
            hT3 = hT.rearrange("p (kc t) -> p kc t", t=TQ)
            with tc.tile_pool(name="wm2s", bufs=3) as wm2s:
                for r in range(DC):
                    wt = wm2s.tile([128, MC * 128], F8, tag="wm2r")
                    nc.sync.dma_start(
                        out=wt[:, :],
                        in_=P["wm28"][:, r * MC * 128:(r + 1) * MC * 128])
                    wv_ = wt.rearrange("p (kc m) -> p kc m", m=128)
                    pt = ps.tile([128, 512], f32, tag="mm")
                    for i in range(MC // 2):
                        nc.tensor.matmul(pt[:, :], wv_[:, 2 * i:2 * i + 2, :],
                                         hT3[:, 2 * i:2 * i + 2, :],
                                         start=(i == 0), stop=(i == MC // 2 - 1),
                                         perf_mode=PM.DoubleRow)
                    t1 = sb_sm.tile([128, 512], f32, tag="drain")
                    nc.scalar.activation(t1[:, :], pt[:, :], AF.Identity,
                                         bias=bm2_t[:, r:r + 1], scale=1.0 / sc["m2"])
                    yt = sb_sm.tile([128, 512], f32, tag="drain")
                    nc.vector.tensor_add(yt[:, :], t1[:, :], x3T[:, r * TQ:(r + 1) * TQ])
                    nc.sync.dma_start(out=P["yT"][r * 128:(r + 1) * 128, :], in_=yt[:, :])


def _build_program(sc):
    nc = bacc.Bacc()
    P = {}
    P["xT"] = nc.declare_dram_parameter("xT", [D, T], dt.float32r, isOutput=False)
    P["encT8"] = nc.declare_dram_parameter("encT8", [128, EC * S], F8, isOutput=False)
    P["mask3"] = nc.declare_dram_parameter("mask3", [128, 3 * 128], dt.bfloat16, isOutput=False)
    for nm, shp in [("wq8", DC * DC * 128), ("wk8", DC * DC * 128),
                    ("wv8", 2 * DC * 512), ("wp8", DC * DC * 128),
                    ("wqc8", DC * DC * 128), ("wkc8", DC * EC * 128),
                    ("wvc8", 2 * EC * 512), ("wo8", DC * DC * 128),
                    ("wm18", MC * DC * 128), ("wm28", DC * MC * 128)]:
        P[nm] = nc.declare_dram_parameter(nm, [128, shp], F8, isOutput=False)
    for nm, n in [("bq", D), ("bproj", D), ("bqc", D), ("bout", D),
                  ("bm1", DM), ("bm2", D)]:
        P[nm] = nc.declare_dram_parameter(nm, [n, 1], dt.float32, isOutput=False)
    P["yT"] = nc.declare_dram_parameter("yT", [D, TQ], dt.float32, isOutput=True)

    with tile.TileContext(nc) as tc:
        _build_body(nc, tc, P, sc)
    nc.compile()
    return nc


def _get_program(sc):
    if "nc" not in _cached:
        _cached["nc"] = _build_program(sc)
    return _cached["nc"]


last_result = None


def kernel(**inputs):
    global last_result
    import os
    trace = bool(os.environ.get("KERNEL_TRACE"))
    in_maps, metas, sc = _prepare_inputs(**inputs)
    nc = _get_program(sc)
    res = run_bass_kernel_spmd(nc, in_maps, list(range(8)), trace=trace)
    last_result = res
    out = np.empty((B, T, D), dtype=np.float32)
    for c, (b, own_blocks) in enumerate(metas):
        yTc = res.results[c]["yT"]            # [D, TQ]
        yt = yTc.T.reshape(4, 128, D)
        for i, blk in enumerate(own_blocks):
            out[b, blk * 128:(blk + 1) * 128, :] = yt[i]
    return out


# revision 29
# speedup vs baseline: 1.4867x; 1.0217x over previous
"""Trainium2 Bass kernel for a transformer decoder block (self-attn + cross-attn + MLP).

Sharding: data-parallel over (batch, strided query blocks) = 8 shards, no
collectives. Core (b, h) owns query blocks {h, h+2, h+4, h+6} of 128 tokens;
the host permutes tokens so own queries come first. The strided split makes
the causal block structure identical on every core: query block i attends to
own key chunks j<=i and other-half key chunks j<=i, with only the two
diagonal blocks needing masks (a constant triangular mask and a per-core
all-or-nothing flag), applied by tiny PE matmuls into the score PSUM.

Layout: transposed activations [feature partition, token free]. Weights are
pre-tiled on the host to [128, ...] fp8 (e4m3, pow2-scaled) so every weight
DMA is a contiguous 2D copy. Big GEMMs run fp8 DoubleRow (2 x 128 contraction
per pass); QK stays bf16. Softmax: exp(s*scale + ln8) gives 8*P directly in
fp8; denominators come from a ones column in the extended V and are folded
into a per-head reciprocal+broadcast epilogue (no cross-engine round trips).
LayerNorm stats (ones-matmuls + squares) chase the producing projection; the
normalize is pipelined in token blocks so matmuls start while later blocks
normalize.
"""

import sys

sys.path.insert(0, "/opt/trn_rl_repo")

import math
import numpy as np
import ml_dtypes

import concourse.bass as bass
import concourse.bacc as bacc
import concourse.mybir as mybir
from concourse import tile
from concourse.bass_utils import run_bass_kernel_spmd

dt = mybir.dt
AF = mybir.ActivationFunctionType
PM = mybir.MatmulPerfMode

# Problem dims (hardcoded per contest contract)
B, T, D, H, HD = 4, 1024, 1024, 16, 64
S, DE, DM = 576, 768, 4096
TQ = T // 2          # queries per core
DC = D // 128        # feature chunks (8)
EC = DE // 128       # enc feature chunks (6)
MC = DM // 128       # mlp hidden chunks (32)
SCALE = HD ** -0.5
EPS = 1e-5
A_X = 16.0           # xhat / enc fp8 scale
A_P = 8.0            # softmax numerator scale (via exp bias ln A_P)
LN8 = math.log(A_P)
NEG = -1.0e7         # additive mask value

F8NP = ml_dtypes.float8_e4m3
BFNP = ml_dtypes.bfloat16
F8 = dt.float8e4

# which GEMMs use fp8 DoubleRow (others bf16): knobs for accuracy fallback
F8_STAGES = {"qkv", "v", "proj", "qc", "kc", "vc", "av", "avc", "out", "mlp1", "mlp2"}

_cached = {}


def _pow2_scale(w):
    am = float(np.abs(w).max())
    return 2.0 ** int(np.floor(np.log2(240.0 / am)))


def _tile_rows(w, nrow, nkc, m):
    """w [nkc*128, nrow*m] -> [128, nrow*nkc*m] with [p, r, kc, m] order."""
    K, N = w.shape
    assert K == nkc * 128 and N == nrow * m
    wt = w.reshape(nkc, 128, nrow, m).transpose(1, 2, 0, 3)
    return np.ascontiguousarray(wt.reshape(128, nrow * nkc * m))


def _prepare_inputs(x, enc, tgt_key_padding_mask, enc_padding_mask,
                    ln1_w, ln1_b, qkv_w, qkv_b, proj_w, proj_b,
                    ln2_w, ln2_b, q_w, q_b, k_w, k_b, v_w, v_b, out_w, out_b,
                    ln3_w, ln3_b, mlp1_w, mlp1_b, mlp2_w, mlp2_b):
    f32 = np.float32
    asf = lambda a: np.asarray(a, dtype=f32)
    x, enc = asf(x), asf(enc)
    ln1_w, ln1_b, ln2_w, ln2_b, ln3_w, ln3_b = map(asf, (ln1_w, ln1_b, ln2_w, ln2_b, ln3_w, ln3_b))
    qkv_w, qkv_b, proj_w, proj_b = map(asf, (qkv_w, qkv_b, proj_w, proj_b))
    q_w, q_b, k_w, k_b, v_w, v_b, out_w, out_b = map(
        asf, (q_w, q_b, k_w, k_b, v_w, v_b, out_w, out_b))
    mlp1_w, mlp1_b, mlp2_w, mlp2_b = map(asf, (mlp1_w, mlp1_b, mlp2_w, mlp2_b))

    # host-side folds (as baseline): LN affine into weights, k-bias dropped
    # (softmax-invariant), v-biases folded into the following projection bias.
    wqkv_f = qkv_w * ln1_w[:, None]
    bqkv = qkv_b + qkv_w.T @ ln1_b
    b_q = bqkv[0:D]
    b_v = bqkv[2 * D:3 * D]
    bprojf = proj_b + proj_w.T @ b_v
    wqf = q_w * ln2_w[:, None]
    bqcf = q_b + q_w.T @ ln2_b
    boutf = out_b + out_w.T @ v_b
    wm1f = mlp1_w * ln3_w[:, None]
    bm1f = mlp1_b + mlp1_w.T @ ln3_b

    # per-tensor pow2 scales; baked into the compiled program's drain scales
    wq_ = wqkv_f[:, 0:D]; wk_ = wqkv_f[:, D:2 * D]; wv_ = wqkv_f[:, 2 * D:3 * D]
    sc = {
        "q": _pow2_scale(wq_), "k": _pow2_scale(wk_), "v": _pow2_scale(wv_),
        "p": _pow2_scale(proj_w), "qc": _pow2_scale(wqf), "kc": _pow2_scale(k_w),
        "vc": _pow2_scale(v_w), "o": _pow2_scale(out_w),
        "m1": _pow2_scale(wm1f), "m2": _pow2_scale(mlp2_w),
    }
    c8 = lambda w, s: np.ascontiguousarray((w * s).astype(F8NP))
    shared = {
        "wq8": c8(_tile_rows(wq_, DC, DC, 128), sc["q"]),
        "wk8": c8(_tile_rows(wk_, DC, DC, 128), sc["k"]),
        "wv8": c8(_tile_rows(wv_, 2, DC, 512), sc["v"]),
        "wp8": c8(_tile_rows(proj_w, DC, DC, 128), sc["p"]),
        "wqc8": c8(_tile_rows(wqf, DC, DC, 128), sc["qc"]),
        "wkc8": c8(_tile_rows(k_w, DC, EC, 128), sc["kc"]),
        "wvc8": c8(_tile_rows(v_w, 2, EC, 512), sc["vc"]),
        "wo8": c8(_tile_rows(out_w, DC, DC, 128), sc["o"]),
        "wm18": c8(_tile_rows(wm1f, MC, DC, 128), sc["m1"]),
        "wm28": c8(_tile_rows(mlp2_w, DC, MC, 128), sc["m2"]),
    }
    col = lambda v: np.ascontiguousarray(v.reshape(-1, 1).astype(f32))
    shared.update({
        "bq": col(b_q), "bproj": col(bprojf), "bqc": col(bqcf),
        "bout": col(boutf), "bm1": col(bm1f), "bm2": col(mlp2_b),
    })
    # encT pre-tiled fp8*A_X: [128, EC*S]
    encT = enc.transpose(0, 2, 1)  # [B, DE, S]

    # mask tiles [128, 3*128] bf16: [triT | flag | identity]
    # triT[i, j] = M[j, i] where M[key, q] = 0 if key<=q else NEG (same block)
    tri = np.where(np.arange(128)[:, None] <= np.arange(128)[None, :], 0.0, NEG)
    triT = tri.T.astype(BFNP)
    ident = np.eye(128, dtype=BFNP)

    in_maps, metas = [], []
    for c in range(8):
        b, h = c // 2, c % 2
        own_blocks = np.arange(h, 8, 2)
        other_blocks = np.arange(1 - h, 8, 2)
        own = (own_blocks[:, None] * 128 + np.arange(128)[None, :]).reshape(-1)
        other = (other_blocks[:, None] * 128 + np.arange(128)[None, :]).reshape(-1)
        perm = np.concatenate([own, other])
        xT_np = np.ascontiguousarray(x[b][perm].T)  # [D, T] own-first
        enc8 = np.ascontiguousarray(
            (encT[b].reshape(EC, 128, S).transpose(1, 0, 2).reshape(128, EC * S)
             * A_X).astype(F8NP))
        flag = np.full((128, 128), NEG if h == 0 else 0.0, dtype=BFNP)
        mask3 = np.ascontiguousarray(np.concatenate([triT, flag, ident], axis=1))
        im = dict(shared)
        im["xT"] = xT_np
        im["encT8"] = enc8
        im["mask3"] = mask3
        in_maps.append(im)
        metas.append((b, own_blocks))
    return in_maps, metas, sc


def _build_body(nc, tc, P, sc):
    from contextlib import ExitStack
    f32, f32r, bf16 = dt.float32, dt.float32r, dt.bfloat16
    ctx = ExitStack()
    with ctx:
        const = ctx.enter_context(tc.tile_pool(name="const", bufs=1))
        rows = ctx.enter_context(tc.tile_pool(name="rows", bufs=2))
        rows1 = ctx.enter_context(tc.tile_pool(name="rows1", bufs=2))
        bcp = ctx.enter_context(tc.tile_pool(name="bcp", bufs=2))
        sb_sm = ctx.enter_context(tc.tile_pool(name="sb_sm", bufs=3))
        ps = ctx.enter_context(tc.tile_pool(name="ps", bufs=2, space="PSUM"))
        resp = ctx.enter_context(tc.tile_pool(name="resp", bufs=1))

        ones32 = const.tile([128, 1], f32, tag="ones32")
        nc.vector.memset(ones32[:, :], 1.0)
        ones = const.tile([128, 1], f32r, tag="ones")
        nc.scalar.activation(ones[:, :], ones32[:, :], AF.Copy)
        ones_bf = const.tile([128, 1], bf16, tag="ones_bf")
        nc.vector.memset(ones_bf[:, :], 1.0)
        eps2 = const.tile([1, 1], f32, tag="eps2")
        nc.vector.memset(eps2[:, :], EPS / (A_X * A_X))
        ln8_t = const.tile([128, 1], f32, tag="ln8")
        nc.vector.memset(ln8_t[:, :], LN8)
        mask_t = const.tile([128, 384], bf16, tag="mask3")
        triT, flagT, ident = mask_t[:, 0:128], mask_t[:, 128:256], mask_t[:, 256:384]

        def load_bias(drh, nr, tag):
            t = const.tile([128, nr], f32, tag=tag, name="b_" + tag)
            nc.sync.dma_start(out=t.rearrange("p (r one) -> p r one", one=1),
                              in_=drh.rearrange("(r p) one -> p r one", p=128))
            return t

        def load_consts():
            nc.sync.dma_start(out=mask_t[:, :], in_=P["mask3"][:, :])
            return (load_bias(P["bq"], DC, "bq"), load_bias(P["bproj"], DC, "bproj"),
                    load_bias(P["bqc"], DC, "bqc"), load_bias(P["bout"], DC, "bout"),
                    load_bias(P["bm1"], MC, "bm1"), load_bias(P["bm2"], DC, "bm2"))

        # residual stream (bf16) + cross tensors spanning many phases
        x2T = resp.tile([128, DC * TQ], bf16, tag="x2T")
        x3T = resp.tile([128, DC * TQ], bf16, tag="x3T")
        qcT = resp.tile([128, DC * TQ], bf16, tag="qcT")
        kcT = resp.tile([128, DC * S], bf16, tag="kcT")
        vcext = resp.tile([128, 5 * H * 65], F8, tag="vcext")
        caT = resp.tile([128, DC * TQ], F8, tag="caT")

        # ---- LayerNorm helpers (transposed layout, stats via ones-matmul) ----
        def ln_chain(sts, stq, c0, W, rb_t, nb_t):
            R = rows1.tile([1, 2048], f32, tag="lnrow", bufs=1)
            t1n, t2 = R[0:1, 0:W], R[0:1, 512:512 + W]
            t5, msq = R[0:1, 1024:1024 + W], R[0:1, 1536:1536 + W]
            nc.vector.tensor_scalar_mul(t1n, sts[0:1, c0:c0 + W], -1.0 / D)
            nc.vector.tensor_scalar_mul(t2, stq[0:1, c0:c0 + W], 1.0 / D)
            nc.vector.tensor_mul(msq, t1n, t1n)
            nc.vector.tensor_sub(t2, t2, msq)
            nc.scalar.activation(t5, t2, AF.Abs_reciprocal_sqrt,
                                 bias=eps2[0:1, 0:1], scale=1.0 / (A_X * A_X))
            nc.vector.tensor_mul(t1n, t1n, t5)        # -mean * A_X * rstd
            nc.gpsimd.partition_broadcast(rb_t[:, 0:W], t5)
            nc.gpsimd.partition_broadcast(nb_t[:, 0:W], t1n)

        gp_mul = nc.vector.tensor_mul  # keep gpsimd broadcast-only (ucode lib swaps cost ~10us)

        def ln_norm_chunk(dst, xsrc, rb_t, nb_t, W):
            xs = rows.tile([128, 512], f32, tag="xs")
            gp_mul(xs[:, 0:W], xsrc, rb_t[:, 0:W])
            nc.vector.tensor_add(dst, xs[:, 0:W], nb_t[:, 0:W])

        # =====================  phase 1-8: LN1, self-attn, proj, LN2, qc  ====
        with tc.tile_pool(name="xtp", bufs=1) as xtp, \
             tc.tile_pool(name="sfp", bufs=1) as sfp:
            xT_t = xtp.tile([128, DC * T], f32r, tag="xT")
            for kc in range(DC):
                nc.sync.dma_start(out=xT_t[:, kc * T:(kc + 1) * T],
                                  in_=P["xT"][kc * 128:(kc + 1) * 128, :])
            bq_t, bp_t, bqc_t, bo_t, bm1_t, bm2_t = load_consts()
            xhat1 = sfp.tile([128, DC * T], F8, tag="xhat1")
            kT = sfp.tile([128, DC * T], bf16, tag="kT")
            qT = sfp.tile([128, DC * TQ], bf16, tag="qT")
            vext = sfp.tile([128, 8 * H * 65], F8, tag="vext")
            saT = sfp.tile([128, DC * TQ], F8, tag="saT")

            nc.vector.memset(
                vext.rearrange("p (c e) -> p c e", e=65)[:, :, 64:65], 1.0)
            nc.vector.memset(vcext[64:128, 4 * H * 65:5 * H * 65], 0.0)
            nc.vector.memset(
                vcext.rearrange("p (c e) -> p c e", e=65)[:, 0:4 * H, 64:65], 1.0)
            nc.vector.memset(
                vcext.rearrange("p (c e) -> p c e", e=65)[0:64, 4 * H:5 * H, 64:65], 1.0)

            xh3 = xhat1.rearrange("p (kc t) -> p kc t", t=T)

            with tc.tile_pool(name="ckA", bufs=1) as ckA:
                encT_t = ckA.tile([128, EC * S], F8, tag="encT")
                nc.sync.dma_start(out=encT_t[:, :], in_=P["encT8"][:, :])
                wkcall = ckA.tile([128, DC * EC * 128], F8, tag="wkc")
                nc.sync.dma_start(out=wkcall[:, :], in_=P["wkc8"][:, :])
                wvcall = ckA.tile([128, 2 * EC * 512], F8, tag="wvc")
                nc.sync.dma_start(out=wvcall[:, :], in_=P["wvc8"][:, :])
                encv = encT_t.rearrange("p (ec s) -> p ec s", s=S)

                def emit_kc_unit(r, et):
                    pt = ps.tile([128, 512], f32, tag="mm")
                    wv_ = wkcall.rearrange("p (r ec m) -> p r ec m", r=DC, m=128)
                    for i in range(EC // 2):
                        nc.tensor.matmul(pt[:, 0:288], wv_[:, r, 2 * i:2 * i + 2, :],
                                         encv[:, 2 * i:2 * i + 2, et * 288:(et + 1) * 288],
                                         start=(i == 0), stop=(i == 2),
                                         perf_mode=PM.DoubleRow)
                    nc.scalar.activation(kcT[:, r * S + et * 288: r * S + (et + 1) * 288],
                                         pt[:, 0:288], AF.Identity, bias=0.0,
                                         scale=1.0 / (A_X * sc["kc"]))

                kc_units = [(r, et) for r in range(DC) for et in range(2)]
                vxv = vext.rearrange("p (tk j e) -> p tk j e", tk=8, j=H)
                wvcv = wvcall.rearrange("p (vf ec m) -> p vf ec m", vf=2, m=512)
                vcxv = vcext.rearrange("p (tk j e) -> p tk j e", tk=5, j=H)

                def emit_vc_unit(vf, tokc):
                    npart = 128 if tokc < 4 else 64
                    pv = ps.tile([128, 512], f32, tag="mm")
                    for i in range(EC // 2):
                        nc.tensor.matmul(pv[:npart, :],
                                         encv[:, 2 * i:2 * i + 2, tokc * 128:tokc * 128 + npart],
                                         wvcv[:, vf, 2 * i:2 * i + 2, :],
                                         start=(i == 0), stop=(i == 2),
                                         perf_mode=PM.DoubleRow)
                    nc.vector.tensor_scalar_mul(
                        vcxv[:npart, tokc, 8 * vf:8 * vf + 8, 0:64],
                        pv[:npart].rearrange("p (j d) -> p j d", j=8), 1.0 / sc["vc"])


                # ---- LN1 stats (chasing x DMA) with cross-kc interleaved ----
                with tc.tile_pool(name="wqks", bufs=4) as wqks, \
                     tc.tile_pool(name="pst", bufs=1, space="PSUM") as pst:
                    st = {nm: pst.tile([128, 512], f32, tag="st_" + nm,
                                       name="st_" + nm)
                          for nm in ("s0", "s1", "q0", "q1")}
                    for kc in range(DC):
                        for _ in range(2):
                            if kc_units:
                                emit_kc_unit(*kc_units.pop(0))
                        for tt in range(2):
                            sq = sb_sm.tile([128, 512], f32r, tag="sq2")
                            nc.scalar.activation(
                                sq[:, :],
                                xT_t[:, kc * T + tt * 512: kc * T + tt * 512 + 512],
                                AF.Square)
                            nc.tensor.matmul(st["s%d" % tt][0:1, :], ones[:, :],
                                             xT_t[:, kc * T + tt * 512: kc * T + tt * 512 + 512],
                                             start=(kc == 0), stop=(kc == DC - 1),
                                             skip_group_check=True)
                            nc.tensor.matmul(st["q%d" % tt][0:1, :], ones[:, :],
                                             sq[:, :],
                                             start=(kc == 0), stop=(kc == DC - 1),
                                             skip_group_check=True)
                    # ---- LN1 tt0 chain + norm, q/k rows chase ----
                    rb0 = bcp.tile([128, 512], f32, tag="rb")
                    nb0 = bcp.tile([128, 512], f32, tag="nb")
                    ln_chain(st["s0"], st["q0"], 0, 512, rb0, nb0)
                    for kc in range(DC):
                        ln_norm_chunk(xhat1[:, kc * T: kc * T + 512],
                                      xT_t[:, kc * T: kc * T + 512], rb0, nb0, 512)
                    while kc_units:
                        emit_kc_unit(*kc_units.pop(0))
                    for vf in range(2):
                        for tokc in range(5):
                            emit_vc_unit(vf, tokc)

                    def qk_row(wdram, r, dst, bias, sscale, tcols):
                        wt = wqks.tile([128, DC * 128], F8, tag="wr")
                        nc.sync.dma_start(out=wt[:, :],
                                          in_=wdram[:, r * DC * 128:(r + 1) * DC * 128])
                        wv_ = wt.rearrange("p (kc m) -> p kc m", m=128)
                        pt = ps.tile([128, 512], f32, tag="mm")
                        for i in range(4):
                            nc.tensor.matmul(pt[:, :], wv_[:, 2 * i:2 * i + 2, :],
                                             xh3[:, 2 * i:2 * i + 2, tcols:tcols + 512],
                                             start=(i == 0), stop=(i == 3),
                                             perf_mode=PM.DoubleRow)
                        if bias is None:
                            nc.vector.tensor_scalar_mul(dst, pt[:, :], sscale)
                        else:
                            nc.scalar.activation(dst, pt[:, :], AF.Identity,
                                                 bias=bias, scale=sscale)

                    for r in range(DC):
                        qk_row(P["wq8"], r, qT[:, r * TQ:(r + 1) * TQ],
                               bq_t[:, r:r + 1], 1.0 / (A_X * sc["q"]), 0)
                        qk_row(P["wk8"], r, kT[:, r * T: r * T + 512],
                               None, 1.0 / (A_X * sc["k"]), 0)

                    # ---- LN1 tt1 chain + norm, k-tt1 rows chase ----
                    rb1 = bcp.tile([128, 512], f32, tag="rb")
                    nb1 = bcp.tile([128, 512], f32, tag="nb")
                    ln_chain(st["s1"], st["q1"], 0, 512, rb1, nb1)
                    for kc in range(DC):
                        ln_norm_chunk(xhat1[:, kc * T + 512: kc * T + 1024],
                                      xT_t[:, kc * T + 512: kc * T + 1024], rb1, nb1, 512)
                    for r in range(DC):
                        qk_row(P["wk8"], r, kT[:, r * T + 512: r * T + 1024],
                               None, 1.0 / (A_X * sc["k"]), 512)

                # ---- V units + self-attention heads ----

                with tc.tile_pool(name="wvp", bufs=1) as wvp:
                    wvall = wvp.tile([128, 2 * DC * 512], F8, tag="wv")
                    nc.sync.dma_start(out=wvall[:, :], in_=P["wv8"][:, :])
                    wvv = wvall.rearrange("p (vf kc m) -> p vf kc m", vf=2, m=512)

                    def emit_v_unit(vf, tokc):
                        pv = ps.tile([128, 512], f32, tag="mm")
                        for i in range(4):
                            nc.tensor.matmul(pv[:, :],
                                             xh3[:, 2 * i:2 * i + 2,
                                                 tokc * 128:(tokc + 1) * 128],
                                             wvv[:, vf, 2 * i:2 * i + 2, :],
                                             start=(i == 0), stop=(i == 3),
                                             perf_mode=PM.DoubleRow)
                        nc.vector.tensor_scalar_mul(
                            vxv[:, tokc, 8 * vf:8 * vf + 8, 0:64],
                            pv.rearrange("p (j d) -> p j d", j=8), 1.0 / sc["v"])

                    for vf in range(2):
                        for tokc in range(8):
                            emit_v_unit(vf, tokc)

                    NOFF = [0, 1024, 1792, 2304]
                    vex5 = vext.rearrange("p (g c je) -> p g c je", g=2, c=4)

                    with tc.tile_pool(name="pp", bufs=2) as pp, \
                         tc.tile_pool(name="pssc", bufs=2, space="PSUM") as pssc, \
                         tc.tile_pool(name="ps2", bufs=2, space="PSUM") as ps2:
                        def self_head(h):
                            hp, hc = (h % 2) * 64, h // 2
                            Pt = pp.tile([128, 2560], F8, tag="P")
                            av = ps2.tile([65, 512], f32, tag="av")
                            pend = None
                            for j in range(4):
                                N = 512 - 128 * j
                                sps = pssc.tile([128, 1024], f32, tag="sc")
                                nc.tensor.matmul(
                                    sps[:, 0:N],
                                    kT[hp:hp + 64, hc * T + j * 128: hc * T + j * 128 + 128],
                                    qT[hp:hp + 64, hc * TQ + j * 128: hc * TQ + TQ],
                                    start=True, stop=False, skip_group_check=True)
                                nc.tensor.matmul(
                                    sps[:, 512:512 + N],
                                    kT[hp:hp + 64, hc * T + 512 + j * 128: hc * T + 512 + j * 128 + 128],
                                    qT[hp:hp + 64, hc * TQ + j * 128: hc * TQ + TQ],
                                    start=True, stop=False, skip_group_check=True)
                                nc.tensor.matmul(sps[:, 0:128], triT, ident,
                                                 start=False, stop=True,
                                                 skip_group_check=True)
                                nc.tensor.matmul(sps[:, 512:640], flagT, ident,
                                                 start=False, stop=True,
                                                 skip_group_check=True)
                                if pend is not None:
                                    jp, Np = pend
                                    nc.tensor.matmul(
                                        av[:, 128 * jp:512],
                                        vex5[:, :, jp, h * 65:(h + 1) * 65],
                                        Pt[:, NOFF[jp]:NOFF[jp] + 2 * Np]
                                        .rearrange("p (two n) -> p two n", two=2),
                                        start=(jp == 0), stop=False,
                                        perf_mode=PM.DoubleRow, skip_group_check=True)
                                nc.scalar.activation(
                                    Pt[:, NOFF[j]:NOFF[j] + 2 * N]
                                    .rearrange("p (two n) -> p two n", two=2),
                                    sps.rearrange("p (two n) -> p two n", two=2)[:, :, 0:N],
                                    AF.Exp, bias=ln8_t[:, 0:1], scale=SCALE)
                                pend = (j, N)
                            jp, Np = pend
                            nc.tensor.matmul(
                                av[:, 128 * jp:512],
                                vex5[:, :, jp, h * 65:(h + 1) * 65],
                                Pt[:, NOFF[jp]:NOFF[jp] + 2 * Np]
                                .rearrange("p (two n) -> p two n", two=2),
                                start=False, stop=True,
                                perf_mode=PM.DoubleRow, skip_group_check=True)
                            # epilogue: saT = av_rows * (1/den)  (= 16*sa in fp8)
                            den_sb = rows1.tile([1, 512], f32, tag="densb")
                            nc.scalar.activation(den_sb[:, :], av[64:65, :],
                                                 AF.Identity, bias=0.0, scale=1.0)
                            rrow = rows1.tile([1, 512], f32, tag="rrow")
                            nc.vector.reciprocal_approx_fast(rrow[:, :], den_sb[:, :])
                            rb64 = rows.tile([64, 512], f32, tag="rb64")
                            nc.gpsimd.partition_broadcast(rb64[:, :], rrow[:, :])
                            nc.vector.tensor_mul(saT[hp:hp + 64, hc * TQ:(hc + 1) * TQ],
                                                 av[0:64, :], rb64[:, :])

                        for h in range(H):
                            self_head(h)

            # ---- proj + residual -> x2T, LN2 stats chase, LN2 + qc ----
            with tc.tile_pool(name="pstL", bufs=1, space="PSUM") as pstL:
                st2s = pstL.tile([128, 512], f32, tag="st2s")
                st2q = pstL.tile([128, 512], f32, tag="st2q")
                with tc.tile_pool(name="wpp", bufs=1) as wpp:
                    wpall = wpp.tile([128, DC * DC * 128], F8, tag="wpj")
                    nc.sync.dma_start(out=wpall[:, :], in_=P["wp8"][:, :])
                    sa3 = saT.rearrange("p (c t) -> p c t", t=TQ)
                    wpv = wpall.rearrange("p (r kc m) -> p r kc m", r=DC, m=128)
                    for r in range(DC):
                        pt = ps.tile([128, 512], f32, tag="mm")
                        for i in range(4):
                            nc.tensor.matmul(pt[:, :], wpv[:, r, 2 * i:2 * i + 2, :],
                                             sa3[:, 2 * i:2 * i + 2, :],
                                             start=(i == 0), stop=(i == 3),
                                             perf_mode=PM.DoubleRow)
                        t1 = sb_sm.tile([128, 512], f32, tag="drain")
                        nc.scalar.activation(t1[:, :], pt[:, :], AF.Identity,
                                             bias=bp_t[:, r:r + 1],
                                             scale=1.0 / (A_X * sc["p"]))
                        nc.vector.tensor_add(x2T[:, r * TQ:(r + 1) * TQ], t1[:, :],
                                             xT_t[:, r * T: r * T + TQ].bitcast(f32))
                        sq = sb_sm.tile([128, 512], bf16, tag="sqb")
                        nc.scalar.activation(sq[:, :], x2T[:, r * TQ:(r + 1) * TQ],
                                             AF.Square)
                        nc.tensor.matmul(st2s[0:1, :], ones_bf[:, :],
                                         x2T[:, r * TQ:(r + 1) * TQ],
                                         start=(r == 0), stop=(r == DC - 1),
                                         skip_group_check=True)
                        nc.tensor.matmul(st2q[0:1, :], ones_bf[:, :], sq[:, :],
                                         start=(r == 0), stop=(r == DC - 1),
                                         skip_group_check=True)

                # ---- LN2 (2 blocks) + qc rows chase ----
                with tc.tile_pool(name="qcp", bufs=1) as qcp:
                    x2hat = qcp.tile([128, DC * TQ], F8, tag="x2hat")
                    wqcall = qcp.tile([128, DC * DC * 128], F8, tag="wqc")
                    nc.sync.dma_start(out=wqcall[:, :], in_=P["wqc8"][:, :])
                    x2h3 = x2hat.rearrange("p (kc t) -> p kc t", t=TQ)
                    wqcv = wqcall.rearrange("p (r kc m) -> p r kc m", r=DC, m=128)
                    for blk in range(2):
                        c0 = blk * 256
                        rbb = bcp.tile([128, 512], f32, tag="rb")
                        nbb = bcp.tile([128, 512], f32, tag="nb")
                        ln_chain(st2s, st2q, c0, 256, rbb, nbb)
                        for kc in range(DC):
                            ln_norm_chunk(x2hat[:, kc * TQ + c0: kc * TQ + c0 + 256],
                                          x2T[:, kc * TQ + c0: kc * TQ + c0 + 256],
                                          rbb, nbb, 256)
                        for r in range(DC):
                            pt = ps.tile([128, 512], f32, tag="mm")
                            for i in range(4):
                                nc.tensor.matmul(pt[:, 0:256],
                                                 wqcv[:, r, 2 * i:2 * i + 2, :],
                                                 x2h3[:, 2 * i:2 * i + 2, c0:c0 + 256],
                                                 start=(i == 0), stop=(i == 3),
                                                 perf_mode=PM.DoubleRow)
                            nc.scalar.activation(qcT[:, r * TQ + c0: r * TQ + c0 + 256],
                                                 pt[:, 0:256], AF.Identity,
                                                 bias=bqc_t[:, r:r + 1],
                                                 scale=1.0 / (A_X * sc["qc"]))
        # xtp/sfp freed here

        # =====================  phase 9-12: cross-attn, out, MLP  ============
        with tc.tile_pool(name="mlpp", bufs=1) as mlpp:
            x3hat = mlpp.tile([128, DC * TQ], F8, tag="x3hat")
            hT = mlpp.tile([128, MC * TQ], F8, tag="hT")
            with tc.tile_pool(name="m1wp", bufs=1) as m1wp:
                wm1all = m1wp.tile([128, MC * DC * 128], F8, tag="wm1")
                nc.sync.dma_start(out=wm1all[:, :], in_=P["wm18"][:, :])
                vcx5 = vcext.rearrange("p (c je) -> p c je", c=5)

                with tc.tile_pool(name="ppc", bufs=2) as ppc, \
                     tc.tile_pool(name="pssc2", bufs=2, space="PSUM") as pssc2, \
                     tc.tile_pool(name="ps2b", bufs=2, space="PSUM") as ps2b:
                    def cross_head(h):
                        hp, hc = (h % 2) * 64, h // 2
                        Pc = ppc.tile([128, 2560], F8, tag="Pc")
                        av = ps2b.tile([65, 512], f32, tag="av")
                        for g in range(2):
                            sps = pssc2.tile([128, 1024], f32, tag="sc")
                            for jj in range(2):
                                c = 2 * g + jj
                                nc.tensor.matmul(
                                    sps[:, jj * 512:(jj + 1) * 512],
                                    kcT[hp:hp + 64, hc * S + c * 128: hc * S + c * 128 + 128],
                                    qcT[hp:hp + 64, hc * TQ:(hc + 1) * TQ],
                                    start=True, stop=True, skip_group_check=True)
                            if g == 1:
                                nc.tensor.matmul(av[:, :],
                                                 vcx5[:, 0:2, h * 65:(h + 1) * 65],
                                                 Pc[:, 0:1024]
                                                 .rearrange("p (two n) -> p two n", two=2),
                                                 start=True, stop=False,
                                                 perf_mode=PM.DoubleRow,
                                                 skip_group_check=True)
                            nc.scalar.activation(Pc[:, g * 1024:(g + 1) * 1024],
                                                 sps[:, :], AF.Exp,
                                                 bias=ln8_t[:, 0:1], scale=SCALE)
                        sps4 = pssc2.tile([128, 1024], f32, tag="sc")
                        nc.tensor.matmul(sps4[0:64, 0:512],
                                         kcT[hp:hp + 64, hc * S + 512: hc * S + 576],
                                         qcT[hp:hp + 64, hc * TQ:(hc + 1) * TQ],
                                         start=True, stop=True, skip_group_check=True)
                        nc.tensor.matmul(av[:, :], vcx5[:, 2:4, h * 65:(h + 1) * 65],
                                         Pc[:, 1024:2048]
                                         .rearrange("p (two n) -> p two n", two=2),
                                         start=False, stop=False,
                                         perf_mode=PM.DoubleRow, skip_group_check=True)
                        nc.scalar.activation(Pc[0:64, 2048:2560], sps4[0:64, 0:512],
                                             AF.Exp, bias=ln8_t[0:64, 0:1], scale=SCALE)
                        nc.tensor.matmul(av[:, :], vcx5[0:64, 4, h * 65:(h + 1) * 65],
                                         Pc[0:64, 2048:2560],
                                         start=False, stop=True, skip_group_check=True)
                        den_sb = rows1.tile([1, 512], f32, tag="densb")
                        nc.scalar.activation(den_sb[:, :], av[64:65, :],
                                             AF.Identity, bias=0.0, scale=1.0)
                        rrow = rows1.tile([1, 512], f32, tag="rrow")
                        nc.vector.reciprocal_approx_fast(rrow[:, :], den_sb[:, :])
                        rb64 = rows.tile([64, 512], f32, tag="rb64")
                        nc.gpsimd.partition_broadcast(rb64[:, :], rrow[:, :])
                        nc.vector.tensor_mul(caT[hp:hp + 64, hc * TQ:(hc + 1) * TQ],
                                             av[0:64, :], rb64[:, :])

                    for h in range(H):
                        cross_head(h)

                # ---- out proj + residual -> x3T, LN3 stats chase ----
                with tc.tile_pool(name="pstM", bufs=1, space="PSUM") as pstM:
                    st3s = pstM.tile([128, 512], f32, tag="st3s")
                    st3q = pstM.tile([128, 512], f32, tag="st3q")
                    ca3 = caT.rearrange("p (c t) -> p c t", t=TQ)
                    with tc.tile_pool(name="wos", bufs=3) as wos:
                        for r in range(DC):
                            wt = wos.tile([128, DC * 128], F8, tag="wor")
                            nc.sync.dma_start(
                                out=wt[:, :],
                                in_=P["wo8"][:, r * DC * 128:(r + 1) * DC * 128])
                            wv_ = wt.rearrange("p (kc m) -> p kc m", m=128)
                            pt = ps.tile([128, 512], f32, tag="mm")
                            for i in range(4):
                                nc.tensor.matmul(pt[:, :], wv_[:, 2 * i:2 * i + 2, :],
                                                 ca3[:, 2 * i:2 * i + 2, :],
                                                 start=(i == 0), stop=(i == 3),
                                                 perf_mode=PM.DoubleRow)
                            t1 = sb_sm.tile([128, 512], f32, tag="drain")
                            nc.scalar.activation(t1[:, :], pt[:, :], AF.Identity,
                                                 bias=bo_t[:, r:r + 1],
                                                 scale=1.0 / (A_X * sc["o"]))
                            nc.vector.tensor_add(x3T[:, r * TQ:(r + 1) * TQ], t1[:, :],
                                                 x2T[:, r * TQ:(r + 1) * TQ])
                            sq = sb_sm.tile([128, 512], bf16, tag="sqb")
                            nc.scalar.activation(sq[:, :], x3T[:, r * TQ:(r + 1) * TQ],
                                                 AF.Square)
                            nc.tensor.matmul(st3s[0:1, :], ones_bf[:, :],
                                             x3T[:, r * TQ:(r + 1) * TQ],
                                             start=(r == 0), stop=(r == DC - 1),
                                             skip_group_check=True)
                            nc.tensor.matmul(st3q[0:1, :], ones_bf[:, :], sq[:, :],
                                             start=(r == 0), stop=(r == DC - 1),
                                             skip_group_check=True)

                    # ---- LN3 (2 blocks) + mlp1 rows chase ----
                    x3h3 = x3hat.rearrange("p (kc t) -> p kc t", t=TQ)
                    wm1v = wm1all.rearrange("p (r kc m) -> p r kc m", r=MC, m=128)
                    rbb = bcp.tile([128, 512], f32, tag="rb")
                    nbb = bcp.tile([128, 512], f32, tag="nb")
                    ln_chain(st3s, st3q, 0, 512, rbb, nbb)
                    for kc in range(DC):
                        ln_norm_chunk(x3hat[:, kc * TQ: (kc + 1) * TQ],
                                      x3T[:, kc * TQ: (kc + 1) * TQ],
                                      rbb, nbb, 512)
                    for r in range(MC):
                        pt = ps.tile([128, 512], f32, tag="mm")
                        for i in range(4):
                            nc.tensor.matmul(pt[:, :],
                                             wm1v[:, r, 2 * i:2 * i + 2, :],
                                             x3h3[:, 2 * i:2 * i + 2, :],
                                             start=(i == 0), stop=(i == 3),
                                             perf_mode=PM.DoubleRow)
                        nc.scalar.activation(hT[:, r * TQ: (r + 1) * TQ],
                                             pt[:, :], AF.Gelu,
                                             bias=bm1_t[:, r:r + 1],
                                             scale=1.0 / (A_X * sc["m1"]))

            # ---- mlp2 + residual -> yT (streamed weights) ----
            hT3 = hT.rearrange("p (kc t) -> p kc t", t=TQ)
            with tc.tile_pool(name="wm2s", bufs=3) as wm2s:
                for r in range(DC):
                    wt = wm2s.tile([128, MC * 128], F8, tag="wm2r")
                    nc.sync.dma_start(
                        out=wt[:, :],
                        in_=P["wm28"][:, r * MC * 128:(r + 1) * MC * 128])
                    wv_ = wt.rearrange("p (kc m) -> p kc m", m=128)
                    pt = ps.tile([128, 512], f32, tag="mm")
                    for i in range(MC // 2):
                        nc.tensor.matmul(pt[:, :], wv_[:, 2 * i:2 * i + 2, :],
                                         hT3[:, 2 * i:2 * i + 2, :],
                                         start=(i == 0), stop=(i == MC // 2 - 1),
                                         perf_mode=PM.DoubleRow)
                    t1 = sb_sm.tile([128, 512], f32, tag="drain")
                    nc.scalar.activation(t1[:, :], pt[:, :], AF.Identity,
                                         bias=bm2_t[:, r:r + 1], scale=1.0 / sc["m2"])
                    yt = sb_sm.tile([128, 512], f32, tag="drain")
                    nc.vector.tensor_add(yt[:, :], t1[:, :], x3T[:, r * TQ:(r + 1) * TQ])
                    nc.sync.dma_start(out=P["yT"][r * 128:(r + 1) * 128, :], in_=yt[:, :])


def _build_program(sc):
    nc = bacc.Bacc()
    P = {}
    P["xT"] = nc.declare_dram_parameter("xT", [D, T], dt.float32r, isOutput=False)
    P["encT8"] = nc.declare_dram_parameter("encT8", [128, EC * S], F8, isOutput=False)
    P["mask3"] = nc.declare_dram_parameter("mask3", [128, 3 * 128], dt.bfloat16, isOutput=False)
    for nm, shp in [("wq8", DC * DC * 128), ("wk8", DC * DC * 128),
                    ("wv8", 2 * DC * 512), ("wp8", DC * DC * 128),
                    ("wqc8", DC * DC * 128), ("wkc8", DC * EC * 128),
                    ("wvc8", 2 * EC * 512), ("wo8", DC * DC * 128),
                    ("wm18", MC * DC * 128), ("wm28", DC * MC * 128)]:
        P[nm] = nc.declare_dram_parameter(nm, [128, shp], F8, isOutput=False)
    for nm, n in [("bq", D), ("bproj", D), ("bqc", D), ("bout", D),
                  ("bm1", DM), ("bm2", D)]:
        P[nm] = nc.declare_dram_parameter(nm, [n, 1], dt.float32, isOutput=False)
    P["yT"] = nc.declare_dram_parameter("yT", [D, TQ], dt.float32, isOutput=True)

    with tile.TileContext(nc) as tc:
        _build_body(nc, tc, P, sc)
    nc.compile()
    return nc


def _get_program(sc):
    if "nc" not in _cached:
        _cached["nc"] = _build_program(sc)
    return _cached["nc"]


last_result = None


def kernel(**inputs):
    global last_result
    import os
    trace = bool(os.environ.get("KERNEL_TRACE"))
    in_maps, metas, sc = _prepare_inputs(**inputs)
    nc = _get_program(sc)
    res = run_bass_kernel_spmd(nc, in_maps, list(range(8)), trace=trace)
    last_result = res
    out = np.empty((B, T, D), dtype=np.float32)
    for c, (b, own_blocks) in enumerate(metas):
        yTc = res.results[c]["yT"]            # [D, TQ]
        yt = yTc.T.reshape(4, 128, D)
        for i, blk in enumerate(own_blocks):
            out[b, blk * 128:(blk + 1) * 128, :] = yt[i]
    return out


# revision 30
# speedup vs baseline: 1.5819x; 1.0640x over previous
"""Trainium2 Bass kernel for a transformer decoder block (self-attn + cross-attn + MLP).

Sharding: data-parallel over (batch, strided query blocks) = 8 shards, no
collectives. Core (b, h) owns query blocks {h, h+2, h+4, h+6} of 128 tokens;
the host permutes tokens so own queries come first. The strided split makes
the causal block structure identical on every core: query block i attends to
own key chunks j<=i and other-half key chunks j<=i, with only the two
diagonal blocks needing masks (a constant triangular mask and a per-core
all-or-nothing flag), applied by tiny PE matmuls into the score PSUM.

Layout: transposed activations [feature partition, token free]. Weights are
pre-tiled on the host to [128, ...] fp8 (e4m3, pow2-scaled) so every weight
DMA is a contiguous 2D copy. Big GEMMs run fp8 DoubleRow (2 x 128 contraction
per pass); QK stays bf16. Softmax: exp(s*scale + ln8) gives 8*P directly in
fp8; denominators come from a ones column in the extended V and are folded
into a per-head reciprocal+broadcast epilogue (no cross-engine round trips).
LayerNorm stats (ones-matmuls + squares) chase the producing projection; the
normalize is pipelined in token blocks so matmuls start while later blocks
normalize.
"""

import sys

sys.path.insert(0, "/opt/trn_rl_repo")

import math
import numpy as np
import ml_dtypes

import concourse.bass as bass
import concourse.bacc as bacc
import concourse.mybir as mybir
from concourse import tile
from concourse.bass_utils import run_bass_kernel_spmd

dt = mybir.dt
AF = mybir.ActivationFunctionType
PM = mybir.MatmulPerfMode

# Problem dims (hardcoded per contest contract)
B, T, D, H, HD = 4, 1024, 1024, 16, 64
S, DE, DM = 576, 768, 4096
TQ = T // 2          # queries per core
DC = D // 128        # feature chunks (8)
EC = DE // 128       # enc feature chunks (6)
MC = DM // 128       # mlp hidden chunks (32)
SCALE = HD ** -0.5
EPS = 1e-5
A_X = 16.0           # xhat / enc fp8 scale
A_P = 8.0            # softmax numerator scale (via exp bias ln A_P)
LN8 = math.log(A_P)
NEG = -1.0e7         # additive mask value

F8NP = ml_dtypes.float8_e4m3
BFNP = ml_dtypes.bfloat16
F8 = dt.float8e4

# which GEMMs use fp8 DoubleRow (others bf16): knobs for accuracy fallback
F8_STAGES = {"qkv", "v", "proj", "qc", "kc", "vc", "av", "avc", "out", "mlp1", "mlp2"}

_cached = {}


def _pow2_scale(w):
    am = float(np.abs(w).max())
    return 2.0 ** int(np.floor(np.log2(240.0 / am)))


def _tile_rows(w, nrow, nkc, m):
    """w [nkc*128, nrow*m] -> [128, nrow*nkc*m] with [p, r, kc, m] order."""
    K, N = w.shape
    assert K == nkc * 128 and N == nrow * m
    wt = w.reshape(nkc, 128, nrow, m).transpose(1, 2, 0, 3)
    return np.ascontiguousarray(wt.reshape(128, nrow * nkc * m))


def _prepare_inputs(x, enc, tgt_key_padding_mask, enc_padding_mask,
                    ln1_w, ln1_b, qkv_w, qkv_b, proj_w, proj_b,
                    ln2_w, ln2_b, q_w, q_b, k_w, k_b, v_w, v_b, out_w, out_b,
                    ln3_w, ln3_b, mlp1_w, mlp1_b, mlp2_w, mlp2_b):
    f32 = np.float32
    asf = lambda a: np.asarray(a, dtype=f32)
    x, enc = asf(x), asf(enc)
    ln1_w, ln1_b, ln2_w, ln2_b, ln3_w, ln3_b = map(asf, (ln1_w, ln1_b, ln2_w, ln2_b, ln3_w, ln3_b))
    qkv_w, qkv_b, proj_w, proj_b = map(asf, (qkv_w, qkv_b, proj_w, proj_b))
    q_w, q_b, k_w, k_b, v_w, v_b, out_w, out_b = map(
        asf, (q_w, q_b, k_w, k_b, v_w, v_b, out_w, out_b))
    mlp1_w, mlp1_b, mlp2_w, mlp2_b = map(asf, (mlp1_w, mlp1_b, mlp2_w, mlp2_b))

    # host-side folds (as baseline): LN affine into weights, k-bias dropped
    # (softmax-invariant), v-biases folded into the following projection bias.
    wqkv_f = qkv_w * ln1_w[:, None]
    bqkv = qkv_b + qkv_w.T @ ln1_b
    b_q = bqkv[0:D]
    b_v = bqkv[2 * D:3 * D]
    bprojf = proj_b + proj_w.T @ b_v
    wqf = q_w * ln2_w[:, None]
    bqcf = q_b + q_w.T @ ln2_b
    boutf = out_b + out_w.T @ v_b
    wm1f = mlp1_w * ln3_w[:, None]
    bm1f = mlp1_b + mlp1_w.T @ ln3_b

    # per-tensor pow2 scales; baked into the compiled program's drain scales
    wq_ = wqkv_f[:, 0:D]; wk_ = wqkv_f[:, D:2 * D]; wv_ = wqkv_f[:, 2 * D:3 * D]
    sc = {
        "q": _pow2_scale(wq_), "k": _pow2_scale(wk_), "v": _pow2_scale(wv_),
        "p": _pow2_scale(proj_w), "qc": _pow2_scale(wqf), "kc": _pow2_scale(k_w),
        "vc": _pow2_scale(v_w), "o": _pow2_scale(out_w),
        "m1": _pow2_scale(wm1f), "m2": _pow2_scale(mlp2_w),
    }
    c8 = lambda w, s: np.ascontiguousarray((w * s).astype(F8NP))
    shared = {
        "wq8": c8(_tile_rows(wq_, DC, DC, 128), sc["q"]),
        "wk8": c8(_tile_rows(wk_, DC, DC, 128), sc["k"]),
        "wv8": c8(_tile_rows(wv_, 2, DC, 512), sc["v"]),
        "wp8": c8(_tile_rows(proj_w, DC, DC, 128), sc["p"]),
        "wqc8": c8(_tile_rows(wqf, DC, DC, 128), sc["qc"]),
        "wkc8": c8(_tile_rows(k_w, DC, EC, 128), sc["kc"]),
        "wvc8": c8(_tile_rows(v_w, 2, EC, 512), sc["vc"]),
        "wo8": c8(_tile_rows(out_w, DC, DC, 128), sc["o"]),
        "wm18": c8(_tile_rows(wm1f, MC, DC, 128), sc["m1"]),
        "wm28": c8(_tile_rows(mlp2_w, DC, MC, 128), sc["m2"]),
    }
    col = lambda v: np.ascontiguousarray(v.reshape(-1, 1).astype(f32))
    shared.update({
        "bq": col(b_q), "bproj": col(bprojf), "bqc": col(bqcf),
        "bout": col(boutf), "bm1": col(bm1f), "bm2": col(mlp2_b),
    })
    # encT pre-tiled fp8*A_X: [128, EC*S]
    encT = enc.transpose(0, 2, 1)  # [B, DE, S]

    # mask tiles [128, 3*128] bf16: [triT | flag | identity]
    # triT[i, j] = M[j, i] where M[key, q] = 0 if key<=q else NEG (same block)
    tri = np.where(np.arange(128)[:, None] <= np.arange(128)[None, :], 0.0, NEG)
    triT = tri.T.astype(BFNP)
    ident = np.eye(128, dtype=BFNP)

    in_maps, metas = [], []
    for c in range(8):
        b, h = c // 2, c % 2
        own_blocks = np.arange(h, 8, 2)
        other_blocks = np.arange(1 - h, 8, 2)
        own = (own_blocks[:, None] * 128 + np.arange(128)[None, :]).reshape(-1)
        other = (other_blocks[:, None] * 128 + np.arange(128)[None, :]).reshape(-1)
        perm = np.concatenate([own, other])
        xT_np = np.ascontiguousarray(x[b][perm].T)  # [D, T] own-first
        enc8 = np.ascontiguousarray(
            (encT[b].reshape(EC, 128, S).transpose(1, 0, 2).reshape(128, EC * S)
             * A_X).astype(F8NP))
        flag = np.full((128, 128), NEG if h == 0 else 0.0, dtype=BFNP)
        mask3 = np.ascontiguousarray(np.concatenate([triT, flag, ident], axis=1))
        im = dict(shared)
        im["xT"] = xT_np
        im["encT8"] = enc8
        im["mask3"] = mask3
        in_maps.append(im)
        metas.append((b, own_blocks))
    return in_maps, metas, sc


def _build_body(nc, tc, P, sc):
    from contextlib import ExitStack
    f32, f32r, bf16 = dt.float32, dt.float32r, dt.bfloat16
    ctx = ExitStack()
    with ctx:
        const = ctx.enter_context(tc.tile_pool(name="const", bufs=1))
        rows = ctx.enter_context(tc.tile_pool(name="rows", bufs=2))
        rows1 = ctx.enter_context(tc.tile_pool(name="rows1", bufs=2))
        bcp = ctx.enter_context(tc.tile_pool(name="bcp", bufs=2))
        sb_sm = ctx.enter_context(tc.tile_pool(name="sb_sm", bufs=3))
        ps = ctx.enter_context(tc.tile_pool(name="ps", bufs=2, space="PSUM"))
        resp = ctx.enter_context(tc.tile_pool(name="resp", bufs=1))

        ones32 = const.tile([128, 1], f32, tag="ones32")
        nc.vector.memset(ones32[:, :], 1.0)
        ones = const.tile([128, 1], f32r, tag="ones")
        nc.scalar.activation(ones[:, :], ones32[:, :], AF.Copy)
        ones_bf = const.tile([128, 1], bf16, tag="ones_bf")
        nc.vector.memset(ones_bf[:, :], 1.0)
        eps2 = const.tile([1, 1], f32, tag="eps2")
        nc.vector.memset(eps2[:, :], EPS / (A_X * A_X))
        ln8_t = const.tile([128, 1], f32, tag="ln8")
        nc.vector.memset(ln8_t[:, :], LN8)
        mask_t = const.tile([128, 384], bf16, tag="mask3")
        triT, flagT, ident = mask_t[:, 0:128], mask_t[:, 128:256], mask_t[:, 256:384]

        def load_bias(drh, nr, tag):
            t = const.tile([128, nr], f32, tag=tag, name="b_" + tag)
            nc.sync.dma_start(out=t.rearrange("p (r one) -> p r one", one=1),
                              in_=drh.rearrange("(r p) one -> p r one", p=128))
            return t

        def load_consts():
            nc.sync.dma_start(out=mask_t[:, :], in_=P["mask3"][:, :])
            return (load_bias(P["bq"], DC, "bq"), load_bias(P["bproj"], DC, "bproj"),
                    load_bias(P["bqc"], DC, "bqc"), load_bias(P["bout"], DC, "bout"),
                    load_bias(P["bm1"], MC, "bm1"), load_bias(P["bm2"], DC, "bm2"))

        # residual stream (bf16) + cross tensors spanning many phases
        x2T = resp.tile([128, DC * TQ], bf16, tag="x2T")
        x3T = resp.tile([128, DC * TQ], bf16, tag="x3T")
        qcT = resp.tile([128, DC * TQ], bf16, tag="qcT")
        kcT = resp.tile([128, DC * S], bf16, tag="kcT")
        vcext = resp.tile([128, 5 * H * 65], F8, tag="vcext")
        caT = resp.tile([128, DC * TQ], F8, tag="caT")

        # ---- LayerNorm helpers (transposed layout, stats via ones-matmul) ----
        def ln_chain(sts, stq, c0, W, rb_t, nb_t):
            R = rows1.tile([1, 2048], f32, tag="lnrow", bufs=1)
            t1n, t2 = R[0:1, 0:W], R[0:1, 512:512 + W]
            t5, msq = R[0:1, 1024:1024 + W], R[0:1, 1536:1536 + W]
            nc.vector.tensor_scalar_mul(t1n, sts[0:1, c0:c0 + W], -1.0 / D)
            nc.vector.tensor_scalar_mul(t2, stq[0:1, c0:c0 + W], 1.0 / D)
            nc.vector.tensor_mul(msq, t1n, t1n)
            nc.vector.tensor_sub(t2, t2, msq)
            nc.scalar.activation(t5, t2, AF.Abs_reciprocal_sqrt,
                                 bias=eps2[0:1, 0:1], scale=1.0 / (A_X * A_X))
            nc.vector.tensor_mul(t1n, t1n, t5)        # -mean * A_X * rstd
            nc.gpsimd.partition_broadcast(rb_t[:, 0:W], t5)
            nc.gpsimd.partition_broadcast(nb_t[:, 0:W], t1n)

        gp_mul = nc.vector.tensor_mul  # keep gpsimd broadcast-only (ucode lib swaps cost ~10us)

        def ln_norm_chunk(dst, xsrc, rb_t, nb_t, W):
            xs = rows.tile([128, 512], f32, tag="xs")
            gp_mul(xs[:, 0:W], xsrc, rb_t[:, 0:W])
            nc.vector.tensor_add(dst, xs[:, 0:W], nb_t[:, 0:W])

        # =====================  phase 1-8: LN1, self-attn, proj, LN2, qc  ====
        with tc.tile_pool(name="xtp", bufs=1) as xtp, \
             tc.tile_pool(name="sfp", bufs=1) as sfp:
            xT_t = xtp.tile([128, DC * T], f32r, tag="xT")
            for kc in range(DC):
                nc.sync.dma_start(out=xT_t[:, kc * T:(kc + 1) * T],
                                  in_=P["xT"][kc * 128:(kc + 1) * 128, :])
            bq_t, bp_t, bqc_t, bo_t, bm1_t, bm2_t = load_consts()
            xhat1 = sfp.tile([128, DC * T], F8, tag="xhat1")
            kT = sfp.tile([128, DC * T], bf16, tag="kT")
            qT = sfp.tile([128, DC * TQ], bf16, tag="qT")
            vext = sfp.tile([128, 8 * H * 65], F8, tag="vext")
            saT = sfp.tile([128, DC * TQ], F8, tag="saT")

            nc.vector.memset(
                vext.rearrange("p (c e) -> p c e", e=65)[:, :, 64:65], 1.0)
            nc.vector.memset(vcext[64:128, 4 * H * 65:5 * H * 65], 0.0)
            nc.vector.memset(
                vcext.rearrange("p (c e) -> p c e", e=65)[:, 0:4 * H, 64:65], 1.0)
            nc.vector.memset(
                vcext.rearrange("p (c e) -> p c e", e=65)[0:64, 4 * H:5 * H, 64:65], 1.0)

            xh3 = xhat1.rearrange("p (kc t) -> p kc t", t=T)

            with tc.tile_pool(name="ckA", bufs=1) as ckA:
                encT_t = ckA.tile([128, EC * S], F8, tag="encT")
                nc.sync.dma_start(out=encT_t[:, :], in_=P["encT8"][:, :])
                wkcall = ckA.tile([128, DC * EC * 128], F8, tag="wkc")
                nc.sync.dma_start(out=wkcall[:, :], in_=P["wkc8"][:, :])
                wvcall = ckA.tile([128, 2 * EC * 512], F8, tag="wvc")
                nc.sync.dma_start(out=wvcall[:, :], in_=P["wvc8"][:, :])
                encv = encT_t.rearrange("p (ec s) -> p ec s", s=S)

                def emit_kc_unit(r, et):
                    pt = ps.tile([128, 512], f32, tag="mm")
                    wv_ = wkcall.rearrange("p (r ec m) -> p r ec m", r=DC, m=128)
                    for i in range(EC // 2):
                        nc.tensor.matmul(pt[:, 0:288], wv_[:, r, 2 * i:2 * i + 2, :],
                                         encv[:, 2 * i:2 * i + 2, et * 288:(et + 1) * 288],
                                         start=(i == 0), stop=(i == 2),
                                         perf_mode=PM.DoubleRow)
                    nc.scalar.activation(kcT[:, r * S + et * 288: r * S + (et + 1) * 288],
                                         pt[:, 0:288], AF.Identity, bias=0.0,
                                         scale=1.0 / (A_X * sc["kc"]))

                kc_units = [(r, et) for r in range(DC) for et in range(2)]
                vxv = vext.rearrange("p (tk j e) -> p tk j e", tk=8, j=H)
                wvcv = wvcall.rearrange("p (vf ec m) -> p vf ec m", vf=2, m=512)
                vcxv = vcext.rearrange("p (tk j e) -> p tk j e", tk=5, j=H)

                def emit_vc_unit(vf, tokc):
                    npart = 128 if tokc < 4 else 64
                    pv = ps.tile([128, 512], f32, tag="mm")
                    for i in range(EC // 2):
                        nc.tensor.matmul(pv[:npart, :],
                                         encv[:, 2 * i:2 * i + 2, tokc * 128:tokc * 128 + npart],
                                         wvcv[:, vf, 2 * i:2 * i + 2, :],
                                         start=(i == 0), stop=(i == 2),
                                         perf_mode=PM.DoubleRow)
                    nc.vector.tensor_scalar_mul(
                        vcxv[:npart, tokc, 8 * vf:8 * vf + 8, 0:64],
                        pv[:npart].rearrange("p (j d) -> p j d", j=8), 1.0 / sc["vc"])


                # ---- LN1 stats (chasing x DMA) with cross-kc interleaved ----
                with tc.tile_pool(name="wqks", bufs=4) as wqks, \
                     tc.tile_pool(name="pst", bufs=1, space="PSUM") as pst:
                    st = {nm: pst.tile([128, 512], f32, tag="st_" + nm,
                                       name="st_" + nm)
                          for nm in ("s0", "s1", "q0", "q1")}
                    for kc in range(DC):
                        for _ in range(2):
                            if kc_units:
                                emit_kc_unit(*kc_units.pop(0))
                        for tt in range(2):
                            sq = sb_sm.tile([128, 512], f32r, tag="sq2")
                            nc.scalar.activation(
                                sq[:, :],
                                xT_t[:, kc * T + tt * 512: kc * T + tt * 512 + 512],
                                AF.Square)
                            nc.tensor.matmul(st["s%d" % tt][0:1, :], ones[:, :],
                                             xT_t[:, kc * T + tt * 512: kc * T + tt * 512 + 512],
                                             start=(kc == 0), stop=(kc == DC - 1),
                                             skip_group_check=True)
                            nc.tensor.matmul(st["q%d" % tt][0:1, :], ones[:, :],
                                             sq[:, :],
                                             start=(kc == 0), stop=(kc == DC - 1),
                                             skip_group_check=True)
                    # ---- LN1 tt0 chain + norm, q/k rows chase ----
                    rb0 = bcp.tile([128, 512], f32, tag="rb")
                    nb0 = bcp.tile([128, 512], f32, tag="nb")
                    ln_chain(st["s0"], st["q0"], 0, 512, rb0, nb0)
                    for kc in range(DC):
                        ln_norm_chunk(xhat1[:, kc * T: kc * T + 512],
                                      xT_t[:, kc * T: kc * T + 512], rb0, nb0, 512)
                    while kc_units:
                        emit_kc_unit(*kc_units.pop(0))
                    vc_units = [(vf, tokc) for vf in range(2) for tokc in range(5)]
                    for _ in range(4):
                        emit_vc_unit(*vc_units.pop(0))

                    def qk_row(wdram, r, dst, bias, sscale, tcols):
                        wt = wqks.tile([128, DC * 128], F8, tag="wr")
                        nc.sync.dma_start(out=wt[:, :],
                                          in_=wdram[:, r * DC * 128:(r + 1) * DC * 128])
                        wv_ = wt.rearrange("p (kc m) -> p kc m", m=128)
                        pt = ps.tile([128, 512], f32, tag="mm")
                        for i in range(4):
                            nc.tensor.matmul(pt[:, :], wv_[:, 2 * i:2 * i + 2, :],
                                             xh3[:, 2 * i:2 * i + 2, tcols:tcols + 512],
                                             start=(i == 0), stop=(i == 3),
                                             perf_mode=PM.DoubleRow)
                        if bias is None:
                            nc.vector.tensor_scalar_mul(dst, pt[:, :], sscale)
                        else:
                            nc.scalar.activation(dst, pt[:, :], AF.Identity,
                                                 bias=bias, scale=sscale)

                    for r in range(DC):
                        qk_row(P["wq8"], r, qT[:, r * TQ:(r + 1) * TQ],
                               bq_t[:, r:r + 1], 1.0 / (A_X * sc["q"]), 0)
                        qk_row(P["wk8"], r, kT[:, r * T: r * T + 512],
                               None, 1.0 / (A_X * sc["k"]), 0)

                    # ---- LN1 tt1 chain + norm, k-tt1 rows chase ----
                    rb1 = bcp.tile([128, 512], f32, tag="rb")
                    nb1 = bcp.tile([128, 512], f32, tag="nb")
                    ln_chain(st["s1"], st["q1"], 0, 512, rb1, nb1)
                    for kc in range(DC):
                        ln_norm_chunk(xhat1[:, kc * T + 512: kc * T + 1024],
                                      xT_t[:, kc * T + 512: kc * T + 1024], rb1, nb1, 512)
                    for r in range(DC):
                        qk_row(P["wk8"], r, kT[:, r * T + 512: r * T + 1024],
                               None, 1.0 / (A_X * sc["k"]), 512)

                # ---- V units + self-attention heads ----

                with tc.tile_pool(name="wvp", bufs=1) as wvp:
                    wvall = wvp.tile([128, 2 * DC * 512], F8, tag="wv")
                    nc.sync.dma_start(out=wvall[:, :], in_=P["wv8"][:, :])
                    wvv = wvall.rearrange("p (vf kc m) -> p vf kc m", vf=2, m=512)

                    def emit_v_unit(vf, tokc):
                        pv = ps.tile([128, 512], f32, tag="mm")
                        for i in range(4):
                            nc.tensor.matmul(pv[:, :],
                                             xh3[:, 2 * i:2 * i + 2,
                                                 tokc * 128:(tokc + 1) * 128],
                                             wvv[:, vf, 2 * i:2 * i + 2, :],
                                             start=(i == 0), stop=(i == 3),
                                             perf_mode=PM.DoubleRow)
                        nc.vector.tensor_scalar_mul(
                            vxv[:, tokc, 8 * vf:8 * vf + 8, 0:64],
                            pv.rearrange("p (j d) -> p j d", j=8), 1.0 / sc["v"])

                    for tokc in range(8):
                        emit_v_unit(0, tokc)

                    NOFF = [0, 1024, 1792, 2304]
                    vex5 = vext.rearrange("p (g c je) -> p g c je", g=2, c=4)

                    with tc.tile_pool(name="pp", bufs=2) as pp, \
                         tc.tile_pool(name="pssc", bufs=2, space="PSUM") as pssc, \
                         tc.tile_pool(name="ps2", bufs=2, space="PSUM") as ps2:
                        def self_head(h):
                            hp, hc = (h % 2) * 64, h // 2
                            Pt = pp.tile([128, 2560], F8, tag="P")
                            av = ps2.tile([65, 512], f32, tag="av")
                            pend = None
                            for j in range(4):
                                N = 512 - 128 * j
                                sps = pssc.tile([128, 1024], f32, tag="sc")
                                nc.tensor.matmul(
                                    sps[:, 0:N],
                                    kT[hp:hp + 64, hc * T + j * 128: hc * T + j * 128 + 128],
                                    qT[hp:hp + 64, hc * TQ + j * 128: hc * TQ + TQ],
                                    start=True, stop=False, skip_group_check=True)
                                nc.tensor.matmul(
                                    sps[:, 512:512 + N],
                                    kT[hp:hp + 64, hc * T + 512 + j * 128: hc * T + 512 + j * 128 + 128],
                                    qT[hp:hp + 64, hc * TQ + j * 128: hc * TQ + TQ],
                                    start=True, stop=False, skip_group_check=True)
                                nc.tensor.matmul(sps[:, 0:128], triT, ident,
                                                 start=False, stop=True,
                                                 skip_group_check=True)
                                nc.tensor.matmul(sps[:, 512:640], flagT, ident,
                                                 start=False, stop=True,
                                                 skip_group_check=True)
                                if pend is not None:
                                    jp, Np = pend
                                    nc.tensor.matmul(
                                        av[:, 128 * jp:512],
                                        vex5[:, :, jp, h * 65:(h + 1) * 65],
                                        Pt[:, NOFF[jp]:NOFF[jp] + 2 * Np]
                                        .rearrange("p (two n) -> p two n", two=2),
                                        start=(jp == 0), stop=False,
                                        perf_mode=PM.DoubleRow, skip_group_check=True)
                                nc.scalar.activation(
                                    Pt[:, NOFF[j]:NOFF[j] + 2 * N]
                                    .rearrange("p (two n) -> p two n", two=2),
                                    sps.rearrange("p (two n) -> p two n", two=2)[:, :, 0:N],
                                    AF.Exp, bias=ln8_t[:, 0:1], scale=SCALE)
                                pend = (j, N)
                            jp, Np = pend
                            nc.tensor.matmul(
                                av[:, 128 * jp:512],
                                vex5[:, :, jp, h * 65:(h + 1) * 65],
                                Pt[:, NOFF[jp]:NOFF[jp] + 2 * Np]
                                .rearrange("p (two n) -> p two n", two=2),
                                start=False, stop=True,
                                perf_mode=PM.DoubleRow, skip_group_check=True)
                            # epilogue: saT = av_rows * (1/den)  (= 16*sa in fp8)
                            den_sb = rows1.tile([1, 512], f32, tag="densb")
                            nc.scalar.activation(den_sb[:, :], av[64:65, :],
                                                 AF.Identity, bias=0.0, scale=1.0)
                            rrow = rows1.tile([1, 512], f32, tag="rrow")
                            nc.vector.reciprocal_approx_fast(rrow[:, :], den_sb[:, :])
                            rb64 = rows.tile([64, 512], f32, tag="rb64")
                            nc.gpsimd.partition_broadcast(rb64[:, :], rrow[:, :])
                            nc.vector.tensor_mul(saT[hp:hp + 64, hc * TQ:(hc + 1) * TQ],
                                                 av[0:64, :], rb64[:, :])

                        for h in range(H):
                            self_head(h)
                            if h < 8:
                                emit_v_unit(1, h)
                            elif vc_units:
                                emit_vc_unit(*vc_units.pop(0))
                        while vc_units:
                            emit_vc_unit(*vc_units.pop(0))

            # ---- proj + residual -> x2T, LN2 stats chase, LN2 + qc ----
            with tc.tile_pool(name="pstL", bufs=1, space="PSUM") as pstL:
                st2s = pstL.tile([128, 512], f32, tag="st2s")
                st2q = pstL.tile([128, 512], f32, tag="st2q")
                with tc.tile_pool(name="wpp", bufs=1) as wpp:
                    wpall = wpp.tile([128, DC * DC * 128], F8, tag="wpj")
                    nc.sync.dma_start(out=wpall[:, :], in_=P["wp8"][:, :])
                    sa3 = saT.rearrange("p (c t) -> p c t", t=TQ)
                    wpv = wpall.rearrange("p (r kc m) -> p r kc m", r=DC, m=128)
                    for r in range(DC):
                        pt = ps.tile([128, 512], f32, tag="mm")
                        for i in range(4):
                            nc.tensor.matmul(pt[:, :], wpv[:, r, 2 * i:2 * i + 2, :],
                                             sa3[:, 2 * i:2 * i + 2, :],
                                             start=(i == 0), stop=(i == 3),
                                             perf_mode=PM.DoubleRow)
                        t1 = sb_sm.tile([128, 512], f32, tag="drain")
                        nc.scalar.activation(t1[:, :], pt[:, :], AF.Identity,
                                             bias=bp_t[:, r:r + 1],
                                             scale=1.0 / (A_X * sc["p"]))
                        nc.vector.tensor_add(x2T[:, r * TQ:(r + 1) * TQ], t1[:, :],
                                             xT_t[:, r * T: r * T + TQ].bitcast(f32))
                        sq = sb_sm.tile([128, 512], bf16, tag="sqb")
                        nc.scalar.activation(sq[:, :], x2T[:, r * TQ:(r + 1) * TQ],
                                             AF.Square)
                        nc.tensor.matmul(st2s[0:1, :], ones_bf[:, :],
                                         x2T[:, r * TQ:(r + 1) * TQ],
                                         start=(r == 0), stop=(r == DC - 1),
                                         skip_group_check=True)
                        nc.tensor.matmul(st2q[0:1, :], ones_bf[:, :], sq[:, :],
                                         start=(r == 0), stop=(r == DC - 1),
                                         skip_group_check=True)

                # ---- LN2 (2 blocks) + qc rows chase ----
                with tc.tile_pool(name="qcp", bufs=1) as qcp:
                    x2hat = qcp.tile([128, DC * TQ], F8, tag="x2hat")
                    wqcall = qcp.tile([128, DC * DC * 128], F8, tag="wqc")
                    nc.sync.dma_start(out=wqcall[:, :], in_=P["wqc8"][:, :])
                    x2h3 = x2hat.rearrange("p (kc t) -> p kc t", t=TQ)
                    wqcv = wqcall.rearrange("p (r kc m) -> p r kc m", r=DC, m=128)
                    for blk in range(2):
                        c0 = blk * 256
                        rbb = bcp.tile([128, 512], f32, tag="rb")
                        nbb = bcp.tile([128, 512], f32, tag="nb")
                        ln_chain(st2s, st2q, c0, 256, rbb, nbb)
                        for kc in range(DC):
                            ln_norm_chunk(x2hat[:, kc * TQ + c0: kc * TQ + c0 + 256],
                                          x2T[:, kc * TQ + c0: kc * TQ + c0 + 256],
                                          rbb, nbb, 256)
                        for r in range(DC):
                            pt = ps.tile([128, 512], f32, tag="mm")
                            for i in range(4):
                                nc.tensor.matmul(pt[:, 0:256],
                                                 wqcv[:, r, 2 * i:2 * i + 2, :],
                                                 x2h3[:, 2 * i:2 * i + 2, c0:c0 + 256],
                                                 start=(i == 0), stop=(i == 3),
                                                 perf_mode=PM.DoubleRow)
                            nc.scalar.activation(qcT[:, r * TQ + c0: r * TQ + c0 + 256],
                                                 pt[:, 0:256], AF.Identity,
                                                 bias=bqc_t[:, r:r + 1],
                                                 scale=1.0 / (A_X * sc["qc"]))
        # xtp/sfp freed here

        # =====================  phase 9-12: cross-attn, out, MLP  ============
        with tc.tile_pool(name="mlpp", bufs=1) as mlpp:
            x3hat = mlpp.tile([128, DC * TQ], F8, tag="x3hat")
            hT = mlpp.tile([128, MC * TQ], F8, tag="hT")
            with tc.tile_pool(name="m1wp", bufs=1) as m1wp:
                wm1all = m1wp.tile([128, MC * DC * 128], F8, tag="wm1")
                nc.sync.dma_start(out=wm1all[:, :], in_=P["wm18"][:, :])
                vcx5 = vcext.rearrange("p (c je) -> p c je", c=5)

                with tc.tile_pool(name="ppc", bufs=2) as ppc, \
                     tc.tile_pool(name="pssc2", bufs=2, space="PSUM") as pssc2, \
                     tc.tile_pool(name="ps2b", bufs=2, space="PSUM") as ps2b:
                    def cross_head(h):
                        hp, hc = (h % 2) * 64, h // 2
                        Pc = ppc.tile([128, 2560], F8, tag="Pc")
                        av = ps2b.tile([65, 512], f32, tag="av")
                        for g in range(2):
                            sps = pssc2.tile([128, 1024], f32, tag="sc")
                            for jj in range(2):
                                c = 2 * g + jj
                                nc.tensor.matmul(
                                    sps[:, jj * 512:(jj + 1) * 512],
                                    kcT[hp:hp + 64, hc * S + c * 128: hc * S + c * 128 + 128],
                                    qcT[hp:hp + 64, hc * TQ:(hc + 1) * TQ],
                                    start=True, stop=True, skip_group_check=True)
                            if g == 1:
                                nc.tensor.matmul(av[:, :],
                                                 vcx5[:, 0:2, h * 65:(h + 1) * 65],
                                                 Pc[:, 0:1024]
                                                 .rearrange("p (two n) -> p two n", two=2),
                                                 start=True, stop=False,
                                                 perf_mode=PM.DoubleRow,
                                                 skip_group_check=True)
                            nc.scalar.activation(Pc[:, g * 1024:(g + 1) * 1024],
                                                 sps[:, :], AF.Exp,
                                                 bias=ln8_t[:, 0:1], scale=SCALE)
                        sps4 = pssc2.tile([128, 1024], f32, tag="sc")
                        nc.tensor.matmul(sps4[0:64, 0:512],
                                         kcT[hp:hp + 64, hc * S + 512: hc * S + 576],
                                         qcT[hp:hp + 64, hc * TQ:(hc + 1) * TQ],
                                         start=True, stop=True, skip_group_check=True)
                        nc.tensor.matmul(av[:, :], vcx5[:, 2:4, h * 65:(h + 1) * 65],
                                         Pc[:, 1024:2048]
                                         .rearrange("p (two n) -> p two n", two=2),
                                         start=False, stop=False,
                                         perf_mode=PM.DoubleRow, skip_group_check=True)
                        nc.scalar.activation(Pc[0:64, 2048:2560], sps4[0:64, 0:512],
                                             AF.Exp, bias=ln8_t[0:64, 0:1], scale=SCALE)
                        nc.tensor.matmul(av[:, :], vcx5[0:64, 4, h * 65:(h + 1) * 65],
                                         Pc[0:64, 2048:2560],
                                         start=False, stop=True, skip_group_check=True)
                        den_sb = rows1.tile([1, 512], f32, tag="densb")
                        nc.vector.tensor_copy(den_sb[:, :], av[64:65, :])
                        rrow = rows1.tile([1, 512], f32, tag="rrow")
                        nc.vector.reciprocal_approx_fast(rrow[:, :], den_sb[:, :])
                        rb64 = rows.tile([64, 512], f32, tag="rb64")
                        nc.gpsimd.partition_broadcast(rb64[:, :], rrow[:, :])
                        nc.vector.tensor_mul(caT[hp:hp + 64, hc * TQ:(hc + 1) * TQ],
                                             av[0:64, :], rb64[:, :])

                    for h in range(H):
                        cross_head(h)

                # ---- out proj + residual -> x3T, LN3 stats chase ----
                with tc.tile_pool(name="pstM", bufs=1, space="PSUM") as pstM:
                    st3s = pstM.tile([128, 512], f32, tag="st3s")
                    st3q = pstM.tile([128, 512], f32, tag="st3q")
                    ca3 = caT.rearrange("p (c t) -> p c t", t=TQ)
                    with tc.tile_pool(name="wos", bufs=3) as wos:
                        for r in range(DC):
                            wt = wos.tile([128, DC * 128], F8, tag="wor")
                            nc.sync.dma_start(
                                out=wt[:, :],
                                in_=P["wo8"][:, r * DC * 128:(r + 1) * DC * 128])
                            wv_ = wt.rearrange("p (kc m) -> p kc m", m=128)
                            pt = ps.tile([128, 512], f32, tag="mm")
                            for i in range(4):
                                nc.tensor.matmul(pt[:, :], wv_[:, 2 * i:2 * i + 2, :],
                                                 ca3[:, 2 * i:2 * i + 2, :],
                                                 start=(i == 0), stop=(i == 3),
                                                 perf_mode=PM.DoubleRow)
                            t1 = sb_sm.tile([128, 512], f32, tag="drain")
                            nc.scalar.activation(t1[:, :], pt[:, :], AF.Identity,
                                                 bias=bo_t[:, r:r + 1],
                                                 scale=1.0 / (A_X * sc["o"]))
                            nc.vector.tensor_add(x3T[:, r * TQ:(r + 1) * TQ], t1[:, :],
                                                 x2T[:, r * TQ:(r + 1) * TQ])
                            sq = sb_sm.tile([128, 512], bf16, tag="sqb")
                            nc.scalar.activation(sq[:, :], x3T[:, r * TQ:(r + 1) * TQ],
                                                 AF.Square)
                            nc.tensor.matmul(st3s[0:1, :], ones_bf[:, :],
                                             x3T[:, r * TQ:(r + 1) * TQ],
                                             start=(r == 0), stop=(r == DC - 1),
                                             skip_group_check=True)
                            nc.tensor.matmul(st3q[0:1, :], ones_bf[:, :], sq[:, :],
                                             start=(r == 0), stop=(r == DC - 1),
                                             skip_group_check=True)

                    # ---- LN3 (2 blocks) + mlp1 rows chase ----
                    x3h3 = x3hat.rearrange("p (kc t) -> p kc t", t=TQ)
                    wm1v = wm1all.rearrange("p (r kc m) -> p r kc m", r=MC, m=128)
                    rbb = bcp.tile([128, 512], f32, tag="rb")
                    nbb = bcp.tile([128, 512], f32, tag="nb")
                    ln_chain(st3s, st3q, 0, 512, rbb, nbb)
                    for kc in range(DC):
                        ln_norm_chunk(x3hat[:, kc * TQ: (kc + 1) * TQ],
                                      x3T[:, kc * TQ: (kc + 1) * TQ],
                                      rbb, nbb, 512)
                    for r in range(MC):
                        pt = ps.tile([128, 512], f32, tag="mm")
                        for i in range(4):
                            nc.tensor.matmul(pt[:, :],
                                             wm1v[:, r, 2 * i:2 * i + 2, :],
                                             x3h3[:, 2 * i:2 * i + 2, :],
                                             start=(i == 0), stop=(i == 3),
                                             perf_mode=PM.DoubleRow)
                        nc.scalar.activation(hT[:, r * TQ: (r + 1) * TQ],
                                             pt[:, :], AF.Gelu,
                                             bias=bm1_t[:, r:r + 1],
                                             scale=1.0 / (A_X * sc["m1"]))

            # ---- mlp2 + residual -> yT (streamed weights) ----
            hT3 = hT.rearrange("p (kc t) -> p kc t", t=TQ)
            with tc.tile_pool(name="wm2s", bufs=3) as wm2s:
                for r in range(DC):
                    wt = wm2s.tile([128, MC * 128], F8, tag="wm2r")
                    nc.sync.dma_start(
                        out=wt[:, :],
                        in_=P["wm28"][:, r * MC * 128:(r + 1) * MC * 128])
                    wv_ = wt.rearrange("p (kc m) -> p kc m", m=128)
                    pt = ps.tile([128, 512], f32, tag="mm")
                    for i in range(MC // 2):
                        nc.tensor.matmul(pt[:, :], wv_[:, 2 * i:2 * i + 2, :],
                                         hT3[:, 2 * i:2 * i + 2, :],
                                         start=(i == 0), stop=(i == MC // 2 - 1),
                                         perf_mode=PM.DoubleRow)
                    t1 = sb_sm.tile([128, 512], f32, tag="drain")
                    nc.scalar.activation(t1[:, :], pt[:, :], AF.Identity,
                                         bias=bm2_t[:, r:r + 1], scale=1.0 / sc["m2"])
                    yt = sb_sm.tile([128, 512], f32, tag="drain")
                    nc.vector.tensor_add(yt[:, :], t1[:, :], x3T[:, r * TQ:(r + 1) * TQ])
                    nc.sync.dma_start(out=P["yT"][r * 128:(r + 1) * 128, :], in_=yt[:, :])


def _build_program(sc):
    nc = bacc.Bacc()
    P = {}
    P["xT"] = nc.declare_dram_parameter("xT", [D, T], dt.float32r, isOutput=False)
    P["encT8"] = nc.declare_dram_parameter("encT8", [128, EC * S], F8, isOutput=False)
    P["mask3"] = nc.declare_dram_parameter("mask3", [128, 3 * 128], dt.bfloat16, isOutput=False)
    for nm, shp in [("wq8", DC * DC * 128), ("wk8", DC * DC * 128),
                    ("wv8", 2 * DC * 512), ("wp8", DC * DC * 128),
                    ("wqc8", DC * DC * 128), ("wkc8", DC * EC * 128),
                    ("wvc8", 2 * EC * 512), ("wo8", DC * DC * 128),
                    ("wm18", MC * DC * 128), ("wm28", DC * MC * 128)]:
        P[nm] = nc.declare_dram_parameter(nm, [128, shp], F8, isOutput=False)
    for nm, n in [("bq", D), ("bproj", D), ("bqc", D), ("bout", D),
                  ("bm1", DM), ("bm2", D)]:
        P[nm] = nc.declare_dram_parameter(nm, [n, 1], dt.float32, isOutput=False)
    P["yT"] = nc.declare_dram_parameter("yT", [D, TQ], dt.float32, isOutput=True)

    with tile.TileContext(nc) as tc:
        _build_body(nc, tc, P, sc)
    nc.compile()
    return nc


def _get_program(sc):
    if "nc" not in _cached:
        _cached["nc"] = _build_program(sc)
    return _cached["nc"]


last_result = None


def kernel(**inputs):
    global last_result
    import os
    trace = bool(os.environ.get("KERNEL_TRACE"))
    in_maps, metas, sc = _prepare_inputs(**inputs)
    nc = _get_program(sc)
    res = run_bass_kernel_spmd(nc, in_maps, list(range(8)), trace=trace)
    last_result = res
    out = np.empty((B, T, D), dtype=np.float32)
    for c, (b, own_blocks) in enumerate(metas):
        yTc = res.results[c]["yT"]            # [D, TQ]
        yt = yTc.T.reshape(4, 128, D)
        for i, blk in enumerate(own_blocks):
            out[b, blk * 128:(blk + 1) * 128, :] = yt[i]
    return out
